# revision 1
# baseline (speedup 1.0000x reference)
"""GCN (2-layer, edge-weighted, log_softmax) on 8 Trainium2 NeuronCores.

Strategy (dst-sharded edges, matmul-based segment-sum):
  - Nodes sharded 12544/core (table rows = node ids, 352 junk tail rows).
  - Layer 1: h = x @ W1 computed data-parallel on node shards -> AllGather
    into a replicated, 256B-strided feature table in HBM.
  - Per-edge gather of 128B rows via the InstDMAGatherAnt SWDGE ucode
    (int16 idx => 4 table chunks of 25088 rows; edges grouped by src chunk).
  - Edges packed into 8-edge same-destination slots; DVE does x-weight and
    an 8->1 tree reduction; a per-column one-hot (is_equal vs iota) matmul
    segment-sums slot partials into PSUM windows of 128 destinations,
    accumulated into an SBUF aggregate laid out [d%128, (d//128)*32+f].
  - Layer 2 aggregates relu(agg1 + b1) with the identical edge structure,
    then applies W2 (+b2 via a ones-row matmul) per 128-node window,
    followed by an on-chip log_softmax.
Host side only packs indices/weights (numpy) and concatenates shards.
"""

import os
import sys

for _p in ("/opt/trn_rl_repo", "/root/.axon_site/_ro/trn_rl_repo"):
    if os.path.isdir(_p) and _p not in sys.path:
        sys.path.insert(0, _p)

import numpy as np

import concourse.ap_utils as ap_utils
import concourse.bass as bass
import concourse.mybir as mybir
from concourse import bacc, tile
from concourse.bass_utils import run_bass_kernel_spmd

CORES = 8
F_IN = 128
F_HID = 32
F_OUT = 40
KSLOT = 8  # edges per slot (same destination)
GK_INSTR = 64  # gather k-columns per instruction (8192 tokens)


class Geo:
    """Problem geometry. Full size by default; shrinkable for simulation."""

    def __init__(self, n_nodes=100000, nsh=12544, chunk=25088, groups=4):
        self.n_nodes = n_nodes
        self.nsh = nsh  # nodes per core shard (mult of 128)
        self.ntab = nsh * CORES  # table rows
        self.chunk = chunk  # gather table chunk rows (<= 32768)
        self.groups = groups
        assert chunk * groups == self.ntab
        assert nsh % 128 == 0
        self.nwin = nsh // 128  # 128-destination windows per core


FULL = Geo()


def _wrap16(flat, T):
    """token i -> [i%16, i//16], replicated to 128 partitions."""
    a = flat.reshape(T // 16, 16).T
    return np.tile(a, (8, 1)).copy()


def pack(edge_index, edge_weight, geo: Geo):
    """Group edges by (core, src-chunk, dst) into 8-edge slots; build the
    shared column->window template and all per-core device arrays."""
    src = np.asarray(edge_index[0], dtype=np.int64)
    dst = np.asarray(edge_index[1], dtype=np.int64)
    w = np.asarray(edge_weight, dtype=np.float32)
    nsh, nwin, G = geo.nsh, geo.nwin, geo.groups

    core = dst // nsh
    pc = []
    cnt = np.zeros((CORES, G, nwin), np.int64)
    for c in range(CORES):
        m = core == c
        s_c = src[m]
        dl = dst[m] - c * nsh
        wc = w[m]
        g = s_c // geo.chunk
        key = g * (2 * nsh) + dl
        order = np.argsort(key, kind="stable")
        sk = (s_c - g * geo.chunk)[order]
        dlk = dl[order]
        wk = wc[order]
        kk = key[order]
        new = np.r_[True, kk[1:] != kk[:-1]]
        run_first = np.flatnonzero(new)
        run_len = np.diff(np.r_[run_first, len(kk)])
        run_id = np.cumsum(new) - 1
        rank = np.arange(len(kk)) - run_first[run_id]
        nsl = (run_len + KSLOT - 1) // KSLOT
        g_run = (g[order])[run_first]
        dl_run = dlk[run_first]
        v_run = dl_run // 128
        np.add.at(cnt[c], (g_run, v_run), nsl)
        pc.append((sk, dlk, wk, rank, run_id, nsl, g_run, dl_run, v_run))

    # shared template
    cap = cnt.max(axis=0)  # [G, nwin]
    cap = ((cap + 31) // 32) * 32
    gslots = cap.sum(axis=1)
    gpad = (-gslots) % 128
    off = np.zeros((G, nwin), np.int64)
    gbase = np.zeros(G + 1, np.int64)
    b = 0
    for g in range(G):
        gbase[g] = b
        for v in range(nwin):
            off[g, v] = b
            b += cap[g, v]
        b += gpad[g]
    gbase[G] = b
    S_T = int(b)
    COLS = S_T // 128
    T = S_T * KSLOT
    KC = T // 128  # k-columns total

    # per-column window lists (template)
    colmeta = []  # (A_col, [wins]) or None
    flat_off = []
    flat_gv = []
    for g in range(G):
        for v in range(nwin):
            if cap[g, v] > 0:
                flat_off.append(int(off[g, v]))
                flat_gv.append((g, v))
    flat_off = np.array(flat_off + [S_T], dtype=np.int64)
    for col in range(COLS):
        lo, hi = col * 128, col * 128 + 128
        i0 = int(np.searchsorted(flat_off, lo, side="right") - 1)
        wins = []
        gcol = None
        for i in range(max(i0, 0), len(flat_gv)):
            o = flat_off[i]
            if o >= hi:
                break
            o2 = flat_off[i + 1]
            if o2 <= lo:
                continue
            gg, vv = flat_gv[i]
            if gcol is None:
                gcol = gg
            if gg == gcol:
                wins.append(vv)
        # group-tail pad regions have no (g,v); wins may be empty
        colmeta.append((wins[0], wins, gcol) if wins else None)

    # per-group k-column ranges for gather instructions
    ginstr = []  # (group, kc0, gk)
    for g in range(G):
        kc_lo = int(gbase[g]) // 16
        kc_hi = int(gbase[g + 1]) // 16
        kc = kc_lo
        while kc < kc_hi:
            gk = min(GK_INSTR, kc_hi - kc)
            ginstr.append((g, kc, gk))
            kc += gk

    # per-core arrays
    inmaps = []
    for c in range(CORES):
        sk, dlk, wk, rank, run_id, nsl, g_run, dl_run, v_run = pc[c]
        n_runs = len(nsl)
        csum = np.cumsum(nsl)
        start_excl = np.r_[0, csum[:-1]]
        gv = g_run * nwin + v_run
        newgv = np.r_[True, gv[1:] != gv[:-1]]
        gv_first = np.flatnonzero(newgv)
        gv_id = np.cumsum(newgv) - 1
        base_in_gv = start_excl - start_excl[gv_first][gv_id]
        run_slot = off[g_run, v_run] + base_in_gv
        slot_e = run_slot[run_id] + rank // KSLOT
        j_e = rank % KSLOT
        cs_e = slot_e // 128
        p_e = slot_e % 128
        tok = (cs_e * KSLOT + j_e) * 128 + p_e

        idx_flat = np.zeros(T, np.int16)
        idx_flat[tok] = sk.astype(np.int16)
        w_flat = np.zeros(T, np.float32)
        w_flat[tok] = wk

        dl_slot = np.full(S_T, 100000.0, np.float32)
        reps = np.repeat(np.arange(n_runs), nsl)
        ar = np.arange(len(reps)) - np.repeat(start_excl, nsl)
        pos = np.repeat(run_slot, nsl) + ar
        dl_slot[pos] = np.repeat(dl_run, nsl).astype(np.float32)
        dcol = dl_slot.reshape(COLS, 128).T.copy()  # [128, COLS]
        for col in range(COLS):
            if colmeta[col] is not None:
                dcol[:, col] -= 128.0 * colmeta[col][0]

        inmaps.append(
            dict(
                gidx=_wrap16(idx_flat, T),
                wgrid=w_flat.reshape(KC, 128).T.copy(),
                dloc=dcol,
            )
        )

    meta = dict(S_T=S_T, COLS=COLS, T=T, KC=KC, colmeta=colmeta, ginstr=ginstr, geo=geo)
    return meta, inmaps


def emit_dma_gather(gp, out_ap, in_ap, idxs_ap, num_idxs, elem_size, elem_step):
    """bass.dma_gather minus the blanket 256B elem assert (verified on HW that
    the non-transpose ucode handles 128B rows)."""
    from concourse.bass import exact_div

    assert idxs_ap.dtype == mybir.dt.int16
    assert in_ap.dtype == out_ap.dtype
    assert in_ap.space == bass.MemorySpace.DRAM
    assert ap_utils.ap_is_contiguous(in_ap.ap[1:])
    assert ap_utils.ap_is_contiguous(out_ap.ap[1:])
    assert ap_utils.ap_is_contiguous(idxs_ap.ap[1:])
    assert in_ap.ap[-1][1] == out_ap.ap[-1][1] == elem_size
    assert out_ap.ap[0][1] * out_ap.ap[1][1] == num_idxs
    assert in_ap.ap[0][0] == elem_step
    stride_bytes_256 = exact_div(elem_step * mybir.dt.size(in_ap.dtype), 256)
    assert stride_bytes_256 < 256
    _in_ap = gp.lower_ap_dma(in_ap, for_custom_bir_dma=True)
    _idxs_ap = gp.lower_ap(idxs_ap)
    _out_ap = gp.lower_ap(out_ap)
    return gp.add_instruction(
        mybir.InstDMAGatherAnt(
            name=gp.bass.get_next_instruction_name(),
            ins=[*_in_ap, _idxs_ap, gp.lower_val_access(gp.to_reg(num_idxs))],
            outs=[_out_ap],
            transpose=False,
            num_idxs=num_idxs,
            elem_size=elem_size,
            stride_bytes_256=stride_bytes_256,
            gen_mode=0,
            single_packet=False,
            queue_num=0,
            sbuf_tokens_per_rank=0,
            sbuf_free_dim_per_rank=0,
            sbuf_free_dim_pad_per_rank=0,
            sbuf_byte_offset=0,
        )
    )


def _b(ap2, reps):
    """broadcast each element of a [P, K] AP over `reps` trailing copies."""
    return bass.AP(tensor=ap2.tensor, offset=ap2.offset, ap=[*ap2.ap, [0, reps]])


def _bcast_col(ap1, n):
    """[P, 1] AP -> [P, n] zero-stride broadcast (drops the unit free dim)."""
    return bass.AP(tensor=ap1.tensor, offset=ap1.offset, ap=[ap1.ap[0], [0, n]])


def build(meta):
    geo: Geo = meta["geo"]
    S_T, COLS, T, KC = meta["S_T"], meta["COLS"], meta["T"], meta["KC"]
    colmeta, ginstr = meta["colmeta"], meta["ginstr"]
    nsh, ntab, nwin, G = geo.nsh, geo.ntab, geo.nwin, geo.groups
    f32 = mybir.dt.float32
    AX = mybir.AxisListType.X
    AF = mybir.ActivationFunctionType

    nc = bacc.Bacc("TRN2", target_bir_lowering=False, debug=False, num_devices=CORES)

    xT = nc.dram_tensor("xT", [F_IN, nsh], f32, kind="ExternalInput")
    gidx = nc.dram_tensor("gidx", [128, T // 16], mybir.dt.int16, kind="ExternalInput")
    wgrid = nc.dram_tensor("wgrid", [128, KC], f32, kind="ExternalInput")
    dloc = nc.dram_tensor("dloc", [128, COLS], f32, kind="ExternalInput")
    iota512 = nc.dram_tensor("iota512", [128, 512], f32, kind="ExternalInput")
    ident = nc.dram_tensor("ident", [128, 128], f32, kind="ExternalInput")
    W1t = nc.dram_tensor("W1t", [F_IN, F_HID], f32, kind="ExternalInput")
    b1t = nc.dram_tensor("b1t", [128, F_HID], f32, kind="ExternalInput")
    W2t = nc.dram_tensor("W2t", [F_HID, F_OUT], f32, kind="ExternalInput")
    b2t = nc.dram_tensor("b2t", [1, F_OUT], f32, kind="ExternalInput")
    onest = nc.dram_tensor("onest", [1, 128], f32, kind="ExternalInput")
    out_t = nc.dram_tensor("out", [nsh, F_OUT], f32, kind="ExternalOutput")

    def shard_ap(tensor):
        # [128, v, f(32 of 64)] view of a [nsh, 64] shard: row = v*128 + p
        return bass.AP(tensor=tensor.tensor, offset=0, ap=[[64, 128], [128 * 64, nwin], [1, 32]])

    with tile.TileContext(nc) as tc:
        with (
            tc.tile_pool(name="const", bufs=1) as cpool,
            tc.tile_pool(name="dram", bufs=1, space="DRAM") as dram,
            tc.tile_pool(name="work", bufs=3) as wp,
            tc.tile_pool(name="scol", bufs=8) as sp,
            tc.tile_pool(name="agg", bufs=1) as apool,
        ):
            iota_t = cpool.tile([128, 512], f32)
            nc.sync.dma_start(out=iota_t[:], in_=iota512[:, :])
            dloc_t = cpool.tile([128, COLS], f32)
            nc.sync.dma_start(out=dloc_t[:], in_=dloc[:, :])
            W1s = cpool.tile([F_IN, F_HID], f32)
            nc.sync.dma_start(out=W1s[:], in_=W1t[:, :])
            b1s = cpool.tile([128, F_HID], f32)
            nc.sync.dma_start(out=b1s[:], in_=b1t[:, :])
            W2s = cpool.tile([F_HID, F_OUT], f32)
            nc.sync.dma_start(out=W2s[:], in_=W2t[:, :])
            b2s = cpool.tile([1, F_OUT], f32)
            nc.sync.dma_start(out=b2s[:], in_=b2t[:, :])
            ones_s = cpool.tile([1, 128], f32)
            nc.sync.dma_start(out=ones_s[:], in_=onest[:, :])
            id_s = cpool.tile([128, 128], f32)
            nc.sync.dma_start(out=id_s[:], in_=ident[:, :])

            shard1 = dram.tile([nsh, 64], f32)
            shard2 = dram.tile([nsh, 64], f32)
            table1 = dram.tile([ntab, 64], f32)
            table2 = dram.tile([ntab, 64], f32)

            # ---- h = x @ W1 on own shard -> shard1 ----
            with (
                tc.tile_pool(name="xt", bufs=1) as xp,
                tc.tile_pool(name="ph", bufs=2, space="PSUM") as ph,
            ):
                half = nsh // 2
                for hh in range(2):
                    xTs = xp.tile([F_IN, half], f32, tag="xts")
                    nc.sync.dma_start(out=xTs[:], in_=xT[:, hh * half : (hh + 1) * half])
                    for tt in range(half // 128):
                        t = hh * (half // 128) + tt
                        hp = ph.tile([128, F_HID], f32, tag="hps")
                        nc.tensor.matmul(
                            out=hp[:], lhsT=xTs[:, tt * 128 : (tt + 1) * 128], rhs=W1s[:],
                            start=True, stop=True,
                        )
                        hs = wp.tile([128, 64], f32, tag="hsb")
                        nc.vector.memset(hs[:], 0.0)
                        nc.vector.tensor_copy(out=hs[:, :F_HID], in_=hp[:])
                        dst = bass.AP(
                            tensor=shard1.tensor, offset=t * 128 * 64,
                            ap=[[64, 128], [1, 64]],
                        )
                        nc.sync.dma_start(out=dst, in_=hs[:])

            nc.gpsimd.collective_compute(
                "AllGather", mybir.AluOpType.bypass,
                ins=[shard1.opt()], outs=[table1.opt()],
                replica_groups=[list(range(CORES))],
            )

            def layer(table, agg_tile, init_b, ps2):
                if init_b is not None:
                    bsrc = bass.AP(
                        tensor=init_b.tensor, offset=init_b[:].offset,
                        ap=[init_b[:].ap[0], [0, nwin], [1, 32]],
                    )
                    nc.vector.tensor_copy(
                        out=agg_tile[:].rearrange("p (v f) -> p v f", f=32), in_=bsrc
                    )
                else:
                    nc.vector.memset(agg_tile[:], 0.0)
                for g, kc0, gk in ginstr:
                    gx = wp.tile([128, gk * 8], mybir.dt.int16, tag="gx")
                    nc.sync.dma_start(out=gx[:], in_=gidx[:, kc0 * 8 : (kc0 + gk) * 8])
                    ws = wp.tile([128, gk], f32, tag="ws")
                    nc.sync.dma_start(out=ws[:], in_=wgrid[:, kc0 : kc0 + gk])
                    msgs = wp.tile([128, gk * 32], f32, tag="msgs")
                    emit_dma_gather(
                        nc.gpsimd,
                        out_ap=msgs[:].rearrange("p (k f) -> p k f", f=32),
                        in_ap=bass.AP(
                            tensor=table.tensor, offset=g * geo.chunk * 64,
                            ap=[[64, geo.chunk], [1, 32]],
                        ),
                        idxs_ap=gx[:],
                        num_idxs=gk * 128,
                        elem_size=32,
                        elem_step=64,
                    )
                    nc.vector.tensor_tensor(
                        out=msgs[:].rearrange("p (k f) -> p k f", f=32),
                        in0=msgs[:].rearrange("p (k f) -> p k f", f=32),
                        in1=_b(ws[:], 32), op=mybir.AluOpType.mult,
                    )
                    # 8 -> 1 tree reduction over k within each slot
                    nb = gk  # 32-elem blocks
                    t1 = wp.tile([128, nb // 2 * 32], f32, tag="t1")
                    nc.vector.tensor_tensor(
                        out=t1[:].rearrange("p (k f) -> p k f", f=32),
                        in0=bass.AP(tensor=msgs.tensor, offset=msgs[:].offset,
                                               ap=[msgs[:].ap[0], [64, nb // 2], [1, 32]]),
                        in1=bass.AP(tensor=msgs.tensor, offset=msgs[:].offset + 32,
                                    ap=[msgs[:].ap[0], [64, nb // 2], [1, 32]]),
                        op=mybir.AluOpType.add,
                    )
                    t2 = wp.tile([128, nb // 4 * 32], f32, tag="t2")
                    nc.vector.tensor_tensor(
                        out=t2[:].rearrange("p (k f) -> p k f", f=32),
                        in0=bass.AP(tensor=t1.tensor, offset=t1[:].offset,
                                               ap=[t1[:].ap[0], [64, nb // 4], [1, 32]]),
                        in1=bass.AP(tensor=t1.tensor, offset=t1[:].offset + 32,
                                    ap=[t1[:].ap[0], [64, nb // 4], [1, 32]]),
                        op=mybir.AluOpType.add,
                    )
                    out1 = wp.tile([128, nb // 8 * 32], f32, tag="out1")
                    nc.vector.tensor_tensor(
                        out=out1[:].rearrange("p (k f) -> p k f", f=32),
                        in0=bass.AP(tensor=t2.tensor, offset=t2[:].offset,
                                                 ap=[t2[:].ap[0], [64, nb // 8], [1, 32]]),
                        in1=bass.AP(tensor=t2.tensor, offset=t2[:].offset + 32,
                                    ap=[t2[:].ap[0], [64, nb // 8], [1, 32]]),
                        op=mybir.AluOpType.add,
                    )
                    # stage 2: per slot-column one-hot matmul into agg windows
                    for sc in range(nb // 8):
                        col = kc0 // 8 + sc
                        cm = colmeta[col]
                        if cm is None:
                            continue
                        a_col, wins, _ = cm
                        for wv in wins:
                            oh = sp.tile([128, 128], f32, tag="oh")
                            nc.vector.tensor_tensor(
                                out=oh[:],
                                in0=iota_t[:, (wv - a_col) * 128 : (wv - a_col + 1) * 128],
                                in1=_bcast_col(dloc_t[:, col : col + 1], 128),
                                op=mybir.AluOpType.is_equal,
                            )
                            pw = ps2.tile([128, 32], f32, tag="pw")
                            nc.tensor.matmul(
                                out=pw[:], lhsT=oh[:],
                                rhs=out1[:, sc * 32 : (sc + 1) * 32],
                                start=True, stop=True,
                            )
                            nc.vector.tensor_tensor(
                                out=agg_tile[:, wv * 32 : (wv + 1) * 32],
                                in0=agg_tile[:, wv * 32 : (wv + 1) * 32],
                                in1=pw[:], op=mybir.AluOpType.add,
                            )

            # ---- layer 1 ----
            agg1 = apool.tile([128, nwin * 32], f32, tag="agg1")
            with tc.tile_pool(name="ps2a", bufs=6, space="PSUM") as ps2:
                layer(table1, agg1, b1s, ps2)
            # relu -> shard2 -> AllGather -> table2
            h2cm = tc.tile_pool(name="h2p", bufs=1)
            h2pool = h2cm.__enter__()
            h2 = h2pool.tile([128, nwin * 64], f32, tag="h2")
            nc.vector.memset(h2[:], 0.0)
            h2v = bass.AP(tensor=h2.tensor, offset=h2[:].offset,
                          ap=[h2[:].ap[0], [64, nwin], [1, 32]])
            nc.scalar.activation(
                out=h2v, in_=agg1[:].rearrange("p (v f) -> p v f", f=32), func=AF.Relu
            )
            dst2 = bass.AP(tensor=shard2.tensor, offset=0,
                           ap=[[64, 128], [128 * 64, nwin], [1, 64]])
            nc.sync.dma_start(out=dst2, in_=h2[:].rearrange("p (v f) -> p v f", f=64))
            h2cm.__exit__(None, None, None)
            nc.gpsimd.collective_compute(
                "AllGather", mybir.AluOpType.bypass,
                ins=[shard2.opt()], outs=[table2.opt()],
                replica_groups=[list(range(CORES))],
            )

            # ---- layer 2 ----
            agg2 = apool.tile([128, nwin * 32], f32, tag="agg1")
            with tc.tile_pool(name="ps2b", bufs=6, space="PSUM") as ps2:
                layer(table2, agg2, None, ps2)

            # ---- out = log_softmax(agg2 @ W2 + b2) ----
            zall = apool.tile([128, nwin * F_OUT], f32, tag="zall")
            sall = apool.tile([128, nwin], f32, tag="sall")
            pf_cm = tc.tile_pool(name="pf", bufs=2, space="PSUM")
            pf = pf_cm.__enter__()
            for v in range(nwin):
                tp = pf.tile([F_HID, 128], f32, tag="tp")
                nc.tensor.transpose(
                    out=tp[:], in_=agg2[:, v * 32 : (v + 1) * 32], identity=id_s[:]
                )
                aT = sp.tile([F_HID, 128], f32, tag="aT")
                nc.vector.tensor_copy(out=aT[:], in_=tp[:])
                zp = pf.tile([128, F_OUT], f32, tag="zp")
                nc.tensor.matmul(out=zp[:], lhsT=aT[:], rhs=W2s[:], start=True, stop=False)
                nc.tensor.matmul(out=zp[:], lhsT=ones_s[:], rhs=b2s[:], start=False, stop=True)
                negm = sp.tile([128, 1], f32, tag="negm")
                nc.vector.reduce_max(out=negm[:], in_=zp[:], axis=AX, negate=True)
                nc.vector.tensor_tensor(
                    out=zall[:, v * F_OUT : (v + 1) * F_OUT],
                    in0=zp[:], in1=_bcast_col(negm[:], F_OUT),
                    op=mybir.AluOpType.add,
                )
                etmp = sp.tile([128, F_OUT], f32, tag="etmp")
                nc.scalar.activation(
                    out=etmp[:], in_=zall[:, v * F_OUT : (v + 1) * F_OUT],
                    func=AF.Exp, accum_out=sall[:, v : v + 1],
                )
            lns = apool.tile([128, nwin], f32, tag="lns")
            nc.scalar.activation(out=lns[:], in_=sall[:], func=AF.Ln)
            for v in range(nwin):
                nc.vector.tensor_tensor(
                    out=zall[:, v * F_OUT : (v + 1) * F_OUT],
                    in0=zall[:, v * F_OUT : (v + 1) * F_OUT],
                    in1=_bcast_col(lns[:, v : v + 1], F_OUT),
                    op=mybir.AluOpType.subtract,
                )
            outdst = bass.AP(
                tensor=out_t, offset=0,
                ap=[[F_OUT, 128], [128 * F_OUT, nwin], [1, F_OUT]],
            )
            nc.sync.dma_start(out=outdst, in_=zall[:].rearrange("p (v f) -> p v f", f=F_OUT))
            pf_cm.__exit__(None, None, None)

    nc.compile()
    return nc


def make_inmaps(meta, inmaps_edges, x, W1, b1, W2, b2):
    geo: Geo = meta["geo"]
    nsh = geo.nsh
    n = geo.n_nodes
    xT_full = np.zeros((F_IN, geo.ntab), np.float32)
    xT_full[:, :n] = np.asarray(x, np.float32).T
    iota = np.tile(np.arange(512, dtype=np.float32)[None, :], (128, 1))
    ident = np.eye(128, dtype=np.float32)
    b1b = np.tile(np.asarray(b1, np.float32)[None, :], (128, 1))
    consts = dict(
        iota512=iota, ident=ident,
        W1t=np.asarray(W1, np.float32), b1t=b1b,
        W2t=np.asarray(W2, np.float32), b2t=np.asarray(b2, np.float32)[None, :],
        onest=np.ones((1, 128), np.float32),
    )
    maps = []
    for c in range(CORES):
        m = dict(inmaps_edges[c])
        m.update(consts)
        m["xT"] = np.ascontiguousarray(xT_full[:, c * nsh : (c + 1) * nsh])
        maps.append(m)
    return maps


_CACHE = {}


def run(x, edge_index, edge_weight, W1, b1, W2, b2, geo=FULL, trace=False):
    key = "geo%d" % geo.n_nodes
    meta, inmaps_edges = pack(edge_index, edge_weight, geo)
    if key in _CACHE:
        nc = _CACHE[key]
    else:
        nc = build(meta)
        _CACHE[key] = nc
    maps = make_inmaps(meta, inmaps_edges, x, W1, b1, W2, b2)
    res = run_bass_kernel_spmd(nc, maps, core_ids=list(range(CORES)), trace=trace)
    n = geo.n_nodes
    out = np.empty((n, F_OUT), np.float32)
    for c in range(CORES):
        lo = c * geo.nsh
        hi = min(lo + geo.nsh, n)
        if hi > lo:
            out[lo:hi] = res.results[c]["out"][: hi - lo]
    return out, res


def kernel(x, edge_index, edge_weight, W1, b1, W2, b2):
    out, _ = run(
        np.asarray(x), np.asarray(edge_index), np.asarray(edge_weight),
        np.asarray(W1), np.asarray(b1), np.asarray(W2), np.asarray(b2),
    )
    return out



# revision 7
# speedup vs baseline: 1.4482x; 1.4482x over previous
"""GCN (2-layer, edge-weighted, log_softmax) on 8 Trainium2 NeuronCores.

Strategy (dst-sharded edges, matmul-based segment-sum):
  - Nodes sharded 12544/core (table rows = node ids, 352 junk tail rows).
  - Layer 1: h = x @ W1 computed data-parallel on node shards -> AllGather
    into a replicated, 256B-strided feature table in HBM.
  - Per-edge gather of 128B rows via the InstDMAGatherAnt SWDGE ucode
    (int16 idx => 4 table chunks of 25088 rows; edges grouped by src chunk).
  - Edges packed into 8-edge same-destination slots; DVE does x-weight and
    an 8->1 tree reduction; a per-column one-hot (is_equal vs iota) matmul
    segment-sums slot partials into PSUM windows of 128 destinations,
    accumulated into an SBUF aggregate laid out [d%128, (d//128)*32+f].
  - Layer 2 aggregates relu(agg1 + b1) with the identical edge structure,
    then applies W2 (+b2 via a ones-row matmul) per 128-node window,
    followed by an on-chip log_softmax.
Host side only packs indices/weights (numpy) and concatenates shards.
"""

import os
import sys

for _p in ("/opt/trn_rl_repo", "/root/.axon_site/_ro/trn_rl_repo"):
    if os.path.isdir(_p) and _p not in sys.path:
        sys.path.insert(0, _p)

import numpy as np

import concourse.ap_utils as ap_utils
import concourse.bass as bass
import concourse.mybir as mybir
from concourse import bacc, tile
from concourse.bass_utils import run_bass_kernel_spmd

CORES = 8
F_IN = 128
F_HID = 32
F_OUT = 40
KSLOT = 8  # edges per slot (same destination)
GK_INSTR = 64  # gather k-columns per instruction (8192 tokens)


class Geo:
    """Problem geometry. Full size by default; shrinkable for simulation."""

    def __init__(self, n_nodes=100000, nsh=12544, chunk=25088, groups=4):
        self.n_nodes = n_nodes
        self.nsh = nsh  # nodes per core shard (mult of 128)
        self.ntab = nsh * CORES  # table rows
        self.chunk = chunk  # gather table chunk rows (<= 32768)
        self.groups = groups
        assert chunk * groups == self.ntab
        assert nsh % 128 == 0
        self.nwin = nsh // 128  # 128-destination windows per core


FULL = Geo()


def _wrap16(flat, T):
    """token i -> [i%16, i//16], replicated to 128 partitions."""
    a = flat.reshape(T // 16, 16).T
    return np.tile(a, (8, 1)).copy()


def pack(edge_index, edge_weight, geo: Geo):
    """Group edges by (core, src-chunk, dst) into 8-edge slots; build the
    shared column->window template and all per-core device arrays."""
    src = np.asarray(edge_index[0], dtype=np.int64)
    dst = np.asarray(edge_index[1], dtype=np.int64)
    w = np.asarray(edge_weight, dtype=np.float32)
    nsh, nwin, G = geo.nsh, geo.nwin, geo.groups

    core = dst // nsh
    pc = []
    cnt = np.zeros((CORES, G, nwin), np.int64)
    for c in range(CORES):
        m = core == c
        s_c = src[m]
        dl = dst[m] - c * nsh
        wc = w[m]
        g = s_c // geo.chunk
        key = g * (2 * nsh) + dl
        order = np.argsort(key, kind="stable")
        sk = (s_c - g * geo.chunk)[order]
        dlk = dl[order]
        wk = wc[order]
        kk = key[order]
        new = np.r_[True, kk[1:] != kk[:-1]]
        run_first = np.flatnonzero(new)
        run_len = np.diff(np.r_[run_first, len(kk)])
        run_id = np.cumsum(new) - 1
        rank = np.arange(len(kk)) - run_first[run_id]
        nsl = (run_len + KSLOT - 1) // KSLOT
        g_run = (g[order])[run_first]
        dl_run = dlk[run_first]
        v_run = dl_run // 128
        np.add.at(cnt[c], (g_run, v_run), nsl)
        pc.append((sk, dlk, wk, rank, run_id, nsl, g_run, dl_run, v_run))

    # shared template
    cap = cnt.max(axis=0)  # [G, nwin]
    cap = ((cap + 31) // 32) * 32
    gslots = cap.sum(axis=1)
    gpad = (-gslots) % 128
    off = np.zeros((G, nwin), np.int64)
    gbase = np.zeros(G + 1, np.int64)
    b = 0
    for g in range(G):
        gbase[g] = b
        for v in range(nwin):
            off[g, v] = b
            b += cap[g, v]
        b += gpad[g]
    gbase[G] = b
    S_T = int(b)
    COLS = S_T // 128
    T = S_T * KSLOT
    KC = T // 128  # k-columns total

    # per-column window lists (template)
    colmeta = []  # (A_col, [wins]) or None
    flat_off = []
    flat_gv = []
    for g in range(G):
        for v in range(nwin):
            if cap[g, v] > 0:
                flat_off.append(int(off[g, v]))
                flat_gv.append((g, v))
    flat_off = np.array(flat_off + [S_T], dtype=np.int64)
    for col in range(COLS):
        lo, hi = col * 128, col * 128 + 128
        i0 = int(np.searchsorted(flat_off, lo, side="right") - 1)
        wins = []
        gcol = None
        for i in range(max(i0, 0), len(flat_gv)):
            o = flat_off[i]
            if o >= hi:
                break
            o2 = flat_off[i + 1]
            if o2 <= lo:
                continue
            gg, vv = flat_gv[i]
            if gcol is None:
                gcol = gg
            if gg == gcol:
                wins.append(vv)
        # group-tail pad regions have no (g,v); wins may be empty
        colmeta.append((wins[0], wins, gcol) if wins else None)

    # per-group k-column ranges for gather instructions
    ginstr = []  # (group, kc0, gk)
    for g in range(G):
        kc_lo = int(gbase[g]) // 16
        kc_hi = int(gbase[g + 1]) // 16
        kc = kc_lo
        while kc < kc_hi:
            gk = min(GK_INSTR, kc_hi - kc)
            ginstr.append((g, kc, gk))
            kc += gk

    # per-core arrays
    inmaps = []
    for c in range(CORES):
        sk, dlk, wk, rank, run_id, nsl, g_run, dl_run, v_run = pc[c]
        n_runs = len(nsl)
        csum = np.cumsum(nsl)
        start_excl = np.r_[0, csum[:-1]]
        gv = g_run * nwin + v_run
        newgv = np.r_[True, gv[1:] != gv[:-1]]
        gv_first = np.flatnonzero(newgv)
        gv_id = np.cumsum(newgv) - 1
        base_in_gv = start_excl - start_excl[gv_first][gv_id]
        run_slot = off[g_run, v_run] + base_in_gv
        slot_e = run_slot[run_id] + rank // KSLOT
        j_e = rank % KSLOT
        cs_e = slot_e // 128
        p_e = slot_e % 128
        tok = (cs_e * KSLOT + j_e) * 128 + p_e

        idx_flat = np.zeros(T, np.int16)
        idx_flat[tok] = sk.astype(np.int16)
        w_flat = np.zeros(T, np.float32)
        w_flat[tok] = wk

        dl_slot = np.full(S_T, 100000.0, np.float32)
        reps = np.repeat(np.arange(n_runs), nsl)
        ar = np.arange(len(reps)) - np.repeat(start_excl, nsl)
        pos = np.repeat(run_slot, nsl) + ar
        dl_slot[pos] = np.repeat(dl_run, nsl).astype(np.float32)
        dcol = dl_slot.reshape(COLS, 128).T.copy()  # [128, COLS]
        for col in range(COLS):
            if colmeta[col] is not None:
                dcol[:, col] -= 128.0 * colmeta[col][0]

        inmaps.append(
            dict(
                gidx=_wrap16(idx_flat, T),
                wgrid=w_flat.reshape(KC, 128).T.copy(),
                dloc=dcol,
            )
        )

    meta = dict(S_T=S_T, COLS=COLS, T=T, KC=KC, colmeta=colmeta, ginstr=ginstr, geo=geo)
    return meta, inmaps


def emit_dma_gather(gp, out_ap, in_ap, idxs_ap, num_idxs, elem_size, elem_step,
                    queue_num=0):
    """bass.dma_gather minus the blanket 256B elem assert (verified on HW that
    the non-transpose ucode handles 128B rows)."""
    from concourse.bass import exact_div

    assert idxs_ap.dtype == mybir.dt.int16
    assert in_ap.dtype == out_ap.dtype
    assert in_ap.space == bass.MemorySpace.DRAM
    assert ap_utils.ap_is_contiguous(in_ap.ap[1:])
    assert ap_utils.ap_is_contiguous(out_ap.ap[1:])
    assert ap_utils.ap_is_contiguous(idxs_ap.ap[1:])
    assert in_ap.ap[-1][1] == out_ap.ap[-1][1] == elem_size
    assert out_ap.ap[0][1] * out_ap.ap[1][1] == num_idxs
    assert in_ap.ap[0][0] == elem_step
    stride_bytes_256 = exact_div(elem_step * mybir.dt.size(in_ap.dtype), 256)
    assert stride_bytes_256 < 256
    _in_ap = gp.lower_ap_dma(in_ap, for_custom_bir_dma=True)
    _idxs_ap = gp.lower_ap(idxs_ap)
    _out_ap = gp.lower_ap(out_ap)
    return gp.add_instruction(
        mybir.InstDMAGatherAnt(
            name=gp.bass.get_next_instruction_name(),
            ins=[*_in_ap, _idxs_ap, gp.lower_val_access(gp.to_reg(num_idxs))],
            outs=[_out_ap],
            transpose=False,
            num_idxs=num_idxs,
            elem_size=elem_size,
            stride_bytes_256=stride_bytes_256,
            gen_mode=0,
            single_packet=False,
            queue_num=queue_num,
            sbuf_tokens_per_rank=0,
            sbuf_free_dim_per_rank=0,
            sbuf_free_dim_pad_per_rank=0,
            sbuf_byte_offset=0,
        )
    )


def _b(ap2, reps):
    """broadcast each element of a [P, K] AP over `reps` trailing copies."""
    return bass.AP(tensor=ap2.tensor, offset=ap2.offset, ap=[*ap2.ap, [0, reps]])


def _bcast_col(ap1, n):
    """[P, 1] AP -> [P, n] zero-stride broadcast (drops the unit free dim)."""
    return bass.AP(tensor=ap1.tensor, offset=ap1.offset, ap=[ap1.ap[0], [0, n]])


def build(meta):
    geo: Geo = meta["geo"]
    S_T, COLS, T, KC = meta["S_T"], meta["COLS"], meta["T"], meta["KC"]
    colmeta, ginstr = meta["colmeta"], meta["ginstr"]
    nsh, ntab, nwin, G = geo.nsh, geo.ntab, geo.nwin, geo.groups
    f32 = mybir.dt.float32
    AX = mybir.AxisListType.X
    AF = mybir.ActivationFunctionType

    nc = bacc.Bacc("TRN2", target_bir_lowering=False, debug=False, num_devices=CORES,
                   num_swdge_queues=4)

    xT = nc.dram_tensor("xT", [F_IN, nsh], f32, kind="ExternalInput")
    gidx = nc.dram_tensor("gidx", [128, T // 16], mybir.dt.int16, kind="ExternalInput")
    wgrid = nc.dram_tensor("wgrid", [128, KC], f32, kind="ExternalInput")
    dloc = nc.dram_tensor("dloc", [128, COLS], f32, kind="ExternalInput")
    iota512 = nc.dram_tensor("iota512", [128, 512], f32, kind="ExternalInput")
    ident = nc.dram_tensor("ident", [128, 128], f32, kind="ExternalInput")
    W1t = nc.dram_tensor("W1t", [F_IN, F_HID], f32, kind="ExternalInput")
    b1t = nc.dram_tensor("b1t", [128, F_HID], f32, kind="ExternalInput")
    W2t = nc.dram_tensor("W2t", [F_HID, F_OUT], f32, kind="ExternalInput")
    b2t = nc.dram_tensor("b2t", [1, F_OUT], f32, kind="ExternalInput")
    onest = nc.dram_tensor("onest", [1, 128], f32, kind="ExternalInput")
    out_t = nc.dram_tensor("out", [nsh, F_OUT], f32, kind="ExternalOutput")

    def shard_ap(tensor):
        # [128, v, f(32 of 64)] view of a [nsh, 64] shard: row = v*128 + p
        return bass.AP(tensor=tensor.tensor, offset=0, ap=[[64, 128], [128 * 64, nwin], [1, 32]])

    with tile.TileContext(nc) as tc:
        with (
            tc.tile_pool(name="const", bufs=1) as cpool,
            tc.tile_pool(name="dram", bufs=1, space="DRAM") as dram,
            tc.tile_pool(name="work", bufs=6) as wp,
            tc.tile_pool(name="scol", bufs=8) as sp,
            tc.tile_pool(name="agg", bufs=1) as apool,
        ):
            iota_t = cpool.tile([128, 512], f32)
            nc.sync.dma_start(out=iota_t[:], in_=iota512[:, :])
            dloc_t = cpool.tile([128, COLS], f32)
            nc.sync.dma_start(out=dloc_t[:], in_=dloc[:, :])
            W1s = cpool.tile([F_IN, F_HID], f32)
            nc.sync.dma_start(out=W1s[:], in_=W1t[:, :])
            b1s = cpool.tile([128, F_HID], f32)
            nc.sync.dma_start(out=b1s[:], in_=b1t[:, :])
            W2s = cpool.tile([F_HID, F_OUT], f32)
            nc.sync.dma_start(out=W2s[:], in_=W2t[:, :])
            b2s = cpool.tile([1, F_OUT], f32)
            nc.sync.dma_start(out=b2s[:], in_=b2t[:, :])
            ones_s = cpool.tile([1, 128], f32)
            nc.sync.dma_start(out=ones_s[:], in_=onest[:, :])
            id_s = cpool.tile([128, 128], f32)
            nc.sync.dma_start(out=id_s[:], in_=ident[:, :])

            shard1 = dram.tile([nsh, 64], f32)
            shard2 = dram.tile([nsh, 64], f32)
            table1 = dram.tile([ntab, 64], f32)
            table2 = dram.tile([ntab, 64], f32)

            # ---- h = x @ W1 on own shard -> shard1 ----
            with (
                tc.tile_pool(name="xt", bufs=1) as xp,
                tc.tile_pool(name="ph", bufs=2, space="PSUM") as ph,
            ):
                half = nsh // 2
                for hh in range(2):
                    xTs = xp.tile([F_IN, half], f32, tag="xts")
                    nc.sync.dma_start(out=xTs[:], in_=xT[:, hh * half : (hh + 1) * half])
                    for tt in range(half // 128):
                        t = hh * (half // 128) + tt
                        hp = ph.tile([128, F_HID], f32, tag="hps")
                        nc.tensor.matmul(
                            out=hp[:], lhsT=xTs[:, tt * 128 : (tt + 1) * 128], rhs=W1s[:],
                            start=True, stop=True,
                        )
                        hs = wp.tile([128, 64], f32, tag="hsb")
                        nc.vector.memset(hs[:], 0.0)
                        nc.vector.tensor_copy(out=hs[:, :F_HID], in_=hp[:])
                        dst = bass.AP(
                            tensor=shard1.tensor, offset=t * 128 * 64,
                            ap=[[64, 128], [1, 64]],
                        )
                        nc.sync.dma_start(out=dst, in_=hs[:])

            nc.gpsimd.collective_compute(
                "AllGather", mybir.AluOpType.bypass,
                ins=[shard1.opt()], outs=[table1.opt()],
                replica_groups=[list(range(CORES))],
            )

            def layer(table, agg_tile, init_b, ps2):
                if init_b is not None:
                    bsrc = bass.AP(
                        tensor=init_b.tensor, offset=init_b[:].offset,
                        ap=[init_b[:].ap[0], [0, nwin], [1, 32]],
                    )
                    nc.vector.tensor_copy(
                        out=agg_tile[:].rearrange("p (v f) -> p v f", f=32), in_=bsrc
                    )
                else:
                    nc.vector.memset(agg_tile[:], 0.0)
                for gi, (g, kc0, gk) in enumerate(ginstr):
                    gx = wp.tile([128, gk * 8], mybir.dt.int16, tag="gx")
                    nc.sync.dma_start(out=gx[:], in_=gidx[:, kc0 * 8 : (kc0 + gk) * 8])
                    ws = wp.tile([128, gk], f32, tag="ws")
                    nc.sync.dma_start(out=ws[:], in_=wgrid[:, kc0 : kc0 + gk])
                    msgs = wp.tile([128, gk * 32], f32, tag="msgs")
                    emit_dma_gather(
                        nc.gpsimd,
                        out_ap=msgs[:].rearrange("p (k f) -> p k f", f=32),
                        in_ap=bass.AP(
                            tensor=table.tensor, offset=g * geo.chunk * 64,
                            ap=[[64, geo.chunk], [1, 32]],
                        ),
                        idxs_ap=gx[:],
                        num_idxs=gk * 128,
                        elem_size=32,
                        elem_step=64,
                        queue_num=gi % 4,
                    )
                    nc.vector.tensor_tensor(
                        out=msgs[:].rearrange("p (k f) -> p k f", f=32),
                        in0=msgs[:].rearrange("p (k f) -> p k f", f=32),
                        in1=_b(ws[:], 32), op=mybir.AluOpType.mult,
                    )
                    # 8 -> 1 tree reduction over k within each slot
                    nb = gk  # 32-elem blocks
                    t1 = wp.tile([128, nb // 2 * 32], f32, tag="t1")
                    nc.vector.tensor_tensor(
                        out=t1[:].rearrange("p (k f) -> p k f", f=32),
                        in0=bass.AP(tensor=msgs.tensor, offset=msgs[:].offset,
                                               ap=[msgs[:].ap[0], [64, nb // 2], [1, 32]]),
                        in1=bass.AP(tensor=msgs.tensor, offset=msgs[:].offset + 32,
                                    ap=[msgs[:].ap[0], [64, nb // 2], [1, 32]]),
                        op=mybir.AluOpType.add,
                    )
                    t2 = wp.tile([128, nb // 4 * 32], f32, tag="t2")
                    nc.vector.tensor_tensor(
                        out=t2[:].rearrange("p (k f) -> p k f", f=32),
                        in0=bass.AP(tensor=t1.tensor, offset=t1[:].offset,
                                               ap=[t1[:].ap[0], [64, nb // 4], [1, 32]]),
                        in1=bass.AP(tensor=t1.tensor, offset=t1[:].offset + 32,
                                    ap=[t1[:].ap[0], [64, nb // 4], [1, 32]]),
                        op=mybir.AluOpType.add,
                    )
                    out1 = wp.tile([128, nb // 8 * 32], f32, tag="out1")
                    nc.vector.tensor_tensor(
                        out=out1[:].rearrange("p (k f) -> p k f", f=32),
                        in0=bass.AP(tensor=t2.tensor, offset=t2[:].offset,
                                                 ap=[t2[:].ap[0], [64, nb // 8], [1, 32]]),
                        in1=bass.AP(tensor=t2.tensor, offset=t2[:].offset + 32,
                                    ap=[t2[:].ap[0], [64, nb // 8], [1, 32]]),
                        op=mybir.AluOpType.add,
                    )
                    # stage 2: per slot-column one-hot matmul into agg windows
                    for sc in range(nb // 8):
                        col = kc0 // 8 + sc
                        cm = colmeta[col]
                        if cm is None:
                            continue
                        a_col, wins, _ = cm
                        for wv in wins:
                            oh = sp.tile([128, 128], f32, tag="oh")
                            nc.vector.tensor_tensor(
                                out=oh[:],
                                in0=iota_t[:, (wv - a_col) * 128 : (wv - a_col + 1) * 128],
                                in1=_bcast_col(dloc_t[:, col : col + 1], 128),
                                op=mybir.AluOpType.is_equal,
                            )
                            pw = ps2.tile([128, 32], f32, tag="pw")
                            nc.tensor.matmul(
                                out=pw[:], lhsT=oh[:],
                                rhs=out1[:, sc * 32 : (sc + 1) * 32],
                                start=True, stop=True,
                            )
                            nc.vector.tensor_tensor(
                                out=agg_tile[:, wv * 32 : (wv + 1) * 32],
                                in0=agg_tile[:, wv * 32 : (wv + 1) * 32],
                                in1=pw[:], op=mybir.AluOpType.add,
                            )

            # ---- layer 1 ----
            agg1 = apool.tile([128, nwin * 32], f32, tag="agg1")
            with tc.tile_pool(name="ps2a", bufs=6, space="PSUM") as ps2:
                layer(table1, agg1, b1s, ps2)
            # relu -> shard2 -> AllGather -> table2
            h2cm = tc.tile_pool(name="h2p", bufs=1)
            h2pool = h2cm.__enter__()
            h2 = h2pool.tile([128, nwin * 64], f32, tag="h2")
            nc.vector.memset(h2[:], 0.0)
            h2v = bass.AP(tensor=h2.tensor, offset=h2[:].offset,
                          ap=[h2[:].ap[0], [64, nwin], [1, 32]])
            nc.scalar.activation(
                out=h2v, in_=agg1[:].rearrange("p (v f) -> p v f", f=32), func=AF.Relu
            )
            dst2 = bass.AP(tensor=shard2.tensor, offset=0,
                           ap=[[64, 128], [128 * 64, nwin], [1, 64]])
            nc.sync.dma_start(out=dst2, in_=h2[:].rearrange("p (v f) -> p v f", f=64))
            h2cm.__exit__(None, None, None)
            nc.gpsimd.collective_compute(
                "AllGather", mybir.AluOpType.bypass,
                ins=[shard2.opt()], outs=[table2.opt()],
                replica_groups=[list(range(CORES))],
            )

            # ---- layer 2 ----
            agg2 = apool.tile([128, nwin * 32], f32, tag="agg1")
            with tc.tile_pool(name="ps2b", bufs=6, space="PSUM") as ps2:
                layer(table2, agg2, None, ps2)

            # ---- out = log_softmax(agg2 @ W2 + b2) ----
            zall = apool.tile([128, nwin * F_OUT], f32, tag="zall")
            sall = apool.tile([128, nwin], f32, tag="sall")
            pf_cm = tc.tile_pool(name="pf", bufs=2, space="PSUM")
            pf = pf_cm.__enter__()
            for v in range(nwin):
                tp = pf.tile([F_HID, 128], f32, tag="tp")
                nc.tensor.transpose(
                    out=tp[:], in_=agg2[:, v * 32 : (v + 1) * 32], identity=id_s[:]
                )
                aT = sp.tile([F_HID, 128], f32, tag="aT")
                nc.vector.tensor_copy(out=aT[:], in_=tp[:])
                zp = pf.tile([128, F_OUT], f32, tag="zp")
                nc.tensor.matmul(out=zp[:], lhsT=aT[:], rhs=W2s[:], start=True, stop=False)
                nc.tensor.matmul(out=zp[:], lhsT=ones_s[:], rhs=b2s[:], start=False, stop=True)
                negm = sp.tile([128, 1], f32, tag="negm")
                nc.vector.reduce_max(out=negm[:], in_=zp[:], axis=AX, negate=True)
                nc.vector.tensor_tensor(
                    out=zall[:, v * F_OUT : (v + 1) * F_OUT],
                    in0=zp[:], in1=_bcast_col(negm[:], F_OUT),
                    op=mybir.AluOpType.add,
                )
                etmp = sp.tile([128, F_OUT], f32, tag="etmp")
                nc.scalar.activation(
                    out=etmp[:], in_=zall[:, v * F_OUT : (v + 1) * F_OUT],
                    func=AF.Exp, accum_out=sall[:, v : v + 1],
                )
            lns = apool.tile([128, nwin], f32, tag="lns")
            nc.scalar.activation(out=lns[:], in_=sall[:], func=AF.Ln)
            for v in range(nwin):
                nc.vector.tensor_tensor(
                    out=zall[:, v * F_OUT : (v + 1) * F_OUT],
                    in0=zall[:, v * F_OUT : (v + 1) * F_OUT],
                    in1=_bcast_col(lns[:, v : v + 1], F_OUT),
                    op=mybir.AluOpType.subtract,
                )
            outdst = bass.AP(
                tensor=out_t, offset=0,
                ap=[[F_OUT, 128], [128 * F_OUT, nwin], [1, F_OUT]],
            )
            nc.sync.dma_start(out=outdst, in_=zall[:].rearrange("p (v f) -> p v f", f=F_OUT))
            pf_cm.__exit__(None, None, None)

    nc.compile()
    return nc


def make_inmaps(meta, inmaps_edges, x, W1, b1, W2, b2):
    geo: Geo = meta["geo"]
    nsh = geo.nsh
    n = geo.n_nodes
    xT_full = np.zeros((F_IN, geo.ntab), np.float32)
    xT_full[:, :n] = np.asarray(x, np.float32).T
    iota = np.tile(np.arange(512, dtype=np.float32)[None, :], (128, 1))
    ident = np.eye(128, dtype=np.float32)
    b1b = np.tile(np.asarray(b1, np.float32)[None, :], (128, 1))
    consts = dict(
        iota512=iota, ident=ident,
        W1t=np.asarray(W1, np.float32), b1t=b1b,
        W2t=np.asarray(W2, np.float32), b2t=np.asarray(b2, np.float32)[None, :],
        onest=np.ones((1, 128), np.float32),
    )
    maps = []
    for c in range(CORES):
        m = dict(inmaps_edges[c])
        m.update(consts)
        m["xT"] = np.ascontiguousarray(xT_full[:, c * nsh : (c + 1) * nsh])
        maps.append(m)
    return maps


_CACHE = {}


def run(x, edge_index, edge_weight, W1, b1, W2, b2, geo=FULL, trace=False):
    key = "geo%d" % geo.n_nodes
    meta, inmaps_edges = pack(edge_index, edge_weight, geo)
    if key in _CACHE:
        nc = _CACHE[key]
    else:
        nc = build(meta)
        _CACHE[key] = nc
    maps = make_inmaps(meta, inmaps_edges, x, W1, b1, W2, b2)
    res = run_bass_kernel_spmd(nc, maps, core_ids=list(range(CORES)), trace=trace)
    n = geo.n_nodes
    out = np.empty((n, F_OUT), np.float32)
    for c in range(CORES):
        lo = c * geo.nsh
        hi = min(lo + geo.nsh, n)
        if hi > lo:
            out[lo:hi] = res.results[c]["out"][: hi - lo]
    return out, res


def kernel(x, edge_index, edge_weight, W1, b1, W2, b2):
    out, _ = run(
        np.asarray(x), np.asarray(edge_index), np.asarray(edge_weight),
        np.asarray(W1), np.asarray(b1), np.asarray(W2), np.asarray(b2),
    )
    return out



# revision 11
# speedup vs baseline: 2.5526x; 1.7625x over previous
"""GCN (2-layer, edge-weighted, log_softmax) on 8 Trainium2 NeuronCores.

v2 strategy (dst-sharded edges, matmul segment-sum, 4-queue SWDGE gather):
  - Nodes sharded 12544/core; fp32 table rows (256B-strided, 32 used) in HBM,
    replicated via AllGather.
  - Edges packed 4-per-slot by (src-chunk, dst). Each 2048-slot instruction
    block issues FOUR 2048-token gathers (one per slot position j) on SWDGE
    queues 0..3 -> concurrent DMA transfers into four separate tiles, so the
    weighted 4->1 reduction is 4 mults + 3 adds, all contiguous two-tile ops.
  - Slot sums (bf16) for the whole layer stay resident in SBUF (out1_all).
  - Scatter runs window-major: once the last instruction touching window v
    completes, a one-shot is_equal builds all of v's one-hot columns (host
    dloc2 template, window-major order), and ~10 bf16 matmuls accumulate
    into a dedicated full PSUM bank (start/stop per window; 4 banks rotate).
    Window closes immediately (bias+relu -> table2 shard, or copy to agg2).
  - Layer 2 reuses the identical edge template on table2 = relu(agg1+b1),
    then W2 + log_softmax per 128-node window.
Host side packs indices/weights (numpy) and concatenates shards.
"""

import os
import sys

for _p in ("/opt/trn_rl_repo", "/root/.axon_site/_ro/trn_rl_repo"):
    if os.path.isdir(_p) and _p not in sys.path:
        sys.path.insert(0, _p)

import numpy as np

import concourse.ap_utils as ap_utils
import concourse.bass as bass
import concourse.mybir as mybir
from concourse import bacc, tile
from concourse.bass_utils import run_bass_kernel_spmd

CORES = 8
F_IN = 128
F_HID = 32
F_OUT = 40
KSLOT = 4      # edges per slot (same destination)
GK = 64        # k-columns per instruction block (8192 tokens, 2048 slots)
SPI = GK * 128 // KSLOT   # slots per instruction block (2048)
SUBK = GK // KSLOT        # k-columns per sub-gather (16 -> 2048 tokens)


class Geo:
    def __init__(self, n_nodes=100000, nsh=12544, chunk=25088, groups=4):
        self.n_nodes = n_nodes
        self.nsh = nsh
        self.ntab = nsh * CORES
        self.chunk = chunk
        self.groups = groups
        assert chunk * groups == self.ntab
        assert nsh % 128 == 0
        self.nwin = nsh // 128


FULL = Geo()


def _wrap16(flat, T):
    a = flat.reshape(T // 16, 16).T
    return np.tile(a, (8, 1)).copy()


def pack(edge_index, edge_weight, geo: Geo):
    src = np.asarray(edge_index[0], dtype=np.int64)
    dst = np.asarray(edge_index[1], dtype=np.int64)
    w = np.asarray(edge_weight, dtype=np.float32)
    nsh, nwin, G = geo.nsh, geo.nwin, geo.groups

    core = dst // nsh
    pc = []
    cnt = np.zeros((CORES, G, nwin), np.int64)
    for c in range(CORES):
        m = core == c
        s_c = src[m]
        dl = dst[m] - c * nsh
        wc = w[m]
        g = s_c // geo.chunk
        key = g * (2 * nsh) + dl
        order = np.argsort(key, kind="stable")
        sk = (s_c - g * geo.chunk)[order]
        dlk = dl[order]
        wk = wc[order]
        kk = key[order]
        new = np.r_[True, kk[1:] != kk[:-1]]
        run_first = np.flatnonzero(new)
        run_len = np.diff(np.r_[run_first, len(kk)])
        run_id = np.cumsum(new) - 1
        rank = np.arange(len(kk)) - run_first[run_id]
        nsl = (run_len + KSLOT - 1) // KSLOT
        g_run = (g[order])[run_first]
        dl_run = dlk[run_first]
        v_run = dl_run // 128
        np.add.at(cnt[c], (g_run, v_run), nsl)
        pc.append((sk, dlk, wk, rank, run_id, nsl, g_run, dl_run, v_run))

    # shared template: exact max-over-cores region capacity; groups padded to
    # whole instruction blocks
    cap = cnt.max(axis=0)  # [G, nwin] slots
    assert (cap.sum(axis=0) > 0).all(), "window with no edges"
    gslots = cap.sum(axis=1)
    gpad = (-gslots) % SPI
    off = np.zeros((G, nwin), np.int64)
    gbase = np.zeros(G + 1, np.int64)
    b = 0
    for g in range(G):
        gbase[g] = b
        for v in range(nwin):
            off[g, v] = b
            b += cap[g, v]
        b += gpad[g]
    gbase[G] = b
    S_T = int(b)
    COLS = S_T // 128
    T = S_T * KSLOT
    KC = T // 128
    NI = S_T // SPI
    instr_group = [int(np.searchsorted(gbase, ii * SPI, side="right") - 1)
                   for ii in range(NI)]

    # window-major colwin template: for each window, its (col) list across
    # groups; complete_at[v] = instruction block that finishes its last region
    win_cols = [[] for _ in range(nwin)]
    for g in range(G):
        for v in range(nwin):
            if cap[g, v] == 0:
                continue
            lo = int(off[g, v])
            hi = lo + int(cap[g, v])
            for col in range(lo // 128, (hi + 127) // 128):
                if col not in win_cols[v]:
                    win_cols[v].append(col)
    complete_at = np.zeros(nwin, np.int64)
    for g in range(G):
        for v in range(nwin):
            if cap[g, v] > 0:
                last = int(off[g, v] + cap[g, v] - 1) // SPI
                complete_at[v] = max(complete_at[v], last)
    sched = [[] for _ in range(NI)]
    for v in range(nwin):
        sched[int(complete_at[v])].append(v)
    # window-major dloc2 column order
    cw_base = np.zeros(nwin + 1, np.int64)
    for v in range(nwin):
        cw_base[v + 1] = cw_base[v] + len(win_cols[v])
    NCW = int(cw_base[nwin])
    OHW = max(len(wc_) for wc_ in win_cols)

    inmaps = []
    for c in range(CORES):
        sk, dlk, wk, rank, run_id, nsl, g_run, dl_run, v_run = pc[c]
        n_runs = len(nsl)
        csum = np.cumsum(nsl)
        start_excl = np.r_[0, csum[:-1]]
        gv = g_run * nwin + v_run
        newgv = np.r_[True, gv[1:] != gv[:-1]]
        gv_first = np.flatnonzero(newgv)
        gv_id = np.cumsum(newgv) - 1
        base_in_gv = start_excl - start_excl[gv_first][gv_id]
        run_slot = off[g_run, v_run] + base_in_gv
        slot_e = run_slot[run_id] + rank // KSLOT
        j_e = rank % KSLOT
        # token position: instruction block ib, local slot ls ->
        #   kc = ib*GK + j*SUBK + ls//128, p = ls%128
        ib = slot_e // SPI
        ls = slot_e - ib * SPI
        kc_e = ib * GK + j_e * SUBK + ls // 128
        p_e = ls % 128
        tok = kc_e * 128 + p_e

        idx_flat = np.zeros(T, np.int16)
        idx_flat[tok] = sk.astype(np.int16)
        w_flat = np.zeros(T, np.float32)
        w_flat[tok] = wk

        dl_slot = np.full(S_T, -1, np.int64)
        reps = np.repeat(np.arange(n_runs), nsl)
        ar = np.arange(len(reps)) - np.repeat(start_excl, nsl)
        pos = np.repeat(run_slot, nsl) + ar
        dl_slot[pos] = np.repeat(dl_run, nsl)
        d2 = np.full((128, NCW), 512.0, np.float32)
        k = 0
        for v in range(nwin):
            for col in win_cols[v]:
                dcol = dl_slot[col * 128:(col + 1) * 128] - 128 * v
                d2[:, k] = np.where((dcol >= 0) & (dcol < 128), dcol, 512)
                k += 1
        inmaps.append(
            dict(
                gidx=_wrap16(idx_flat, T),
                wgrid=w_flat.reshape(KC, 128).T.copy(),
                dloc2=d2,
            )
        )

    meta = dict(S_T=S_T, COLS=COLS, T=T, KC=KC, NCW=NCW, NI=NI, OHW=OHW,
                win_cols=win_cols, cw_base=cw_base, sched=sched,
                instr_group=instr_group, geo=geo)
    return meta, inmaps


def emit_dma_gather(gp, out_ap, in_ap, idxs_ap, num_idxs, elem_size, elem_step,
                    queue_num=0):
    """bass.dma_gather minus the blanket 256B elem assert (verified on HW that
    the non-transpose ucode handles 128B rows)."""
    from concourse.bass import exact_div

    assert idxs_ap.dtype == mybir.dt.int16
    assert in_ap.dtype == out_ap.dtype
    assert in_ap.space == bass.MemorySpace.DRAM
    stride_bytes_256 = exact_div(elem_step * mybir.dt.size(in_ap.dtype), 256)
    _in_ap = gp.lower_ap_dma(in_ap, for_custom_bir_dma=True)
    _idxs_ap = gp.lower_ap(idxs_ap)
    _out_ap = gp.lower_ap(out_ap)
    return gp.add_instruction(
        mybir.InstDMAGatherAnt(
            name=gp.bass.get_next_instruction_name(),
            ins=[*_in_ap, _idxs_ap, gp.lower_val_access(gp.to_reg(num_idxs))],
            outs=[_out_ap],
            transpose=False,
            num_idxs=num_idxs,
            elem_size=elem_size,
            stride_bytes_256=stride_bytes_256,
            gen_mode=0,
            single_packet=False,
            queue_num=queue_num,
            sbuf_tokens_per_rank=0,
            sbuf_free_dim_per_rank=0,
            sbuf_free_dim_pad_per_rank=0,
            sbuf_byte_offset=0,
        )
    )


def _b(ap2, reps):
    return bass.AP(tensor=ap2.tensor, offset=ap2.offset, ap=[*ap2.ap, [0, reps]])


def _bcast_col(ap1, n):
    return bass.AP(tensor=ap1.tensor, offset=ap1.offset, ap=[ap1.ap[0], [0, n]])


def build(meta):
    geo: Geo = meta["geo"]
    S_T, COLS, T, KC, NCW, NI, OHW = (meta["S_T"], meta["COLS"], meta["T"],
                                      meta["KC"], meta["NCW"], meta["NI"],
                                      meta["OHW"])
    win_cols, cw_base, sched = meta["win_cols"], meta["cw_base"], meta["sched"]
    instr_group = meta["instr_group"]
    nsh, ntab, nwin, G = geo.nsh, geo.ntab, geo.nwin, geo.groups
    f32 = mybir.dt.float32
    bf16 = mybir.dt.bfloat16
    AX = mybir.AxisListType.X
    AF = mybir.ActivationFunctionType

    nc = bacc.Bacc("TRN2", target_bir_lowering=False, debug=False,
                   num_devices=CORES, num_swdge_queues=4)

    xT = nc.dram_tensor("xT", [F_IN, nsh], f32, kind="ExternalInput")
    gidx = nc.dram_tensor("gidx", [128, T // 16], mybir.dt.int16,
                          kind="ExternalInput")
    wgrid = nc.dram_tensor("wgrid", [128, KC], f32, kind="ExternalInput")
    dloc2 = nc.dram_tensor("dloc2", [128, NCW], f32, kind="ExternalInput")
    iota128 = nc.dram_tensor("iota128", [128, 128], f32, kind="ExternalInput")
    ident = nc.dram_tensor("ident", [128, 128], f32, kind="ExternalInput")
    W1t = nc.dram_tensor("W1t", [F_IN, F_HID], f32, kind="ExternalInput")
    b1t = nc.dram_tensor("b1t", [128, F_HID], f32, kind="ExternalInput")
    W2t = nc.dram_tensor("W2t", [F_HID, F_OUT], f32, kind="ExternalInput")
    b2t = nc.dram_tensor("b2t", [1, F_OUT], f32, kind="ExternalInput")
    onest = nc.dram_tensor("onest", [1, 128], f32, kind="ExternalInput")
    out_t = nc.dram_tensor("out", [nsh, F_OUT], f32, kind="ExternalOutput")

    with tile.TileContext(nc) as tc:
        with (
            tc.tile_pool(name="const", bufs=1) as cpool,
            tc.tile_pool(name="dram", bufs=1, space="DRAM") as dram,
            tc.tile_pool(name="gpool", bufs=3) as gp_,
            tc.tile_pool(name="tpool", bufs=3) as tp_,
            tc.tile_pool(name="ohp", bufs=4) as ohp,
            tc.tile_pool(name="agg", bufs=1) as apool,
        ):
            iota_t = cpool.tile([128, 128], f32)
            nc.sync.dma_start(out=iota_t[:], in_=iota128[:, :])
            dloc_t = cpool.tile([128, NCW], f32)
            nc.sync.dma_start(out=dloc_t[:], in_=dloc2[:, :])
            W1s = cpool.tile([F_IN, F_HID], f32)
            nc.sync.dma_start(out=W1s[:], in_=W1t[:, :])
            b1s = cpool.tile([128, F_HID], f32)
            nc.sync.dma_start(out=b1s[:], in_=b1t[:, :])
            W2s = cpool.tile([F_HID, F_OUT], f32)
            nc.sync.dma_start(out=W2s[:], in_=W2t[:, :])
            b2s = cpool.tile([1, F_OUT], f32)
            nc.sync.dma_start(out=b2s[:], in_=b2t[:, :])
            ones_s = cpool.tile([1, 128], f32)
            nc.sync.dma_start(out=ones_s[:], in_=onest[:, :])
            id_s = cpool.tile([128, 128], f32)
            nc.sync.dma_start(out=id_s[:], in_=ident[:, :])

            shard1 = dram.tile([nsh, 64], f32)
            shard2 = dram.tile([nsh, 64], f32)
            table1 = dram.tile([ntab, 64], f32)
            table2 = dram.tile([ntab, 64], f32)

            # ---- h = x @ W1 on own shard -> shard1 ----
            with (
                tc.tile_pool(name="xt", bufs=1) as xp,
                tc.tile_pool(name="ph", bufs=2, space="PSUM") as ph,
            ):
                half = nsh // 2
                for hh in range(2):
                    xTs = xp.tile([F_IN, half], f32, tag="xts")
                    nc.sync.dma_start(out=xTs[:],
                                      in_=xT[:, hh * half:(hh + 1) * half])
                    for tt in range(half // 128):
                        hp = ph.tile([128, F_HID], f32, tag="hps")
                        nc.tensor.matmul(
                            out=hp[:], lhsT=xTs[:, tt * 128:(tt + 1) * 128],
                            rhs=W1s[:], start=True, stop=True,
                        )
                        hs = tp_.tile([128, 64], f32, tag="hsb")
                        nc.vector.memset(hs[:], 0.0)
                        nc.vector.tensor_copy(out=hs[:, :F_HID], in_=hp[:])
                        t = hh * (half // 128) + tt
                        dstp = bass.AP(
                            tensor=shard1.tensor, offset=t * 128 * 64,
                            ap=[[64, 128], [1, 64]],
                        )
                        nc.sync.dma_start(out=dstp, in_=hs[:])

            nc.gpsimd.collective_compute(
                "AllGather", mybir.AluOpType.bypass,
                ins=[shard1.opt()], outs=[table1.opt()],
                replica_groups=[list(range(CORES))],
            )

            def run_layer(table, out1_all, close_fn, psw):
                for ii in range(NI):
                    g = instr_group[ii]
                    kc0 = ii * GK
                    msub = []
                    for j in range(KSLOT):
                        kcj = kc0 + j * SUBK
                        gx = gp_.tile([128, SUBK * 8], mybir.dt.int16,
                                      tag=f"gx{j}", name=f"gx{j}")
                        nc.sync.dma_start(out=gx[:],
                                          in_=gidx[:, kcj * 8:(kcj + SUBK) * 8])
                        ws = gp_.tile([128, SUBK], f32, tag=f"ws{j}",
                                      name=f"ws{j}")
                        nc.sync.dma_start(out=ws[:],
                                          in_=wgrid[:, kcj:kcj + SUBK])
                        mg = gp_.tile([128, SUBK * 32], f32, tag=f"mg{j}",
                                      name=f"mg{j}")
                        emit_dma_gather(
                            nc.gpsimd,
                            out_ap=mg[:].rearrange("p (k f) -> p k f", f=32),
                            in_ap=bass.AP(
                                tensor=table.tensor,
                                offset=g * geo.chunk * 64,
                                ap=[[64, geo.chunk], [1, 32]],
                            ),
                            idxs_ap=gx[:],
                            num_idxs=SUBK * 128,
                            elem_size=32,
                            elem_step=64,
                            queue_num=j,
                        )
                        nc.vector.tensor_tensor(
                            out=mg[:].rearrange("p (k f) -> p k f", f=32),
                            in0=mg[:].rearrange("p (k f) -> p k f", f=32),
                            in1=_b(ws[:], 32), op=mybir.AluOpType.mult,
                        )
                        msub.append(mg)
                    u0 = tp_.tile([128, SUBK * 32], f32, tag="u0")
                    nc.vector.tensor_tensor(out=u0[:], in0=msub[0][:],
                                            in1=msub[1][:],
                                            op=mybir.AluOpType.add)
                    u1 = tp_.tile([128, SUBK * 32], f32, tag="u1")
                    nc.vector.tensor_tensor(out=u1[:], in0=msub[2][:],
                                            in1=msub[3][:],
                                            op=mybir.AluOpType.add)
                    nc.vector.tensor_tensor(
                        out=out1_all[:, ii * SPI // 128 * 32:
                                     (ii + 1) * SPI // 128 * 32],
                        in0=u0[:], in1=u1[:], op=mybir.AluOpType.add,
                    )
                    for v in sched[ii]:
                        cols = win_cols[v]
                        nv = len(cols)
                        cw0 = int(cw_base[v])
                        oh = ohp.tile([128, OHW * 128], bf16, tag="oh")
                        nc.vector.tensor_tensor(
                            out=oh[:, :nv * 128].rearrange(
                                "p (b j) -> p b j", j=128),
                            in0=bass.AP(tensor=iota_t.tensor,
                                        offset=iota_t[:].offset,
                                        ap=[iota_t[:].ap[0], [0, nv], [1, 128]]),
                            in1=bass.AP(tensor=dloc_t.tensor,
                                        offset=dloc_t[:].offset + cw0,
                                        ap=[dloc_t[:].ap[0], [1, nv], [0, 128]]),
                            op=mybir.AluOpType.is_equal,
                        )
                        pwin = psw.tile([128, 512], f32, tag="pwin")
                        for i, col in enumerate(cols):
                            nc.tensor.matmul(
                                out=pwin[:, 0:F_HID],
                                lhsT=oh[:, i * 128:(i + 1) * 128],
                                rhs=out1_all[:, col * 32:(col + 1) * 32],
                                start=(i == 0),
                                stop=(i == nv - 1),
                            )
                        close_fn(v, pwin)

            # ---- layer 1 ----
            l1cm = tc.tile_pool(name="l1s", bufs=1)
            l1pool = l1cm.__enter__()
            out1a = l1pool.tile([128, COLS * 32], bf16, tag="o1a")
            h2 = l1pool.tile([128, nwin * 64], f32, tag="h2")
            nc.vector.memset(h2[:], 0.0)

            def close1(v, pwin):
                tmp = tp_.tile([128, F_HID], f32, tag="clo")
                nc.vector.tensor_tensor(
                    out=tmp[:], in0=pwin[:, 0:F_HID], in1=b1s[:],
                    op=mybir.AluOpType.add,
                )
                nc.scalar.activation(
                    out=h2[:, v * 64:v * 64 + F_HID], in_=tmp[:], func=AF.Relu,
                )

            psw_cm = tc.tile_pool(name="psw", bufs=4, space="PSUM")
            psw = psw_cm.__enter__()
            run_layer(table1, out1a, close1, psw)
            psw_cm.__exit__(None, None, None)
            dst2 = bass.AP(tensor=shard2.tensor, offset=0,
                           ap=[[64, 128], [128 * 64, nwin], [1, 64]])
            nc.sync.dma_start(out=dst2, in_=h2[:].rearrange(
                "p (v f) -> p v f", f=64))
            l1cm.__exit__(None, None, None)
            nc.gpsimd.collective_compute(
                "AllGather", mybir.AluOpType.bypass,
                ins=[shard2.opt()], outs=[table2.opt()],
                replica_groups=[list(range(CORES))],
            )

            # ---- layer 2 ----
            l2cm = tc.tile_pool(name="l2s", bufs=1)
            l2pool = l2cm.__enter__()
            out1b = l2pool.tile([128, COLS * 32], bf16, tag="o1b")
            agg2 = apool.tile([128, nwin * F_HID], f32, tag="agg2")

            def close2(v, pwin):
                nc.vector.tensor_copy(
                    out=agg2[:, v * F_HID:(v + 1) * F_HID],
                    in_=pwin[:, 0:F_HID],
                )

            psw_cm2 = tc.tile_pool(name="psw2", bufs=4, space="PSUM")
            psw2 = psw_cm2.__enter__()
            run_layer(table2, out1b, close2, psw2)
            psw_cm2.__exit__(None, None, None)
            l2cm.__exit__(None, None, None)

            # ---- out = log_softmax(agg2 @ W2 + b2) ----
            zall = apool.tile([128, nwin * F_OUT], f32, tag="zall")
            sall = apool.tile([128, nwin], f32, tag="sall")
            pf_cm = tc.tile_pool(name="pf", bufs=2, space="PSUM")
            pf = pf_cm.__enter__()
            for v in range(nwin):
                tp2 = pf.tile([F_HID, 128], f32, tag="tp")
                nc.tensor.transpose(
                    out=tp2[:], in_=agg2[:, v * F_HID:(v + 1) * F_HID],
                    identity=id_s[:],
                )
                aT = ohp.tile([F_HID, 128], f32, tag="aT")
                nc.vector.tensor_copy(out=aT[:], in_=tp2[:])
                zp = pf.tile([128, F_OUT], f32, tag="zp")
                nc.tensor.matmul(out=zp[:], lhsT=aT[:], rhs=W2s[:],
                                 start=True, stop=False)
                nc.tensor.matmul(out=zp[:], lhsT=ones_s[:], rhs=b2s[:],
                                 start=False, stop=True)
                negm = ohp.tile([128, 1], f32, tag="negm")
                nc.vector.reduce_max(out=negm[:], in_=zp[:], axis=AX,
                                     negate=True)
                nc.vector.tensor_tensor(
                    out=zall[:, v * F_OUT:(v + 1) * F_OUT],
                    in0=zp[:], in1=_bcast_col(negm[:], F_OUT),
                    op=mybir.AluOpType.add,
                )
                etmp = ohp.tile([128, F_OUT], f32, tag="etmp")
                nc.scalar.activation(
                    out=etmp[:], in_=zall[:, v * F_OUT:(v + 1) * F_OUT],
                    func=AF.Exp, accum_out=sall[:, v:v + 1],
                )
            lns = apool.tile([128, nwin], f32, tag="lns")
            nc.scalar.activation(out=lns[:], in_=sall[:], func=AF.Ln)
            for v in range(nwin):
                nc.vector.tensor_tensor(
                    out=zall[:, v * F_OUT:(v + 1) * F_OUT],
                    in0=zall[:, v * F_OUT:(v + 1) * F_OUT],
                    in1=_bcast_col(lns[:, v:v + 1], F_OUT),
                    op=mybir.AluOpType.subtract,
                )
            outdst = bass.AP(
                tensor=out_t, offset=0,
                ap=[[F_OUT, 128], [128 * F_OUT, nwin], [1, F_OUT]],
            )
            nc.sync.dma_start(out=outdst, in_=zall[:].rearrange(
                "p (v f) -> p v f", f=F_OUT))
            pf_cm.__exit__(None, None, None)

    nc.compile()
    return nc


def make_inmaps(meta, inmaps_edges, x, W1, b1, W2, b2):
    geo: Geo = meta["geo"]
    nsh = geo.nsh
    n = geo.n_nodes
    xT_full = np.zeros((F_IN, geo.ntab), np.float32)
    xT_full[:, :n] = np.asarray(x, np.float32).T
    iota = np.tile(np.arange(128, dtype=np.float32)[None, :], (128, 1))
    ident = np.eye(128, dtype=np.float32)
    b1b = np.tile(np.asarray(b1, np.float32)[None, :], (128, 1))
    consts = dict(
        iota128=iota, ident=ident,
        W1t=np.asarray(W1, np.float32), b1t=b1b,
        W2t=np.asarray(W2, np.float32), b2t=np.asarray(b2, np.float32)[None, :],
        onest=np.ones((1, 128), np.float32),
    )
    maps = []
    for c in range(CORES):
        m = dict(inmaps_edges[c])
        m.update(consts)
        m["xT"] = np.ascontiguousarray(xT_full[:, c * nsh:(c + 1) * nsh])
        maps.append(m)
    return maps


_CACHE = {}


def run(x, edge_index, edge_weight, W1, b1, W2, b2, geo=FULL, trace=False):
    key = "geo%d" % geo.n_nodes
    meta, inmaps_edges = pack(edge_index, edge_weight, geo)
    if key in _CACHE:
        nc = _CACHE[key]
    else:
        nc = build(meta)
        _CACHE[key] = nc
    maps = make_inmaps(meta, inmaps_edges, x, W1, b1, W2, b2)
    res = run_bass_kernel_spmd(nc, maps, core_ids=list(range(CORES)), trace=trace)
    n = geo.n_nodes
    out = np.empty((n, F_OUT), np.float32)
    for c in range(CORES):
        lo = c * geo.nsh
        hi = min(lo + geo.nsh, n)
        if hi > lo:
            out[lo:hi] = res.results[c]["out"][: hi - lo]
    return out, res


def kernel(x, edge_index, edge_weight, W1, b1, W2, b2):
    out, _ = run(
        np.asarray(x), np.asarray(edge_index), np.asarray(edge_weight),
        np.asarray(W1), np.asarray(b1), np.asarray(W2), np.asarray(b2),
    )
    return out


# revision 13
# speedup vs baseline: 3.6810x; 1.4420x over previous
"""GCN (2-layer, edge-weighted, log_softmax) on 8 Trainium2 NeuronCores.

v2 strategy (dst-sharded edges, matmul segment-sum, 4-queue SWDGE gather):
  - Nodes sharded 12544/core; fp32 table rows (256B-strided, 32 used) in HBM,
    replicated via AllGather.
  - Edges packed 4-per-slot by (src-chunk, dst). Each 2048-slot instruction
    block issues FOUR 2048-token gathers (one per slot position j) on SWDGE
    queues 0..3 -> concurrent DMA transfers into four separate tiles, so the
    weighted 4->1 reduction is 4 mults + 3 adds, all contiguous two-tile ops.
  - Slot sums (bf16) for the whole layer stay resident in SBUF (out1_all).
  - Scatter runs window-major: once the last instruction touching window v
    completes, a one-shot is_equal builds all of v's one-hot columns (host
    dloc2 template, window-major order), and ~10 bf16 matmuls accumulate
    into a dedicated full PSUM bank (start/stop per window; 4 banks rotate).
    Window closes immediately (bias+relu -> table2 shard, or copy to agg2).
  - Layer 2 reuses the identical edge template on table2 = relu(agg1+b1),
    then W2 + log_softmax per 128-node window.
Host side packs indices/weights (numpy) and concatenates shards.
"""

import os
import sys

for _p in ("/opt/trn_rl_repo", "/root/.axon_site/_ro/trn_rl_repo"):
    if os.path.isdir(_p) and _p not in sys.path:
        sys.path.insert(0, _p)

import numpy as np

import concourse.ap_utils as ap_utils
import concourse.bass as bass
import concourse.mybir as mybir
from concourse import bacc, tile
from concourse.bass_utils import run_bass_kernel_spmd

CORES = 8
F_IN = 128
F_HID = 32
F_OUT = 40
KSLOT = 4      # edges per slot (same destination)
GK = 64        # k-columns per instruction block (8192 tokens, 2048 slots)
SPI = GK * 128 // KSLOT   # slots per instruction block (2048)
SUBK = GK // KSLOT        # k-columns per sub-gather (16 -> 2048 tokens)


class Geo:
    def __init__(self, n_nodes=100000, nsh=12544, chunk=25088, groups=4):
        self.n_nodes = n_nodes
        self.nsh = nsh
        self.ntab = nsh * CORES
        self.chunk = chunk
        self.groups = groups
        assert chunk * groups == self.ntab
        assert nsh % 128 == 0
        self.nwin = nsh // 128


FULL = Geo()


def _wrap16(flat, T):
    a = flat.reshape(T // 16, 16).T
    return np.tile(a, (8, 1)).copy()


def pack(edge_index, edge_weight, geo: Geo):
    src = np.asarray(edge_index[0], dtype=np.int64)
    dst = np.asarray(edge_index[1], dtype=np.int64)
    w = np.asarray(edge_weight, dtype=np.float32)
    nsh, nwin, G = geo.nsh, geo.nwin, geo.groups

    core = dst // nsh
    pc = []
    cnt = np.zeros((CORES, G, nwin), np.int64)
    for c in range(CORES):
        m = core == c
        s_c = src[m]
        dl = dst[m] - c * nsh
        wc = w[m]
        g = s_c // geo.chunk
        key = g * (2 * nsh) + dl
        order = np.argsort(key, kind="stable")
        sk = (s_c - g * geo.chunk)[order]
        dlk = dl[order]
        wk = wc[order]
        kk = key[order]
        new = np.r_[True, kk[1:] != kk[:-1]]
        run_first = np.flatnonzero(new)
        run_len = np.diff(np.r_[run_first, len(kk)])
        run_id = np.cumsum(new) - 1
        rank = np.arange(len(kk)) - run_first[run_id]
        nsl = (run_len + KSLOT - 1) // KSLOT
        g_run = (g[order])[run_first]
        dl_run = dlk[run_first]
        v_run = dl_run // 128
        np.add.at(cnt[c], (g_run, v_run), nsl)
        pc.append((sk, dlk, wk, rank, run_id, nsl, g_run, dl_run, v_run))

    # shared template: exact max-over-cores region capacity; groups padded to
    # whole instruction blocks
    cap = cnt.max(axis=0)  # [G, nwin] slots
    assert (cap.sum(axis=0) > 0).all(), "window with no edges"
    gslots = cap.sum(axis=1)
    gpad = (-gslots) % SPI
    off = np.zeros((G, nwin), np.int64)
    gbase = np.zeros(G + 1, np.int64)
    b = 0
    for g in range(G):
        gbase[g] = b
        for v in range(nwin):
            off[g, v] = b
            b += cap[g, v]
        b += gpad[g]
    gbase[G] = b
    S_T = int(b)
    COLS = S_T // 128
    T = S_T * KSLOT
    KC = T // 128
    NI = S_T // SPI
    instr_group = [int(np.searchsorted(gbase, ii * SPI, side="right") - 1)
                   for ii in range(NI)]

    # window-major colwin template: for each window, its (col) list across
    # groups; complete_at[v] = instruction block that finishes its last region
    win_cols = [[] for _ in range(nwin)]
    for g in range(G):
        for v in range(nwin):
            if cap[g, v] == 0:
                continue
            lo = int(off[g, v])
            hi = lo + int(cap[g, v])
            for col in range(lo // 128, (hi + 127) // 128):
                if col not in win_cols[v]:
                    win_cols[v].append(col)
    complete_at = np.zeros(nwin, np.int64)
    for g in range(G):
        for v in range(nwin):
            if cap[g, v] > 0:
                last = int(off[g, v] + cap[g, v] - 1) // SPI
                complete_at[v] = max(complete_at[v], last)
    sched = [[] for _ in range(NI)]
    for v in range(nwin):
        sched[int(complete_at[v])].append(v)
    # window-major dloc2 column order
    cw_base = np.zeros(nwin + 1, np.int64)
    for v in range(nwin):
        cw_base[v + 1] = cw_base[v] + len(win_cols[v])
    NCW = int(cw_base[nwin])
    OHW = max(len(wc_) for wc_ in win_cols)

    inmaps = []
    for c in range(CORES):
        sk, dlk, wk, rank, run_id, nsl, g_run, dl_run, v_run = pc[c]
        n_runs = len(nsl)
        csum = np.cumsum(nsl)
        start_excl = np.r_[0, csum[:-1]]
        gv = g_run * nwin + v_run
        newgv = np.r_[True, gv[1:] != gv[:-1]]
        gv_first = np.flatnonzero(newgv)
        gv_id = np.cumsum(newgv) - 1
        base_in_gv = start_excl - start_excl[gv_first][gv_id]
        run_slot = off[g_run, v_run] + base_in_gv
        slot_e = run_slot[run_id] + rank // KSLOT
        j_e = rank % KSLOT
        # token position: instruction block ib, local slot ls ->
        #   kc = ib*GK + j*SUBK + ls//128, p = ls%128
        ib = slot_e // SPI
        ls = slot_e - ib * SPI
        kc_e = ib * GK + j_e * SUBK + ls // 128
        p_e = ls % 128
        tok = kc_e * 128 + p_e

        idx_flat = np.zeros(T, np.int16)
        idx_flat[tok] = sk.astype(np.int16)
        w_flat = np.zeros(T, np.float32)
        w_flat[tok] = wk

        dl_slot = np.full(S_T, -1, np.int64)
        reps = np.repeat(np.arange(n_runs), nsl)
        ar = np.arange(len(reps)) - np.repeat(start_excl, nsl)
        pos = np.repeat(run_slot, nsl) + ar
        dl_slot[pos] = np.repeat(dl_run, nsl)
        d2 = np.full((128, NCW), 512.0, np.float32)
        k = 0
        for v in range(nwin):
            for col in win_cols[v]:
                dcol = dl_slot[col * 128:(col + 1) * 128] - 128 * v
                d2[:, k] = np.where((dcol >= 0) & (dcol < 128), dcol, 512)
                k += 1
        inmaps.append(
            dict(
                gidx=_wrap16(idx_flat, T),
                wgrid=w_flat.reshape(KC, 128).T.astype(np.float16),
                dloc2=d2,
            )
        )

    meta = dict(S_T=S_T, COLS=COLS, T=T, KC=KC, NCW=NCW, NI=NI, OHW=OHW,
                win_cols=win_cols, cw_base=cw_base, sched=sched,
                instr_group=instr_group, geo=geo)
    return meta, inmaps


def emit_dma_gather(gp, out_ap, in_ap, idxs_ap, num_idxs, elem_size, elem_step,
                    queue_num=0):
    """bass.dma_gather minus the blanket 256B elem assert (verified on HW that
    the non-transpose ucode handles 128B rows)."""
    from concourse.bass import exact_div

    assert idxs_ap.dtype == mybir.dt.int16
    assert in_ap.dtype == out_ap.dtype
    assert in_ap.space == bass.MemorySpace.DRAM
    stride_bytes_256 = exact_div(elem_step * mybir.dt.size(in_ap.dtype), 256)
    _in_ap = gp.lower_ap_dma(in_ap, for_custom_bir_dma=True)
    _idxs_ap = gp.lower_ap(idxs_ap)
    _out_ap = gp.lower_ap(out_ap)
    return gp.add_instruction(
        mybir.InstDMAGatherAnt(
            name=gp.bass.get_next_instruction_name(),
            ins=[*_in_ap, _idxs_ap, gp.lower_val_access(gp.to_reg(num_idxs))],
            outs=[_out_ap],
            transpose=False,
            num_idxs=num_idxs,
            elem_size=elem_size,
            stride_bytes_256=stride_bytes_256,
            gen_mode=0,
            single_packet=False,
            queue_num=queue_num,
            sbuf_tokens_per_rank=0,
            sbuf_free_dim_per_rank=0,
            sbuf_free_dim_pad_per_rank=0,
            sbuf_byte_offset=0,
        )
    )


def _b(ap2, reps):
    return bass.AP(tensor=ap2.tensor, offset=ap2.offset, ap=[*ap2.ap, [0, reps]])


def _bcast_col(ap1, n):
    return bass.AP(tensor=ap1.tensor, offset=ap1.offset, ap=[ap1.ap[0], [0, n]])


def build(meta):
    geo: Geo = meta["geo"]
    S_T, COLS, T, KC, NCW, NI, OHW = (meta["S_T"], meta["COLS"], meta["T"],
                                      meta["KC"], meta["NCW"], meta["NI"],
                                      meta["OHW"])
    win_cols, cw_base, sched = meta["win_cols"], meta["cw_base"], meta["sched"]
    instr_group = meta["instr_group"]
    nsh, ntab, nwin, G = geo.nsh, geo.ntab, geo.nwin, geo.groups
    f32 = mybir.dt.float32
    f16 = mybir.dt.float16
    bf16 = mybir.dt.bfloat16
    AX = mybir.AxisListType.X
    AF = mybir.ActivationFunctionType

    nc = bacc.Bacc("TRN2", target_bir_lowering=False, debug=False,
                   num_devices=CORES, num_swdge_queues=4)

    xT = nc.dram_tensor("xT", [F_IN, nsh], f32, kind="ExternalInput")
    gidx = nc.dram_tensor("gidx", [128, T // 16], mybir.dt.int16,
                          kind="ExternalInput")
    wgrid = nc.dram_tensor("wgrid", [128, KC], mybir.dt.float16, kind="ExternalInput")
    dloc2 = nc.dram_tensor("dloc2", [128, NCW], f32, kind="ExternalInput")
    iota128 = nc.dram_tensor("iota128", [128, 128], f32, kind="ExternalInput")
    ident = nc.dram_tensor("ident", [128, 128], f32, kind="ExternalInput")
    W1t = nc.dram_tensor("W1t", [F_IN, F_HID], f32, kind="ExternalInput")
    b1t = nc.dram_tensor("b1t", [128, F_HID], f32, kind="ExternalInput")
    W2t = nc.dram_tensor("W2t", [F_HID, F_OUT], f32, kind="ExternalInput")
    b2t = nc.dram_tensor("b2t", [1, F_OUT], f32, kind="ExternalInput")
    onest = nc.dram_tensor("onest", [1, 128], f32, kind="ExternalInput")
    out_t = nc.dram_tensor("out", [nsh, F_OUT], f32, kind="ExternalOutput")

    with tile.TileContext(nc) as tc:
        with (
            tc.tile_pool(name="const", bufs=1) as cpool,
            tc.tile_pool(name="dram", bufs=1, space="DRAM") as dram,
            tc.tile_pool(name="gpool", bufs=3) as gp_,
            tc.tile_pool(name="tpool", bufs=3) as tp_,
            tc.tile_pool(name="ohp", bufs=4) as ohp,
            tc.tile_pool(name="agg", bufs=1) as apool,
        ):
            iota_t = cpool.tile([128, 128], f32)
            nc.sync.dma_start(out=iota_t[:], in_=iota128[:, :])
            dloc_t = cpool.tile([128, NCW], f32)
            nc.sync.dma_start(out=dloc_t[:], in_=dloc2[:, :])
            W1s = cpool.tile([F_IN, F_HID], f32)
            nc.sync.dma_start(out=W1s[:], in_=W1t[:, :])
            b1s = cpool.tile([128, F_HID], f32)
            nc.sync.dma_start(out=b1s[:], in_=b1t[:, :])
            W2s = cpool.tile([F_HID, F_OUT], f32)
            nc.sync.dma_start(out=W2s[:], in_=W2t[:, :])
            b2s = cpool.tile([1, F_OUT], f32)
            nc.sync.dma_start(out=b2s[:], in_=b2t[:, :])
            ones_s = cpool.tile([1, 128], f32)
            nc.sync.dma_start(out=ones_s[:], in_=onest[:, :])
            id_s = cpool.tile([128, 128], f32)
            nc.sync.dma_start(out=id_s[:], in_=ident[:, :])

            shard1 = dram.tile([nsh, 128], f16)
            shard2 = dram.tile([nsh, 128], f16)
            table1 = dram.tile([ntab, 128], f16)
            table2 = dram.tile([ntab, 128], f16)

            # ---- h = x @ W1 on own shard -> shard1 ----
            with (
                tc.tile_pool(name="xt", bufs=1) as xp,
                tc.tile_pool(name="ph", bufs=2, space="PSUM") as ph,
            ):
                half = nsh // 2
                for hh in range(2):
                    xTs = xp.tile([F_IN, half], f32, tag="xts")
                    nc.sync.dma_start(out=xTs[:],
                                      in_=xT[:, hh * half:(hh + 1) * half])
                    for tt in range(half // 128):
                        hp = ph.tile([128, F_HID], f32, tag="hps")
                        nc.tensor.matmul(
                            out=hp[:], lhsT=xTs[:, tt * 128:(tt + 1) * 128],
                            rhs=W1s[:], start=True, stop=True,
                        )
                        hs = tp_.tile([128, 128], f16, tag="hsb")
                        nc.vector.memset(hs[:], 0.0)
                        nc.vector.tensor_copy(out=hs[:, :F_HID], in_=hp[:])
                        t = hh * (half // 128) + tt
                        dstp = bass.AP(
                            tensor=shard1.tensor, offset=t * 128 * 128,
                            ap=[[128, 128], [1, 128]],
                        )
                        nc.sync.dma_start(out=dstp, in_=hs[:])

            nc.gpsimd.collective_compute(
                "AllGather", mybir.AluOpType.bypass,
                ins=[shard1.opt()], outs=[table1.opt()],
                replica_groups=[list(range(CORES))],
            )

            CH = 4  # instruction blocks per input-prefetch chunk

            def run_layer(table, out1_all, close_fn, psw):
                gxc = wsc = None
                for ii in range(NI):
                    if ii % CH == 0:
                        nb_ = min(CH, NI - ii)
                        gxc = gp_.tile([128, CH * GK * 8], mybir.dt.int16,
                                       tag="gxc", name="gxc")
                        nc.sync.dma_start(
                            out=gxc[:, :nb_ * GK * 8],
                            in_=gidx[:, ii * GK * 8:(ii + nb_) * GK * 8])
                        wsc = gp_.tile([128, CH * GK], f16, tag="wsc",
                                       name="wsc")
                        nc.scalar.dma_start(
                            out=wsc[:, :nb_ * GK],
                            in_=wgrid[:, ii * GK:(ii + nb_) * GK])
                    g = instr_group[ii]
                    io = ii % CH
                    msub = []
                    for j in range(KSLOT):
                        oj = io * GK * 8 + j * SUBK * 8
                        mg = gp_.tile([128, SUBK * 32], f16, tag=f"mg{j}",
                                      name=f"mg{j}")
                        emit_dma_gather(
                            nc.gpsimd,
                            out_ap=mg[:].rearrange("p (k f) -> p k f", f=32),
                            in_ap=bass.AP(
                                tensor=table.tensor,
                                offset=g * geo.chunk * 128,
                                ap=[[128, geo.chunk], [1, 32]],
                            ),
                            idxs_ap=gxc[:, oj:oj + SUBK * 8],
                            num_idxs=SUBK * 128,
                            elem_size=32,
                            elem_step=128,
                            queue_num=j,
                        )
                        nc.vector.tensor_tensor(
                            out=mg[:].rearrange("p (k f) -> p k f", f=32),
                            in0=mg[:].rearrange("p (k f) -> p k f", f=32),
                            in1=_b(wsc[:, io * GK + j * SUBK:
                                       io * GK + (j + 1) * SUBK], 32),
                            op=mybir.AluOpType.mult,
                        )
                        msub.append(mg)
                    u0 = tp_.tile([128, SUBK * 32], f16, tag="u0")
                    nc.vector.tensor_tensor(out=u0[:], in0=msub[0][:],
                                            in1=msub[1][:],
                                            op=mybir.AluOpType.add)
                    u1 = tp_.tile([128, SUBK * 32], f16, tag="u1")
                    nc.vector.tensor_tensor(out=u1[:], in0=msub[2][:],
                                            in1=msub[3][:],
                                            op=mybir.AluOpType.add)
                    nc.vector.tensor_tensor(
                        out=out1_all[:, ii * SPI // 128 * 32:
                                     (ii + 1) * SPI // 128 * 32],
                        in0=u0[:], in1=u1[:], op=mybir.AluOpType.add,
                    )
                    for v in sched[ii]:
                        cols = win_cols[v]
                        nv = len(cols)
                        cw0 = int(cw_base[v])
                        oh = ohp.tile([128, OHW * 128], bf16, tag="oh")
                        nc.vector.tensor_tensor(
                            out=oh[:, :nv * 128].rearrange(
                                "p (b j) -> p b j", j=128),
                            in0=bass.AP(tensor=iota_t.tensor,
                                        offset=iota_t[:].offset,
                                        ap=[iota_t[:].ap[0], [0, nv], [1, 128]]),
                            in1=bass.AP(tensor=dloc_t.tensor,
                                        offset=dloc_t[:].offset + cw0,
                                        ap=[dloc_t[:].ap[0], [1, nv], [0, 128]]),
                            op=mybir.AluOpType.is_equal,
                        )
                        pwin = psw.tile([128, 512], f32, tag="pwin")
                        for i, col in enumerate(cols):
                            nc.tensor.matmul(
                                out=pwin[:, 0:F_HID],
                                lhsT=oh[:, i * 128:(i + 1) * 128],
                                rhs=out1_all[:, col * 32:(col + 1) * 32],
                                start=(i == 0),
                                stop=(i == nv - 1),
                            )
                        close_fn(v, pwin)

            # ---- layer 1 ----
            l1cm = tc.tile_pool(name="l1s", bufs=1)
            l1pool = l1cm.__enter__()
            out1a = l1pool.tile([128, COLS * 32], bf16, tag="o1a")
            h2 = l1pool.tile([128, nwin * 128], f16, tag="h2")
            nc.vector.memset(h2[:], 0.0)

            def close1(v, pwin):
                tmp = tp_.tile([128, F_HID], f32, tag="clo")
                nc.vector.tensor_tensor(
                    out=tmp[:], in0=pwin[:, 0:F_HID], in1=b1s[:],
                    op=mybir.AluOpType.add,
                )
                nc.scalar.activation(
                    out=h2[:, v * 128:v * 128 + F_HID], in_=tmp[:],
                    func=AF.Relu,
                )

            psw_cm = tc.tile_pool(name="psw", bufs=4, space="PSUM")
            psw = psw_cm.__enter__()
            run_layer(table1, out1a, close1, psw)
            psw_cm.__exit__(None, None, None)
            dst2 = bass.AP(tensor=shard2.tensor, offset=0,
                           ap=[[128, 128], [128 * 128, nwin], [1, 128]])
            nc.sync.dma_start(out=dst2, in_=h2[:].rearrange(
                "p (v f) -> p v f", f=128))
            l1cm.__exit__(None, None, None)
            nc.gpsimd.collective_compute(
                "AllGather", mybir.AluOpType.bypass,
                ins=[shard2.opt()], outs=[table2.opt()],
                replica_groups=[list(range(CORES))],
            )

            # ---- layer 2 ----
            l2cm = tc.tile_pool(name="l2s", bufs=1)
            l2pool = l2cm.__enter__()
            out1b = l2pool.tile([128, COLS * 32], bf16, tag="o1b")
            agg2 = apool.tile([128, nwin * F_HID], f32, tag="agg2")

            def close2(v, pwin):
                nc.vector.tensor_copy(
                    out=agg2[:, v * F_HID:(v + 1) * F_HID],
                    in_=pwin[:, 0:F_HID],
                )

            psw_cm2 = tc.tile_pool(name="psw2", bufs=4, space="PSUM")
            psw2 = psw_cm2.__enter__()
            run_layer(table2, out1b, close2, psw2)
            psw_cm2.__exit__(None, None, None)
            l2cm.__exit__(None, None, None)

            # ---- out = log_softmax(agg2 @ W2 + b2) ----
            zall = apool.tile([128, nwin * F_OUT], f32, tag="zall")
            sall = apool.tile([128, nwin], f32, tag="sall")
            pf_cm = tc.tile_pool(name="pf", bufs=2, space="PSUM")
            pf = pf_cm.__enter__()
            for v in range(nwin):
                tp2 = pf.tile([F_HID, 128], f32, tag="tp")
                nc.tensor.transpose(
                    out=tp2[:], in_=agg2[:, v * F_HID:(v + 1) * F_HID],
                    identity=id_s[:],
                )
                aT = ohp.tile([F_HID, 128], f32, tag="aT")
                nc.vector.tensor_copy(out=aT[:], in_=tp2[:])
                zp = pf.tile([128, F_OUT], f32, tag="zp")
                nc.tensor.matmul(out=zp[:], lhsT=aT[:], rhs=W2s[:],
                                 start=True, stop=False)
                nc.tensor.matmul(out=zp[:], lhsT=ones_s[:], rhs=b2s[:],
                                 start=False, stop=True)
                negm = ohp.tile([128, 1], f32, tag="negm")
                nc.vector.reduce_max(out=negm[:], in_=zp[:], axis=AX,
                                     negate=True)
                nc.vector.tensor_tensor(
                    out=zall[:, v * F_OUT:(v + 1) * F_OUT],
                    in0=zp[:], in1=_bcast_col(negm[:], F_OUT),
                    op=mybir.AluOpType.add,
                )
                etmp = ohp.tile([128, F_OUT], f32, tag="etmp")
                nc.scalar.activation(
                    out=etmp[:], in_=zall[:, v * F_OUT:(v + 1) * F_OUT],
                    func=AF.Exp, accum_out=sall[:, v:v + 1],
                )
            lns = apool.tile([128, nwin], f32, tag="lns")
            nc.scalar.activation(out=lns[:], in_=sall[:], func=AF.Ln)
            for v in range(nwin):
                nc.vector.tensor_tensor(
                    out=zall[:, v * F_OUT:(v + 1) * F_OUT],
                    in0=zall[:, v * F_OUT:(v + 1) * F_OUT],
                    in1=_bcast_col(lns[:, v:v + 1], F_OUT),
                    op=mybir.AluOpType.subtract,
                )
            outdst = bass.AP(
                tensor=out_t, offset=0,
                ap=[[F_OUT, 128], [128 * F_OUT, nwin], [1, F_OUT]],
            )
            nc.sync.dma_start(out=outdst, in_=zall[:].rearrange(
                "p (v f) -> p v f", f=F_OUT))
            pf_cm.__exit__(None, None, None)

    nc.compile()
    return nc


def make_inmaps(meta, inmaps_edges, x, W1, b1, W2, b2):
    geo: Geo = meta["geo"]
    nsh = geo.nsh
    n = geo.n_nodes
    xT_full = np.zeros((F_IN, geo.ntab), np.float32)
    xT_full[:, :n] = np.asarray(x, np.float32).T
    iota = np.tile(np.arange(128, dtype=np.float32)[None, :], (128, 1))
    ident = np.eye(128, dtype=np.float32)
    b1b = np.tile(np.asarray(b1, np.float32)[None, :], (128, 1))
    consts = dict(
        iota128=iota, ident=ident,
        W1t=np.asarray(W1, np.float32), b1t=b1b,
        W2t=np.asarray(W2, np.float32), b2t=np.asarray(b2, np.float32)[None, :],
        onest=np.ones((1, 128), np.float32),
    )
    maps = []
    for c in range(CORES):
        m = dict(inmaps_edges[c])
        m.update(consts)
        m["xT"] = np.ascontiguousarray(xT_full[:, c * nsh:(c + 1) * nsh])
        maps.append(m)
    return maps


_CACHE = {}


def run(x, edge_index, edge_weight, W1, b1, W2, b2, geo=FULL, trace=False):
    key = "geo%d" % geo.n_nodes
    meta, inmaps_edges = pack(edge_index, edge_weight, geo)
    if key in _CACHE:
        nc = _CACHE[key]
    else:
        nc = build(meta)
        _CACHE[key] = nc
    maps = make_inmaps(meta, inmaps_edges, x, W1, b1, W2, b2)
    res = run_bass_kernel_spmd(nc, maps, core_ids=list(range(CORES)), trace=trace)
    n = geo.n_nodes
    out = np.empty((n, F_OUT), np.float32)
    for c in range(CORES):
        lo = c * geo.nsh
        hi = min(lo + geo.nsh, n)
        if hi > lo:
            out[lo:hi] = res.results[c]["out"][: hi - lo]
    return out, res


def kernel(x, edge_index, edge_weight, W1, b1, W2, b2):
    out, _ = run(
        np.asarray(x), np.asarray(edge_index), np.asarray(edge_weight),
        np.asarray(W1), np.asarray(b1), np.asarray(W2), np.asarray(b2),
    )
    return out


# revision 15
# speedup vs baseline: 3.7875x; 1.0289x over previous
"""GCN (2-layer, edge-weighted, log_softmax) on 8 Trainium2 NeuronCores.

v2 strategy (dst-sharded edges, matmul segment-sum, 4-queue SWDGE gather):
  - Nodes sharded 12544/core; fp32 table rows (256B-strided, 32 used) in HBM,
    replicated via AllGather.
  - Edges packed 4-per-slot by (src-chunk, dst). Each 2048-slot instruction
    block issues FOUR 2048-token gathers (one per slot position j) on SWDGE
    queues 0..3 -> concurrent DMA transfers into four separate tiles, so the
    weighted 4->1 reduction is 4 mults + 3 adds, all contiguous two-tile ops.
  - Slot sums (bf16) for the whole layer stay resident in SBUF (out1_all).
  - Scatter runs window-major: once the last instruction touching window v
    completes, a one-shot is_equal builds all of v's one-hot columns (host
    dloc2 template, window-major order), and ~10 bf16 matmuls accumulate
    into a dedicated full PSUM bank (start/stop per window; 4 banks rotate).
    Window closes immediately (bias+relu -> table2 shard, or copy to agg2).
  - Layer 2 reuses the identical edge template on table2 = relu(agg1+b1),
    then W2 + log_softmax per 128-node window.
Host side packs indices/weights (numpy) and concatenates shards.
"""

import os
import sys

for _p in ("/opt/trn_rl_repo", "/root/.axon_site/_ro/trn_rl_repo"):
    if os.path.isdir(_p) and _p not in sys.path:
        sys.path.insert(0, _p)

import numpy as np

import concourse.ap_utils as ap_utils
import concourse.bass as bass
import concourse.mybir as mybir
from concourse import bacc, tile
from concourse.bass_utils import run_bass_kernel_spmd

CORES = 8
F_IN = 128
F_HID = 32
F_OUT = 40
KSLOT = 4      # edges per slot (same destination)
GK = 32        # k-columns per instruction block (4096 tokens, 1024 slots)
NCOH = 4       # window cohorts (striped region order for scatter overlap)
SPI = GK * 128 // KSLOT   # slots per instruction block (2048)
SUBK = GK // KSLOT        # k-columns per sub-gather (16 -> 2048 tokens)


class Geo:
    def __init__(self, n_nodes=100000, nsh=12544, chunk=25088, groups=4):
        self.n_nodes = n_nodes
        self.nsh = nsh
        self.ntab = nsh * CORES
        self.chunk = chunk
        self.groups = groups
        assert chunk * groups == self.ntab
        assert nsh % 128 == 0
        self.nwin = nsh // 128


FULL = Geo()


def _wrap16(flat, T):
    a = flat.reshape(T // 16, 16).T
    return np.tile(a, (8, 1)).copy()


def pack(edge_index, edge_weight, geo: Geo):
    src = np.asarray(edge_index[0], dtype=np.int64)
    dst = np.asarray(edge_index[1], dtype=np.int64)
    w = np.asarray(edge_weight, dtype=np.float32)
    nsh, nwin, G = geo.nsh, geo.nwin, geo.groups

    core = dst // nsh
    pc = []
    cnt = np.zeros((CORES, G, nwin), np.int64)
    for c in range(CORES):
        m = core == c
        s_c = src[m]
        dl = dst[m] - c * nsh
        wc = w[m]
        g = s_c // geo.chunk
        key = g * (2 * nsh) + dl
        order = np.argsort(key, kind="stable")
        sk = (s_c - g * geo.chunk)[order]
        dlk = dl[order]
        wk = wc[order]
        kk = key[order]
        new = np.r_[True, kk[1:] != kk[:-1]]
        run_first = np.flatnonzero(new)
        run_len = np.diff(np.r_[run_first, len(kk)])
        run_id = np.cumsum(new) - 1
        rank = np.arange(len(kk)) - run_first[run_id]
        nsl = (run_len + KSLOT - 1) // KSLOT
        g_run = (g[order])[run_first]
        dl_run = dlk[run_first]
        v_run = dl_run // 128
        np.add.at(cnt[c], (g_run, v_run), nsl)
        pc.append((sk, dlk, wk, rank, run_id, nsl, g_run, dl_run, v_run))

    # shared template: exact max-over-cores region capacity; region order is
    # (cohort, group, window) so each cohort's windows complete mid-layer;
    # every (cohort, group) segment is padded to whole instruction blocks
    cap = cnt.max(axis=0)  # [G, nwin] slots
    assert (cap.sum(axis=0) > 0).all(), "window with no edges"
    cohorts = np.array_split(np.arange(nwin), NCOH)
    off = np.zeros((G, nwin), np.int64)
    seg_bounds = []  # (start, end, g)
    b = 0
    for ch in range(NCOH):
        for g in range(G):
            s0 = b
            for v in cohorts[ch]:
                off[g, v] = b
                b += cap[g, v]
            b = -(-b // SPI) * SPI
            seg_bounds.append((s0, b, g))
    S_T = int(b)
    COLS = S_T // 128
    T = S_T * KSLOT
    KC = T // 128
    NI = S_T // SPI
    instr_group = np.zeros(NI, np.int64)
    for s0, s1, g in seg_bounds:
        instr_group[s0 // SPI:s1 // SPI] = g
    instr_group = [int(x) for x in instr_group]

    # window-major colwin template: for each window, its (col) list across
    # groups; complete_at[v] = instruction block that finishes its last region
    win_cols = [[] for _ in range(nwin)]
    for g in range(G):
        for v in range(nwin):
            if cap[g, v] == 0:
                continue
            lo = int(off[g, v])
            hi = lo + int(cap[g, v])
            for col in range(lo // 128, (hi + 127) // 128):
                if col not in win_cols[v]:
                    win_cols[v].append(col)
    complete_at = np.zeros(nwin, np.int64)
    for g in range(G):
        for v in range(nwin):
            if cap[g, v] > 0:
                last = int(off[g, v] + cap[g, v] - 1) // SPI
                complete_at[v] = max(complete_at[v], last)
    sched = [[] for _ in range(NI)]
    for v in range(nwin):
        sched[int(complete_at[v])].append(v)
    # window-major dloc2 column order
    cw_base = np.zeros(nwin + 1, np.int64)
    for v in range(nwin):
        cw_base[v + 1] = cw_base[v] + len(win_cols[v])
    NCW = int(cw_base[nwin])
    OHW = max(len(wc_) for wc_ in win_cols)

    inmaps = []
    for c in range(CORES):
        sk, dlk, wk, rank, run_id, nsl, g_run, dl_run, v_run = pc[c]
        n_runs = len(nsl)
        csum = np.cumsum(nsl)
        start_excl = np.r_[0, csum[:-1]]
        gv = g_run * nwin + v_run
        newgv = np.r_[True, gv[1:] != gv[:-1]]
        gv_first = np.flatnonzero(newgv)
        gv_id = np.cumsum(newgv) - 1
        base_in_gv = start_excl - start_excl[gv_first][gv_id]
        run_slot = off[g_run, v_run] + base_in_gv
        slot_e = run_slot[run_id] + rank // KSLOT
        j_e = rank % KSLOT
        # token position: instruction block ib, local slot ls ->
        #   kc = ib*GK + j*SUBK + ls//128, p = ls%128
        ib = slot_e // SPI
        ls = slot_e - ib * SPI
        kc_e = ib * GK + j_e * SUBK + ls // 128
        p_e = ls % 128
        tok = kc_e * 128 + p_e

        idx_flat = np.zeros(T, np.int16)
        idx_flat[tok] = sk.astype(np.int16)
        w_flat = np.zeros(T, np.float32)
        w_flat[tok] = wk

        dl_slot = np.full(S_T, -1, np.int64)
        reps = np.repeat(np.arange(n_runs), nsl)
        ar = np.arange(len(reps)) - np.repeat(start_excl, nsl)
        pos = np.repeat(run_slot, nsl) + ar
        dl_slot[pos] = np.repeat(dl_run, nsl)
        d2 = np.full((128, NCW), 512.0, np.float32)
        k = 0
        for v in range(nwin):
            for col in win_cols[v]:
                dcol = dl_slot[col * 128:(col + 1) * 128] - 128 * v
                d2[:, k] = np.where((dcol >= 0) & (dcol < 128), dcol, 512)
                k += 1
        inmaps.append(
            dict(
                gidx=_wrap16(idx_flat, T),
                wgrid=w_flat.reshape(KC, 128).T.astype(np.float16),
                dloc2=d2,
            )
        )

    meta = dict(S_T=S_T, COLS=COLS, T=T, KC=KC, NCW=NCW, NI=NI, OHW=OHW,
                win_cols=win_cols, cw_base=cw_base, sched=sched,
                instr_group=instr_group, geo=geo)
    return meta, inmaps


def emit_dma_gather(gp, out_ap, in_ap, idxs_ap, num_idxs, elem_size, elem_step,
                    queue_num=0):
    """bass.dma_gather minus the blanket 256B elem assert (verified on HW that
    the non-transpose ucode handles 128B rows)."""
    from concourse.bass import exact_div

    assert idxs_ap.dtype == mybir.dt.int16
    assert in_ap.dtype == out_ap.dtype
    assert in_ap.space == bass.MemorySpace.DRAM
    stride_bytes_256 = exact_div(elem_step * mybir.dt.size(in_ap.dtype), 256)
    _in_ap = gp.lower_ap_dma(in_ap, for_custom_bir_dma=True)
    _idxs_ap = gp.lower_ap(idxs_ap)
    _out_ap = gp.lower_ap(out_ap)
    return gp.add_instruction(
        mybir.InstDMAGatherAnt(
            name=gp.bass.get_next_instruction_name(),
            ins=[*_in_ap, _idxs_ap, gp.lower_val_access(gp.to_reg(num_idxs))],
            outs=[_out_ap],
            transpose=False,
            num_idxs=num_idxs,
            elem_size=elem_size,
            stride_bytes_256=stride_bytes_256,
            gen_mode=0,
            single_packet=False,
            queue_num=queue_num,
            sbuf_tokens_per_rank=0,
            sbuf_free_dim_per_rank=0,
            sbuf_free_dim_pad_per_rank=0,
            sbuf_byte_offset=0,
        )
    )


def _b(ap2, reps):
    return bass.AP(tensor=ap2.tensor, offset=ap2.offset, ap=[*ap2.ap, [0, reps]])


def _bcast_col(ap1, n):
    return bass.AP(tensor=ap1.tensor, offset=ap1.offset, ap=[ap1.ap[0], [0, n]])


def build(meta):
    geo: Geo = meta["geo"]
    S_T, COLS, T, KC, NCW, NI, OHW = (meta["S_T"], meta["COLS"], meta["T"],
                                      meta["KC"], meta["NCW"], meta["NI"],
                                      meta["OHW"])
    win_cols, cw_base, sched = meta["win_cols"], meta["cw_base"], meta["sched"]
    instr_group = meta["instr_group"]
    nsh, ntab, nwin, G = geo.nsh, geo.ntab, geo.nwin, geo.groups
    f32 = mybir.dt.float32
    f16 = mybir.dt.float16
    bf16 = mybir.dt.bfloat16
    AX = mybir.AxisListType.X
    AF = mybir.ActivationFunctionType

    nc = bacc.Bacc("TRN2", target_bir_lowering=False, debug=False,
                   num_devices=CORES, num_swdge_queues=4)

    xT = nc.dram_tensor("xT", [F_IN, nsh], f32, kind="ExternalInput")
    gidx = nc.dram_tensor("gidx", [128, T // 16], mybir.dt.int16,
                          kind="ExternalInput")
    wgrid = nc.dram_tensor("wgrid", [128, KC], mybir.dt.float16, kind="ExternalInput")
    dloc2 = nc.dram_tensor("dloc2", [128, NCW], f32, kind="ExternalInput")
    iota128 = nc.dram_tensor("iota128", [128, 128], f32, kind="ExternalInput")
    ident = nc.dram_tensor("ident", [128, 128], f32, kind="ExternalInput")
    W1t = nc.dram_tensor("W1t", [F_IN, F_HID], f32, kind="ExternalInput")
    b1t = nc.dram_tensor("b1t", [128, F_HID], f32, kind="ExternalInput")
    W2t = nc.dram_tensor("W2t", [F_HID, F_OUT], f32, kind="ExternalInput")
    b2t = nc.dram_tensor("b2t", [1, F_OUT], f32, kind="ExternalInput")
    onest = nc.dram_tensor("onest", [1, 128], f32, kind="ExternalInput")
    out_t = nc.dram_tensor("out", [nsh, F_OUT], f32, kind="ExternalOutput")

    with tile.TileContext(nc) as tc:
        with (
            tc.tile_pool(name="const", bufs=1) as cpool,
            tc.tile_pool(name="dram", bufs=1, space="DRAM") as dram,
            tc.tile_pool(name="gpool", bufs=6) as gp_,
            tc.tile_pool(name="tpool", bufs=4) as tp_,
            tc.tile_pool(name="ohp", bufs=4) as ohp,
            tc.tile_pool(name="agg", bufs=1) as apool,
        ):
            iota_t = cpool.tile([128, 128], f32)
            nc.sync.dma_start(out=iota_t[:], in_=iota128[:, :])
            dloc_t = cpool.tile([128, NCW], f32)
            nc.sync.dma_start(out=dloc_t[:], in_=dloc2[:, :])
            W1s = cpool.tile([F_IN, F_HID], f32)
            nc.sync.dma_start(out=W1s[:], in_=W1t[:, :])
            b1s = cpool.tile([128, F_HID], f32)
            nc.sync.dma_start(out=b1s[:], in_=b1t[:, :])
            W2s = cpool.tile([F_HID, F_OUT], f32)
            nc.sync.dma_start(out=W2s[:], in_=W2t[:, :])
            b2s = cpool.tile([1, F_OUT], f32)
            nc.sync.dma_start(out=b2s[:], in_=b2t[:, :])
            ones_s = cpool.tile([1, 128], f32)
            nc.sync.dma_start(out=ones_s[:], in_=onest[:, :])
            id_s = cpool.tile([128, 128], f32)
            nc.sync.dma_start(out=id_s[:], in_=ident[:, :])

            shard1 = dram.tile([nsh, 128], f16)
            shard2 = dram.tile([nsh, 128], f16)
            table1 = dram.tile([ntab, 128], f16)
            table2 = dram.tile([ntab, 128], f16)

            # ---- h = x @ W1 on own shard -> shard1 ----
            with (
                tc.tile_pool(name="xt", bufs=1) as xp,
                tc.tile_pool(name="ph", bufs=2, space="PSUM") as ph,
            ):
                half = nsh // 2
                for hh in range(2):
                    xTs = xp.tile([F_IN, half], f32, tag="xts")
                    nc.sync.dma_start(out=xTs[:],
                                      in_=xT[:, hh * half:(hh + 1) * half])
                    for tt in range(half // 128):
                        hp = ph.tile([128, F_HID], f32, tag="hps")
                        nc.tensor.matmul(
                            out=hp[:], lhsT=xTs[:, tt * 128:(tt + 1) * 128],
                            rhs=W1s[:], start=True, stop=True,
                        )
                        hs = tp_.tile([128, 128], f16, tag="hsb")
                        nc.vector.memset(hs[:], 0.0)
                        nc.vector.tensor_copy(out=hs[:, :F_HID], in_=hp[:])
                        t = hh * (half // 128) + tt
                        dstp = bass.AP(
                            tensor=shard1.tensor, offset=t * 128 * 128,
                            ap=[[128, 128], [1, 128]],
                        )
                        nc.sync.dma_start(out=dstp, in_=hs[:])

            nc.gpsimd.collective_compute(
                "AllGather", mybir.AluOpType.bypass,
                ins=[shard1.opt()], outs=[table1.opt()],
                replica_groups=[list(range(CORES))],
            )

            CH = 8  # instruction blocks per input-prefetch chunk

            def run_layer(table, out1_all, close_fn, psw):
                gxc = wsc = None
                for ii in range(NI):
                    if ii % CH == 0:
                        nb_ = min(CH, NI - ii)
                        gxc = gp_.tile([128, CH * GK * 8], mybir.dt.int16,
                                       tag="gxc", name="gxc")
                        nc.sync.dma_start(
                            out=gxc[:, :nb_ * GK * 8],
                            in_=gidx[:, ii * GK * 8:(ii + nb_) * GK * 8])
                        wsc = gp_.tile([128, CH * GK], f16, tag="wsc",
                                       name="wsc")
                        nc.scalar.dma_start(
                            out=wsc[:, :nb_ * GK],
                            in_=wgrid[:, ii * GK:(ii + nb_) * GK])
                    g = instr_group[ii]
                    io = ii % CH
                    msub = []
                    for j in range(KSLOT):
                        oj = io * GK * 8 + j * SUBK * 8
                        mg = gp_.tile([128, SUBK * 32], f16, tag=f"mg{j}",
                                      name=f"mg{j}")
                        emit_dma_gather(
                            nc.gpsimd,
                            out_ap=mg[:].rearrange("p (k f) -> p k f", f=32),
                            in_ap=bass.AP(
                                tensor=table.tensor,
                                offset=g * geo.chunk * 128,
                                ap=[[128, geo.chunk], [1, 32]],
                            ),
                            idxs_ap=gxc[:, oj:oj + SUBK * 8],
                            num_idxs=SUBK * 128,
                            elem_size=32,
                            elem_step=128,
                            queue_num=j,
                        )
                        nc.vector.tensor_tensor(
                            out=mg[:].rearrange("p (k f) -> p k f", f=32),
                            in0=mg[:].rearrange("p (k f) -> p k f", f=32),
                            in1=_b(wsc[:, io * GK + j * SUBK:
                                       io * GK + (j + 1) * SUBK], 32),
                            op=mybir.AluOpType.mult,
                        )
                        msub.append(mg)
                    u0 = tp_.tile([128, SUBK * 32], f16, tag="u0")
                    nc.vector.tensor_tensor(out=u0[:], in0=msub[0][:],
                                            in1=msub[1][:],
                                            op=mybir.AluOpType.add)
                    u1 = tp_.tile([128, SUBK * 32], f16, tag="u1")
                    nc.vector.tensor_tensor(out=u1[:], in0=msub[2][:],
                                            in1=msub[3][:],
                                            op=mybir.AluOpType.add)
                    nc.vector.tensor_tensor(
                        out=out1_all[:, ii * SPI // 128 * 32:
                                     (ii + 1) * SPI // 128 * 32],
                        in0=u0[:], in1=u1[:], op=mybir.AluOpType.add,
                    )
                    for v in sched[ii]:
                        cols = win_cols[v]
                        nv = len(cols)
                        cw0 = int(cw_base[v])
                        oh = ohp.tile([128, OHW * 128], bf16, tag="oh")
                        nc.vector.tensor_tensor(
                            out=oh[:, :nv * 128].rearrange(
                                "p (b j) -> p b j", j=128),
                            in0=bass.AP(tensor=iota_t.tensor,
                                        offset=iota_t[:].offset,
                                        ap=[iota_t[:].ap[0], [0, nv], [1, 128]]),
                            in1=bass.AP(tensor=dloc_t.tensor,
                                        offset=dloc_t[:].offset + cw0,
                                        ap=[dloc_t[:].ap[0], [1, nv], [0, 128]]),
                            op=mybir.AluOpType.is_equal,
                        )
                        pwin = psw.tile([128, 512], f32, tag="pwin")
                        for i, col in enumerate(cols):
                            nc.tensor.matmul(
                                out=pwin[:, 0:F_HID],
                                lhsT=oh[:, i * 128:(i + 1) * 128],
                                rhs=out1_all[:, col * 32:(col + 1) * 32],
                                start=(i == 0),
                                stop=(i == nv - 1),
                            )
                        close_fn(v, pwin)

            # ---- layer 1 ----
            l1cm = tc.tile_pool(name="l1s", bufs=1)
            l1pool = l1cm.__enter__()
            out1a = l1pool.tile([128, COLS * 32], bf16, tag="o1a")
            h2 = l1pool.tile([128, nwin * 128], f16, tag="h2")
            nc.vector.memset(h2[:], 0.0)

            def close1(v, pwin):
                tmp = tp_.tile([128, F_HID], f32, tag="clo")
                nc.vector.tensor_tensor(
                    out=tmp[:], in0=pwin[:, 0:F_HID], in1=b1s[:],
                    op=mybir.AluOpType.add,
                )
                nc.scalar.activation(
                    out=h2[:, v * 128:v * 128 + F_HID], in_=tmp[:],
                    func=AF.Relu,
                )

            psw_cm = tc.tile_pool(name="psw", bufs=4, space="PSUM")
            psw = psw_cm.__enter__()
            run_layer(table1, out1a, close1, psw)
            psw_cm.__exit__(None, None, None)
            dst2 = bass.AP(tensor=shard2.tensor, offset=0,
                           ap=[[128, 128], [128 * 128, nwin], [1, 128]])
            nc.sync.dma_start(out=dst2, in_=h2[:].rearrange(
                "p (v f) -> p v f", f=128))
            l1cm.__exit__(None, None, None)
            nc.gpsimd.collective_compute(
                "AllGather", mybir.AluOpType.bypass,
                ins=[shard2.opt()], outs=[table2.opt()],
                replica_groups=[list(range(CORES))],
            )

            # ---- layer 2 (close folds in W2 + bias + exp/max pieces) ----
            l2cm = tc.tile_pool(name="l2s", bufs=1)
            l2pool = l2cm.__enter__()
            out1b = l2pool.tile([128, COLS * 32], bf16, tag="o1b")
            zall = apool.tile([128, nwin * F_OUT], f32, tag="zall")
            sall = apool.tile([128, nwin], f32, tag="sall")
            pf_cm = tc.tile_pool(name="pf", bufs=2, space="PSUM")
            pf = pf_cm.__enter__()

            def close2(v, pwin):
                ag = ohp.tile([128, F_HID], f32, tag="ag")
                nc.vector.tensor_copy(out=ag[:], in_=pwin[:, 0:F_HID])
                tp2 = pf.tile([F_HID, 128], f32, tag="tp")
                nc.tensor.transpose(out=tp2[:], in_=ag[:], identity=id_s[:])
                aT = ohp.tile([F_HID, 128], f32, tag="aT")
                nc.vector.tensor_copy(out=aT[:], in_=tp2[:])
                zp = pf.tile([128, F_OUT], f32, tag="zp")
                nc.tensor.matmul(out=zp[:], lhsT=aT[:], rhs=W2s[:],
                                 start=True, stop=False)
                nc.tensor.matmul(out=zp[:], lhsT=ones_s[:], rhs=b2s[:],
                                 start=False, stop=True)
                negm = ohp.tile([128, 1], f32, tag="negm")
                nc.vector.reduce_max(out=negm[:], in_=zp[:], axis=AX,
                                     negate=True)
                nc.vector.tensor_tensor(
                    out=zall[:, v * F_OUT:(v + 1) * F_OUT],
                    in0=zp[:], in1=_bcast_col(negm[:], F_OUT),
                    op=mybir.AluOpType.add,
                )
                etmp = ohp.tile([128, F_OUT], f32, tag="etmp")
                nc.scalar.activation(
                    out=etmp[:], in_=zall[:, v * F_OUT:(v + 1) * F_OUT],
                    func=AF.Exp, accum_out=sall[:, v:v + 1],
                )

            psw_cm2 = tc.tile_pool(name="psw2", bufs=4, space="PSUM")
            psw2 = psw_cm2.__enter__()
            run_layer(table2, out1b, close2, psw2)
            psw_cm2.__exit__(None, None, None)
            l2cm.__exit__(None, None, None)

            lns = apool.tile([128, nwin], f32, tag="lns")
            nc.scalar.activation(out=lns[:], in_=sall[:], func=AF.Ln)
            for v in range(nwin):
                nc.vector.tensor_tensor(
                    out=zall[:, v * F_OUT:(v + 1) * F_OUT],
                    in0=zall[:, v * F_OUT:(v + 1) * F_OUT],
                    in1=_bcast_col(lns[:, v:v + 1], F_OUT),
                    op=mybir.AluOpType.subtract,
                )
            outdst = bass.AP(
                tensor=out_t, offset=0,
                ap=[[F_OUT, 128], [128 * F_OUT, nwin], [1, F_OUT]],
            )
            nc.sync.dma_start(out=outdst, in_=zall[:].rearrange(
                "p (v f) -> p v f", f=F_OUT))
            pf_cm.__exit__(None, None, None)

    nc.compile()
    return nc


def make_inmaps(meta, inmaps_edges, x, W1, b1, W2, b2):
    geo: Geo = meta["geo"]
    nsh = geo.nsh
    n = geo.n_nodes
    xT_full = np.zeros((F_IN, geo.ntab), np.float32)
    xT_full[:, :n] = np.asarray(x, np.float32).T
    iota = np.tile(np.arange(128, dtype=np.float32)[None, :], (128, 1))
    ident = np.eye(128, dtype=np.float32)
    b1b = np.tile(np.asarray(b1, np.float32)[None, :], (128, 1))
    consts = dict(
        iota128=iota, ident=ident,
        W1t=np.asarray(W1, np.float32), b1t=b1b,
        W2t=np.asarray(W2, np.float32), b2t=np.asarray(b2, np.float32)[None, :],
        onest=np.ones((1, 128), np.float32),
    )
    maps = []
    for c in range(CORES):
        m = dict(inmaps_edges[c])
        m.update(consts)
        m["xT"] = np.ascontiguousarray(xT_full[:, c * nsh:(c + 1) * nsh])
        maps.append(m)
    return maps


_CACHE = {}


def run(x, edge_index, edge_weight, W1, b1, W2, b2, geo=FULL, trace=False):
    key = "geo%d" % geo.n_nodes
    meta, inmaps_edges = pack(edge_index, edge_weight, geo)
    if key in _CACHE:
        nc = _CACHE[key]
    else:
        nc = build(meta)
        _CACHE[key] = nc
    maps = make_inmaps(meta, inmaps_edges, x, W1, b1, W2, b2)
    res = run_bass_kernel_spmd(nc, maps, core_ids=list(range(CORES)), trace=trace)
    n = geo.n_nodes
    out = np.empty((n, F_OUT), np.float32)
    for c in range(CORES):
        lo = c * geo.nsh
        hi = min(lo + geo.nsh, n)
        if hi > lo:
            out[lo:hi] = res.results[c]["out"][: hi - lo]
    return out, res


def kernel(x, edge_index, edge_weight, W1, b1, W2, b2):
    out, _ = run(
        np.asarray(x), np.asarray(edge_index), np.asarray(edge_weight),
        np.asarray(W1), np.asarray(b1), np.asarray(W2), np.asarray(b2),
    )
    return out


# revision 16
# speedup vs baseline: 3.8154x; 1.0073x over previous
"""GCN (2-layer, edge-weighted, log_softmax) on 8 Trainium2 NeuronCores.

v2 strategy (dst-sharded edges, matmul segment-sum, 4-queue SWDGE gather):
  - Nodes sharded 12544/core; fp32 table rows (256B-strided, 32 used) in HBM,
    replicated via AllGather.
  - Edges packed 4-per-slot by (src-chunk, dst). Each 2048-slot instruction
    block issues FOUR 2048-token gathers (one per slot position j) on SWDGE
    queues 0..3 -> concurrent DMA transfers into four separate tiles, so the
    weighted 4->1 reduction is 4 mults + 3 adds, all contiguous two-tile ops.
  - Slot sums (bf16) for the whole layer stay resident in SBUF (out1_all).
  - Scatter runs window-major: once the last instruction touching window v
    completes, a one-shot is_equal builds all of v's one-hot columns (host
    dloc2 template, window-major order), and ~10 bf16 matmuls accumulate
    into a dedicated full PSUM bank (start/stop per window; 4 banks rotate).
    Window closes immediately (bias+relu -> table2 shard, or copy to agg2).
  - Layer 2 reuses the identical edge template on table2 = relu(agg1+b1),
    then W2 + log_softmax per 128-node window.
Host side packs indices/weights (numpy) and concatenates shards.
"""

import os
import sys

for _p in ("/opt/trn_rl_repo", "/root/.axon_site/_ro/trn_rl_repo"):
    if os.path.isdir(_p) and _p not in sys.path:
        sys.path.insert(0, _p)

import numpy as np

import concourse.ap_utils as ap_utils
import concourse.bass as bass
import concourse.mybir as mybir
from concourse import bacc, tile
from concourse.bass_utils import run_bass_kernel_spmd

CORES = 8
F_IN = 128
F_HID = 32
F_OUT = 40
KSLOT = 4      # edges per slot (same destination)
GK = 32        # k-columns per instruction block (4096 tokens, 1024 slots)
NCOH = 4       # window cohorts (striped region order for scatter overlap)
SPI = GK * 128 // KSLOT   # slots per instruction block (2048)
SUBK = GK // KSLOT        # k-columns per sub-gather (16 -> 2048 tokens)


class Geo:
    def __init__(self, n_nodes=100000, nsh=12544, chunk=25088, groups=4):
        self.n_nodes = n_nodes
        self.nsh = nsh
        self.ntab = nsh * CORES
        self.chunk = chunk
        self.groups = groups
        assert chunk * groups == self.ntab
        assert nsh % 128 == 0
        self.nwin = nsh // 128


FULL = Geo()


def _wrap16(flat, T):
    a = flat.reshape(T // 16, 16).T
    return np.tile(a, (8, 1)).copy()


def pack(edge_index, edge_weight, geo: Geo):
    src = np.asarray(edge_index[0], dtype=np.int64)
    dst = np.asarray(edge_index[1], dtype=np.int64)
    w = np.asarray(edge_weight, dtype=np.float32)
    nsh, nwin, G = geo.nsh, geo.nwin, geo.groups

    core = dst // nsh
    pc = []
    cnt = np.zeros((CORES, G, nwin), np.int64)
    for c in range(CORES):
        m = core == c
        s_c = src[m]
        dl = dst[m] - c * nsh
        wc = w[m]
        g = s_c // geo.chunk
        key = g * (2 * nsh) + dl
        order = np.argsort(key, kind="stable")
        sk = (s_c - g * geo.chunk)[order]
        dlk = dl[order]
        wk = wc[order]
        kk = key[order]
        new = np.r_[True, kk[1:] != kk[:-1]]
        run_first = np.flatnonzero(new)
        run_len = np.diff(np.r_[run_first, len(kk)])
        run_id = np.cumsum(new) - 1
        rank = np.arange(len(kk)) - run_first[run_id]
        nsl = (run_len + KSLOT - 1) // KSLOT
        g_run = (g[order])[run_first]
        dl_run = dlk[run_first]
        v_run = dl_run // 128
        np.add.at(cnt[c], (g_run, v_run), nsl)
        pc.append((sk, dlk, wk, rank, run_id, nsl, g_run, dl_run, v_run))

    # shared template: exact max-over-cores region capacity; region order is
    # (cohort, group, window) so each cohort's windows complete mid-layer;
    # every (cohort, group) segment is padded to whole instruction blocks
    cap = cnt.max(axis=0)  # [G, nwin] slots
    assert (cap.sum(axis=0) > 0).all(), "window with no edges"
    cohorts = np.array_split(np.arange(nwin), NCOH)
    off = np.zeros((G, nwin), np.int64)
    seg_bounds = []  # (start, end, g)
    b = 0
    for ch in range(NCOH):
        for g in range(G):
            s0 = b
            for v in cohorts[ch]:
                off[g, v] = b
                b += cap[g, v]
            b = -(-b // SPI) * SPI
            seg_bounds.append((s0, b, g))
    S_T = int(b)
    COLS = S_T // 128
    T = S_T * KSLOT
    KC = T // 128
    NI = S_T // SPI
    instr_group = np.zeros(NI, np.int64)
    for s0, s1, g in seg_bounds:
        instr_group[s0 // SPI:s1 // SPI] = g
    instr_group = [int(x) for x in instr_group]

    # window-major colwin template: for each window, its (col) list across
    # groups; complete_at[v] = instruction block that finishes its last region
    win_cols = [[] for _ in range(nwin)]
    for g in range(G):
        for v in range(nwin):
            if cap[g, v] == 0:
                continue
            lo = int(off[g, v])
            hi = lo + int(cap[g, v])
            for col in range(lo // 128, (hi + 127) // 128):
                if col not in win_cols[v]:
                    win_cols[v].append(col)
    complete_at = np.zeros(nwin, np.int64)
    for g in range(G):
        for v in range(nwin):
            if cap[g, v] > 0:
                last = int(off[g, v] + cap[g, v] - 1) // SPI
                complete_at[v] = max(complete_at[v], last)
    sched = [[] for _ in range(NI)]
    for v in range(nwin):
        sched[int(complete_at[v])].append(v)
    # window-major dloc2 column order
    cw_base = np.zeros(nwin + 1, np.int64)
    for v in range(nwin):
        cw_base[v + 1] = cw_base[v] + len(win_cols[v])
    NCW = int(cw_base[nwin])
    OHW = max(len(wc_) for wc_ in win_cols)

    inmaps = []
    for c in range(CORES):
        sk, dlk, wk, rank, run_id, nsl, g_run, dl_run, v_run = pc[c]
        n_runs = len(nsl)
        csum = np.cumsum(nsl)
        start_excl = np.r_[0, csum[:-1]]
        gv = g_run * nwin + v_run
        newgv = np.r_[True, gv[1:] != gv[:-1]]
        gv_first = np.flatnonzero(newgv)
        gv_id = np.cumsum(newgv) - 1
        base_in_gv = start_excl - start_excl[gv_first][gv_id]
        run_slot = off[g_run, v_run] + base_in_gv
        slot_e = run_slot[run_id] + rank // KSLOT
        j_e = rank % KSLOT
        # token position: instruction block ib, local slot ls ->
        #   kc = ib*GK + j*SUBK + ls//128, p = ls%128
        ib = slot_e // SPI
        ls = slot_e - ib * SPI
        kc_e = ib * GK + j_e * SUBK + ls // 128
        p_e = ls % 128
        tok = kc_e * 128 + p_e

        idx_flat = np.zeros(T, np.int16)
        idx_flat[tok] = sk.astype(np.int16)
        w_flat = np.zeros(T, np.float32)
        w_flat[tok] = wk

        dl_slot = np.full(S_T, -1, np.int64)
        reps = np.repeat(np.arange(n_runs), nsl)
        ar = np.arange(len(reps)) - np.repeat(start_excl, nsl)
        pos = np.repeat(run_slot, nsl) + ar
        dl_slot[pos] = np.repeat(dl_run, nsl)
        d2 = np.full((128, NCW), 512.0, np.float32)
        k = 0
        for v in range(nwin):
            for col in win_cols[v]:
                dcol = dl_slot[col * 128:(col + 1) * 128] - 128 * v
                d2[:, k] = np.where((dcol >= 0) & (dcol < 128), dcol, 512)
                k += 1
        inmaps.append(
            dict(
                gidx=_wrap16(idx_flat, T),
                wgrid=w_flat.reshape(KC, 128).T.astype(np.float16),
                dloc2=d2,
            )
        )

    meta = dict(S_T=S_T, COLS=COLS, T=T, KC=KC, NCW=NCW, NI=NI, OHW=OHW,
                win_cols=win_cols, cw_base=cw_base, sched=sched,
                instr_group=instr_group, geo=geo)
    return meta, inmaps


def emit_dma_gather(gp, out_ap, in_ap, idxs_ap, num_idxs, elem_size, elem_step,
                    queue_num=0):
    """bass.dma_gather minus the blanket 256B elem assert (verified on HW that
    the non-transpose ucode handles 128B rows)."""
    from concourse.bass import exact_div

    assert idxs_ap.dtype == mybir.dt.int16
    assert in_ap.dtype == out_ap.dtype
    assert in_ap.space == bass.MemorySpace.DRAM
    stride_bytes_256 = exact_div(elem_step * mybir.dt.size(in_ap.dtype), 256)
    _in_ap = gp.lower_ap_dma(in_ap, for_custom_bir_dma=True)
    _idxs_ap = gp.lower_ap(idxs_ap)
    _out_ap = gp.lower_ap(out_ap)
    return gp.add_instruction(
        mybir.InstDMAGatherAnt(
            name=gp.bass.get_next_instruction_name(),
            ins=[*_in_ap, _idxs_ap, gp.lower_val_access(gp.to_reg(num_idxs))],
            outs=[_out_ap],
            transpose=False,
            num_idxs=num_idxs,
            elem_size=elem_size,
            stride_bytes_256=stride_bytes_256,
            gen_mode=0,
            single_packet=False,
            queue_num=queue_num,
            sbuf_tokens_per_rank=0,
            sbuf_free_dim_per_rank=0,
            sbuf_free_dim_pad_per_rank=0,
            sbuf_byte_offset=0,
        )
    )


def _b(ap2, reps):
    return bass.AP(tensor=ap2.tensor, offset=ap2.offset, ap=[*ap2.ap, [0, reps]])


def _bcast_col(ap1, n):
    return bass.AP(tensor=ap1.tensor, offset=ap1.offset, ap=[ap1.ap[0], [0, n]])


def build(meta):
    geo: Geo = meta["geo"]
    S_T, COLS, T, KC, NCW, NI, OHW = (meta["S_T"], meta["COLS"], meta["T"],
                                      meta["KC"], meta["NCW"], meta["NI"],
                                      meta["OHW"])
    win_cols, cw_base, sched = meta["win_cols"], meta["cw_base"], meta["sched"]
    instr_group = meta["instr_group"]
    nsh, ntab, nwin, G = geo.nsh, geo.ntab, geo.nwin, geo.groups
    f32 = mybir.dt.float32
    f16 = mybir.dt.float16
    bf16 = mybir.dt.bfloat16
    AX = mybir.AxisListType.X
    AF = mybir.ActivationFunctionType

    nc = bacc.Bacc("TRN2", target_bir_lowering=False, debug=False,
                   num_devices=CORES, num_swdge_queues=4)

    xT = nc.dram_tensor("xT", [F_IN, nsh], f32, kind="ExternalInput")
    gidx = nc.dram_tensor("gidx", [128, T // 16], mybir.dt.int16,
                          kind="ExternalInput")
    wgrid = nc.dram_tensor("wgrid", [128, KC], mybir.dt.float16, kind="ExternalInput")
    dloc2 = nc.dram_tensor("dloc2", [128, NCW], f32, kind="ExternalInput")
    iota128 = nc.dram_tensor("iota128", [128, 128], f32, kind="ExternalInput")
    ident = nc.dram_tensor("ident", [128, 128], f32, kind="ExternalInput")
    W1t = nc.dram_tensor("W1t", [F_IN, F_HID], f32, kind="ExternalInput")
    b1t = nc.dram_tensor("b1t", [128, F_HID], f32, kind="ExternalInput")
    W2t = nc.dram_tensor("W2t", [F_HID, F_OUT], f32, kind="ExternalInput")
    b2t = nc.dram_tensor("b2t", [1, F_OUT], f32, kind="ExternalInput")
    onest = nc.dram_tensor("onest", [1, 128], f32, kind="ExternalInput")
    out_t = nc.dram_tensor("out", [nsh, F_OUT], f32, kind="ExternalOutput")

    with tile.TileContext(nc) as tc:
        with (
            tc.tile_pool(name="const", bufs=1) as cpool,
            tc.tile_pool(name="dram", bufs=1, space="DRAM") as dram,
            tc.tile_pool(name="gpool", bufs=6) as gp_,
            tc.tile_pool(name="tpool", bufs=4) as tp_,
            tc.tile_pool(name="ohp", bufs=4) as ohp,
            tc.tile_pool(name="agg", bufs=1) as apool,
        ):
            iota_t = cpool.tile([128, 128], f32)
            nc.sync.dma_start(out=iota_t[:], in_=iota128[:, :])
            dloc_t = cpool.tile([128, NCW], f32)
            nc.sync.dma_start(out=dloc_t[:], in_=dloc2[:, :])
            W1s = cpool.tile([F_IN, F_HID], f32)
            nc.sync.dma_start(out=W1s[:], in_=W1t[:, :])
            b1s = cpool.tile([128, F_HID], f32)
            nc.sync.dma_start(out=b1s[:], in_=b1t[:, :])
            W2s = cpool.tile([F_HID, F_OUT], f32)
            nc.sync.dma_start(out=W2s[:], in_=W2t[:, :])
            b2s = cpool.tile([1, F_OUT], f32)
            nc.sync.dma_start(out=b2s[:], in_=b2t[:, :])
            ones_s = cpool.tile([1, 128], f32)
            nc.sync.dma_start(out=ones_s[:], in_=onest[:, :])
            id_s = cpool.tile([128, 128], f32)
            nc.sync.dma_start(out=id_s[:], in_=ident[:, :])

            shard1 = dram.tile([nsh, 128], f16)
            shard2 = dram.tile([nsh, 128], f16)
            table1 = dram.tile([ntab, 128], f16)
            table2 = dram.tile([ntab, 128], f16)

            # ---- h = x @ W1 on own shard -> shard1 ----
            with (
                tc.tile_pool(name="xt", bufs=1) as xp,
                tc.tile_pool(name="ph", bufs=2, space="PSUM") as ph,
            ):
                half = nsh // 2
                for hh in range(2):
                    xTs = xp.tile([F_IN, half], f32, tag="xts")
                    nc.sync.dma_start(out=xTs[:],
                                      in_=xT[:, hh * half:(hh + 1) * half])
                    for tt in range(half // 128):
                        hp = ph.tile([128, F_HID], f32, tag="hps")
                        nc.tensor.matmul(
                            out=hp[:], lhsT=xTs[:, tt * 128:(tt + 1) * 128],
                            rhs=W1s[:], start=True, stop=True,
                        )
                        hs = tp_.tile([128, 128], f16, tag="hsb")
                        nc.vector.memset(hs[:], 0.0)
                        nc.vector.tensor_copy(out=hs[:, :F_HID], in_=hp[:])
                        t = hh * (half // 128) + tt
                        dstp = bass.AP(
                            tensor=shard1.tensor, offset=t * 128 * 128,
                            ap=[[128, 128], [1, 128]],
                        )
                        nc.sync.dma_start(out=dstp, in_=hs[:])

            nc.gpsimd.collective_compute(
                "AllGather", mybir.AluOpType.bypass,
                ins=[shard1.opt()], outs=[table1.opt()],
                replica_groups=[list(range(CORES))],
            )

            CH = 8  # instruction blocks per input-prefetch chunk

            def run_layer(table, out1_all, close_fn, psw):
                gxc = wsc = None
                for ii in range(NI):
                    if ii % CH == 0:
                        nb_ = min(CH, NI - ii)
                        gxc = gp_.tile([128, CH * GK * 8], mybir.dt.int16,
                                       tag="gxc", name="gxc")
                        nc.sync.dma_start(
                            out=gxc[:, :nb_ * GK * 8],
                            in_=gidx[:, ii * GK * 8:(ii + nb_) * GK * 8])
                        wsc = gp_.tile([128, CH * GK], f16, tag="wsc",
                                       name="wsc")
                        nc.scalar.dma_start(
                            out=wsc[:, :nb_ * GK],
                            in_=wgrid[:, ii * GK:(ii + nb_) * GK])
                    g = instr_group[ii]
                    io = ii % CH
                    mg = gp_.tile([128, GK * 32], f16, tag="mg")
                    emit_dma_gather(
                        nc.gpsimd,
                        out_ap=mg[:].rearrange("p (k f) -> p k f", f=32),
                        in_ap=bass.AP(
                            tensor=table.tensor,
                            offset=g * geo.chunk * 128,
                            ap=[[128, geo.chunk], [1, 32]],
                        ),
                        idxs_ap=gxc[:, io * GK * 8:(io + 1) * GK * 8],
                        num_idxs=GK * 128,
                        elem_size=32,
                        elem_step=128,
                        queue_num=ii % 4,
                    )
                    nc.vector.tensor_tensor(
                        out=mg[:].rearrange("p (k f) -> p k f", f=32),
                        in0=mg[:].rearrange("p (k f) -> p k f", f=32),
                        in1=_b(wsc[:, io * GK:(io + 1) * GK], 32),
                        op=mybir.AluOpType.mult,
                    )
                    h1 = GK * 16
                    t1 = tp_.tile([128, GK * 16], f16, tag="t1")
                    nc.vector.tensor_tensor(out=t1[:], in0=mg[:, :h1],
                                            in1=mg[:, h1:2 * h1],
                                            op=mybir.AluOpType.add)
                    h2_ = GK * 8
                    nc.vector.tensor_tensor(
                        out=out1_all[:, ii * SPI // 128 * 32:
                                     (ii + 1) * SPI // 128 * 32],
                        in0=t1[:, :h2_], in1=t1[:, h2_:2 * h2_],
                        op=mybir.AluOpType.add,
                    )
                    for v in sched[ii]:
                        cols = win_cols[v]
                        nv = len(cols)
                        cw0 = int(cw_base[v])
                        oh = ohp.tile([128, OHW * 128], bf16, tag="oh")
                        nc.vector.tensor_tensor(
                            out=oh[:, :nv * 128].rearrange(
                                "p (b j) -> p b j", j=128),
                            in0=bass.AP(tensor=iota_t.tensor,
                                        offset=iota_t[:].offset,
                                        ap=[iota_t[:].ap[0], [0, nv], [1, 128]]),
                            in1=bass.AP(tensor=dloc_t.tensor,
                                        offset=dloc_t[:].offset + cw0,
                                        ap=[dloc_t[:].ap[0], [1, nv], [0, 128]]),
                            op=mybir.AluOpType.is_equal,
                        )
                        pwin = psw.tile([128, 512], f32, tag="pwin")
                        for i, col in enumerate(cols):
                            nc.tensor.matmul(
                                out=pwin[:, 0:F_HID],
                                lhsT=oh[:, i * 128:(i + 1) * 128],
                                rhs=out1_all[:, col * 32:(col + 1) * 32],
                                start=(i == 0),
                                stop=(i == nv - 1),
                            )
                        close_fn(v, pwin)

            # ---- layer 1 ----
            l1cm = tc.tile_pool(name="l1s", bufs=1)
            l1pool = l1cm.__enter__()
            out1a = l1pool.tile([128, COLS * 32], bf16, tag="o1a")
            h2 = l1pool.tile([128, nwin * 128], f16, tag="h2")
            nc.vector.memset(h2[:], 0.0)

            def close1(v, pwin):
                tmp = tp_.tile([128, F_HID], f32, tag="clo")
                nc.vector.tensor_tensor(
                    out=tmp[:], in0=pwin[:, 0:F_HID], in1=b1s[:],
                    op=mybir.AluOpType.add,
                )
                nc.scalar.activation(
                    out=h2[:, v * 128:v * 128 + F_HID], in_=tmp[:],
                    func=AF.Relu,
                )

            psw_cm = tc.tile_pool(name="psw", bufs=4, space="PSUM")
            psw = psw_cm.__enter__()
            run_layer(table1, out1a, close1, psw)
            psw_cm.__exit__(None, None, None)
            dst2 = bass.AP(tensor=shard2.tensor, offset=0,
                           ap=[[128, 128], [128 * 128, nwin], [1, 128]])
            nc.sync.dma_start(out=dst2, in_=h2[:].rearrange(
                "p (v f) -> p v f", f=128))
            l1cm.__exit__(None, None, None)
            nc.gpsimd.collective_compute(
                "AllGather", mybir.AluOpType.bypass,
                ins=[shard2.opt()], outs=[table2.opt()],
                replica_groups=[list(range(CORES))],
            )

            # ---- layer 2 (close folds in W2 + bias + exp/max pieces) ----
            l2cm = tc.tile_pool(name="l2s", bufs=1)
            l2pool = l2cm.__enter__()
            out1b = l2pool.tile([128, COLS * 32], bf16, tag="o1b")
            zall = apool.tile([128, nwin * F_OUT], f32, tag="zall")
            sall = apool.tile([128, nwin], f32, tag="sall")
            pf_cm = tc.tile_pool(name="pf", bufs=2, space="PSUM")
            pf = pf_cm.__enter__()

            def close2(v, pwin):
                ag = ohp.tile([128, F_HID], f32, tag="ag")
                nc.vector.tensor_copy(out=ag[:], in_=pwin[:, 0:F_HID])
                tp2 = pf.tile([F_HID, 128], f32, tag="tp")
                nc.tensor.transpose(out=tp2[:], in_=ag[:], identity=id_s[:])
                aT = ohp.tile([F_HID, 128], f32, tag="aT")
                nc.vector.tensor_copy(out=aT[:], in_=tp2[:])
                zp = pf.tile([128, F_OUT], f32, tag="zp")
                nc.tensor.matmul(out=zp[:], lhsT=aT[:], rhs=W2s[:],
                                 start=True, stop=False)
                nc.tensor.matmul(out=zp[:], lhsT=ones_s[:], rhs=b2s[:],
                                 start=False, stop=True)
                negm = ohp.tile([128, 1], f32, tag="negm")
                nc.vector.reduce_max(out=negm[:], in_=zp[:], axis=AX,
                                     negate=True)
                nc.vector.tensor_tensor(
                    out=zall[:, v * F_OUT:(v + 1) * F_OUT],
                    in0=zp[:], in1=_bcast_col(negm[:], F_OUT),
                    op=mybir.AluOpType.add,
                )
                etmp = ohp.tile([128, F_OUT], f32, tag="etmp")
                nc.scalar.activation(
                    out=etmp[:], in_=zall[:, v * F_OUT:(v + 1) * F_OUT],
                    func=AF.Exp, accum_out=sall[:, v:v + 1],
                )

            psw_cm2 = tc.tile_pool(name="psw2", bufs=4, space="PSUM")
            psw2 = psw_cm2.__enter__()
            run_layer(table2, out1b, close2, psw2)
            psw_cm2.__exit__(None, None, None)
            l2cm.__exit__(None, None, None)

            lns = apool.tile([128, nwin], f32, tag="lns")
            nc.scalar.activation(out=lns[:], in_=sall[:], func=AF.Ln)
            for v in range(nwin):
                nc.vector.tensor_tensor(
                    out=zall[:, v * F_OUT:(v + 1) * F_OUT],
                    in0=zall[:, v * F_OUT:(v + 1) * F_OUT],
                    in1=_bcast_col(lns[:, v:v + 1], F_OUT),
                    op=mybir.AluOpType.subtract,
                )
            outdst = bass.AP(
                tensor=out_t, offset=0,
                ap=[[F_OUT, 128], [128 * F_OUT, nwin], [1, F_OUT]],
            )
            nc.sync.dma_start(out=outdst, in_=zall[:].rearrange(
                "p (v f) -> p v f", f=F_OUT))
            pf_cm.__exit__(None, None, None)

    nc.compile()
    return nc


def make_inmaps(meta, inmaps_edges, x, W1, b1, W2, b2):
    geo: Geo = meta["geo"]
    nsh = geo.nsh
    n = geo.n_nodes
    xT_full = np.zeros((F_IN, geo.ntab), np.float32)
    xT_full[:, :n] = np.asarray(x, np.float32).T
    iota = np.tile(np.arange(128, dtype=np.float32)[None, :], (128, 1))
    ident = np.eye(128, dtype=np.float32)
    b1b = np.tile(np.asarray(b1, np.float32)[None, :], (128, 1))
    consts = dict(
        iota128=iota, ident=ident,
        W1t=np.asarray(W1, np.float32), b1t=b1b,
        W2t=np.asarray(W2, np.float32), b2t=np.asarray(b2, np.float32)[None, :],
        onest=np.ones((1, 128), np.float32),
    )
    maps = []
    for c in range(CORES):
        m = dict(inmaps_edges[c])
        m.update(consts)
        m["xT"] = np.ascontiguousarray(xT_full[:, c * nsh:(c + 1) * nsh])
        maps.append(m)
    return maps


_CACHE = {}


def run(x, edge_index, edge_weight, W1, b1, W2, b2, geo=FULL, trace=False):
    key = "geo%d" % geo.n_nodes
    meta, inmaps_edges = pack(edge_index, edge_weight, geo)
    if key in _CACHE:
        nc = _CACHE[key]
    else:
        nc = build(meta)
        _CACHE[key] = nc
    maps = make_inmaps(meta, inmaps_edges, x, W1, b1, W2, b2)
    res = run_bass_kernel_spmd(nc, maps, core_ids=list(range(CORES)), trace=trace)
    n = geo.n_nodes
    out = np.empty((n, F_OUT), np.float32)
    for c in range(CORES):
        lo = c * geo.nsh
        hi = min(lo + geo.nsh, n)
        if hi > lo:
            out[lo:hi] = res.results[c]["out"][: hi - lo]
    return out, res


def kernel(x, edge_index, edge_weight, W1, b1, W2, b2):
    out, _ = run(
        np.asarray(x), np.asarray(edge_index), np.asarray(edge_weight),
        np.asarray(W1), np.asarray(b1), np.asarray(W2), np.asarray(b2),
    )
    return out


# revision 17
# speedup vs baseline: 4.1050x; 1.0759x over previous
"""GCN (2-layer, edge-weighted, log_softmax) on 8 Trainium2 NeuronCores.

v2 strategy (dst-sharded edges, matmul segment-sum, 4-queue SWDGE gather):
  - Nodes sharded 12544/core; fp32 table rows (256B-strided, 32 used) in HBM,
    replicated via AllGather.
  - Edges packed 4-per-slot by (src-chunk, dst). Each 2048-slot instruction
    block issues FOUR 2048-token gathers (one per slot position j) on SWDGE
    queues 0..3 -> concurrent DMA transfers into four separate tiles, so the
    weighted 4->1 reduction is 4 mults + 3 adds, all contiguous two-tile ops.
  - Slot sums (bf16) for the whole layer stay resident in SBUF (out1_all).
  - Scatter runs window-major: once the last instruction touching window v
    completes, a one-shot is_equal builds all of v's one-hot columns (host
    dloc2 template, window-major order), and ~10 bf16 matmuls accumulate
    into a dedicated full PSUM bank (start/stop per window; 4 banks rotate).
    Window closes immediately (bias+relu -> table2 shard, or copy to agg2).
  - Layer 2 reuses the identical edge template on table2 = relu(agg1+b1),
    then W2 + log_softmax per 128-node window.
Host side packs indices/weights (numpy) and concatenates shards.
"""

import os
import sys

for _p in ("/opt/trn_rl_repo", "/root/.axon_site/_ro/trn_rl_repo"):
    if os.path.isdir(_p) and _p not in sys.path:
        sys.path.insert(0, _p)

import numpy as np

import concourse.ap_utils as ap_utils
import concourse.bass as bass
import concourse.mybir as mybir
from concourse import bacc, tile
from concourse.bass_utils import run_bass_kernel_spmd

CORES = 8
F_IN = 128
F_HID = 32
F_OUT = 40
KSLOT = 4      # edges per slot (same destination)
GK = 32        # k-columns per instruction block (4096 tokens, 1024 slots)
NCOH = 4       # window cohorts (striped region order for scatter overlap)
SPI = GK * 128 // KSLOT   # slots per instruction block (2048)
SUBK = GK // KSLOT        # k-columns per sub-gather (16 -> 2048 tokens)


class Geo:
    def __init__(self, n_nodes=100000, nsh=12544, chunk=25088, groups=4):
        self.n_nodes = n_nodes
        self.nsh = nsh
        self.ntab = nsh * CORES
        self.chunk = chunk
        self.groups = groups
        assert chunk * groups == self.ntab
        assert nsh % 128 == 0
        self.nwin = nsh // 128


FULL = Geo()


def _wrap16(flat, T):
    a = flat.reshape(T // 16, 16).T
    return np.tile(a, (8, 1)).copy()


def pack(edge_index, edge_weight, geo: Geo):
    src = np.asarray(edge_index[0], dtype=np.int64)
    dst = np.asarray(edge_index[1], dtype=np.int64)
    w = np.asarray(edge_weight, dtype=np.float32)
    nsh, nwin, G = geo.nsh, geo.nwin, geo.groups

    core = dst // nsh
    pc = []
    cnt = np.zeros((CORES, G, nwin), np.int64)
    for c in range(CORES):
        m = core == c
        s_c = src[m]
        dl = dst[m] - c * nsh
        wc = w[m]
        g = s_c // geo.chunk
        key = g * (2 * nsh) + dl
        order = np.argsort(key, kind="stable")
        sk = (s_c - g * geo.chunk)[order]
        dlk = dl[order]
        wk = wc[order]
        kk = key[order]
        new = np.r_[True, kk[1:] != kk[:-1]]
        run_first = np.flatnonzero(new)
        run_len = np.diff(np.r_[run_first, len(kk)])
        run_id = np.cumsum(new) - 1
        rank = np.arange(len(kk)) - run_first[run_id]
        nsl = (run_len + KSLOT - 1) // KSLOT
        g_run = (g[order])[run_first]
        dl_run = dlk[run_first]
        v_run = dl_run // 128
        np.add.at(cnt[c], (g_run, v_run), nsl)
        pc.append((sk, dlk, wk, rank, run_id, nsl, g_run, dl_run, v_run))

    # shared template: exact max-over-cores region capacity; region order is
    # (cohort, group, window) so each cohort's windows complete mid-layer;
    # every (cohort, group) segment is padded to whole instruction blocks
    cap = cnt.max(axis=0)  # [G, nwin] slots
    assert (cap.sum(axis=0) > 0).all(), "window with no edges"
    cohorts = np.array_split(np.arange(nwin), NCOH)
    off = np.zeros((G, nwin), np.int64)
    seg_bounds = []  # (start, end, g)
    b = 0
    for ch in range(NCOH):
        for g in range(G):
            s0 = b
            for v in cohorts[ch]:
                off[g, v] = b
                b += cap[g, v]
            b = -(-b // SPI) * SPI
            seg_bounds.append((s0, b, g))
    S_T = int(b)
    COLS = S_T // 128
    T = S_T * KSLOT
    KC = T // 128
    NI = S_T // SPI
    instr_group = np.zeros(NI, np.int64)
    for s0, s1, g in seg_bounds:
        instr_group[s0 // SPI:s1 // SPI] = g
    instr_group = [int(x) for x in instr_group]

    # window-major colwin template: for each window, its (col) list across
    # groups; complete_at[v] = instruction block that finishes its last region
    win_cols = [[] for _ in range(nwin)]
    for g in range(G):
        for v in range(nwin):
            if cap[g, v] == 0:
                continue
            lo = int(off[g, v])
            hi = lo + int(cap[g, v])
            for col in range(lo // 128, (hi + 127) // 128):
                if col not in win_cols[v]:
                    win_cols[v].append(col)
    complete_at = np.zeros(nwin, np.int64)
    for g in range(G):
        for v in range(nwin):
            if cap[g, v] > 0:
                last = int(off[g, v] + cap[g, v] - 1) // SPI
                complete_at[v] = max(complete_at[v], last)
    sched = [[] for _ in range(NI)]
    for v in range(nwin):
        sched[int(complete_at[v])].append(v)
    # window-major dloc2 column order
    cw_base = np.zeros(nwin + 1, np.int64)
    for v in range(nwin):
        cw_base[v + 1] = cw_base[v] + len(win_cols[v])
    NCW = int(cw_base[nwin])
    OHW = max(len(wc_) for wc_ in win_cols)

    inmaps = []
    for c in range(CORES):
        sk, dlk, wk, rank, run_id, nsl, g_run, dl_run, v_run = pc[c]
        n_runs = len(nsl)
        csum = np.cumsum(nsl)
        start_excl = np.r_[0, csum[:-1]]
        gv = g_run * nwin + v_run
        newgv = np.r_[True, gv[1:] != gv[:-1]]
        gv_first = np.flatnonzero(newgv)
        gv_id = np.cumsum(newgv) - 1
        base_in_gv = start_excl - start_excl[gv_first][gv_id]
        run_slot = off[g_run, v_run] + base_in_gv
        slot_e = run_slot[run_id] + rank // KSLOT
        j_e = rank % KSLOT
        # token position: instruction block ib, local slot ls ->
        #   kc = ib*GK + j*SUBK + ls//128, p = ls%128
        ib = slot_e // SPI
        ls = slot_e - ib * SPI
        kc_e = ib * GK + j_e * SUBK + ls // 128
        p_e = ls % 128
        tok = kc_e * 128 + p_e

        idx_flat = np.zeros(T, np.int16)
        idx_flat[tok] = sk.astype(np.int16)
        w_flat = np.zeros(T, np.float32)
        w_flat[tok] = wk

        dl_slot = np.full(S_T, -1, np.int64)
        reps = np.repeat(np.arange(n_runs), nsl)
        ar = np.arange(len(reps)) - np.repeat(start_excl, nsl)
        pos = np.repeat(run_slot, nsl) + ar
        dl_slot[pos] = np.repeat(dl_run, nsl)
        d2 = np.full((128, NCW), 512.0, np.float32)
        k = 0
        for v in range(nwin):
            for col in win_cols[v]:
                dcol = dl_slot[col * 128:(col + 1) * 128] - 128 * v
                d2[:, k] = np.where((dcol >= 0) & (dcol < 128), dcol, 512)
                k += 1
        inmaps.append(
            dict(
                gidx=_wrap16(idx_flat, T),
                wgrid=w_flat.reshape(KC, 128).T.astype(np.float16),
                dloc2=d2,
            )
        )

    meta = dict(S_T=S_T, COLS=COLS, T=T, KC=KC, NCW=NCW, NI=NI, OHW=OHW,
                win_cols=win_cols, cw_base=cw_base, sched=sched,
                instr_group=instr_group, geo=geo)
    return meta, inmaps


def emit_dma_gather(gp, out_ap, in_ap, idxs_ap, num_idxs, elem_size, elem_step,
                    queue_num=0):
    """bass.dma_gather minus the blanket 256B elem assert (verified on HW that
    the non-transpose ucode handles 128B rows)."""
    from concourse.bass import exact_div

    assert idxs_ap.dtype == mybir.dt.int16
    assert in_ap.dtype == out_ap.dtype
    assert in_ap.space == bass.MemorySpace.DRAM
    stride_bytes_256 = exact_div(elem_step * mybir.dt.size(in_ap.dtype), 256)
    _in_ap = gp.lower_ap_dma(in_ap, for_custom_bir_dma=True)
    _idxs_ap = gp.lower_ap(idxs_ap)
    _out_ap = gp.lower_ap(out_ap)
    return gp.add_instruction(
        mybir.InstDMAGatherAnt(
            name=gp.bass.get_next_instruction_name(),
            ins=[*_in_ap, _idxs_ap, gp.lower_val_access(gp.to_reg(num_idxs))],
            outs=[_out_ap],
            transpose=False,
            num_idxs=num_idxs,
            elem_size=elem_size,
            stride_bytes_256=stride_bytes_256,
            gen_mode=0,
            single_packet=False,
            queue_num=queue_num,
            sbuf_tokens_per_rank=0,
            sbuf_free_dim_per_rank=0,
            sbuf_free_dim_pad_per_rank=0,
            sbuf_byte_offset=0,
        )
    )


def _b(ap2, reps):
    return bass.AP(tensor=ap2.tensor, offset=ap2.offset, ap=[*ap2.ap, [0, reps]])


def _bcast_col(ap1, n):
    return bass.AP(tensor=ap1.tensor, offset=ap1.offset, ap=[ap1.ap[0], [0, n]])


def build(meta):
    geo: Geo = meta["geo"]
    S_T, COLS, T, KC, NCW, NI, OHW = (meta["S_T"], meta["COLS"], meta["T"],
                                      meta["KC"], meta["NCW"], meta["NI"],
                                      meta["OHW"])
    win_cols, cw_base, sched = meta["win_cols"], meta["cw_base"], meta["sched"]
    instr_group = meta["instr_group"]
    nsh, ntab, nwin, G = geo.nsh, geo.ntab, geo.nwin, geo.groups
    f32 = mybir.dt.float32
    f16 = mybir.dt.float16
    bf16 = mybir.dt.bfloat16
    AX = mybir.AxisListType.X
    AF = mybir.ActivationFunctionType

    nc = bacc.Bacc("TRN2", target_bir_lowering=False, debug=False,
                   num_devices=CORES, num_swdge_queues=4)

    xT = nc.dram_tensor("xT", [F_IN, nsh], f32, kind="ExternalInput")
    gidx = nc.dram_tensor("gidx", [128, T // 16], mybir.dt.int16,
                          kind="ExternalInput")
    wgrid = nc.dram_tensor("wgrid", [128, KC], mybir.dt.float16, kind="ExternalInput")
    dloc2 = nc.dram_tensor("dloc2", [128, NCW], f32, kind="ExternalInput")
    iota128 = nc.dram_tensor("iota128", [128, 128], f32, kind="ExternalInput")
    ident = nc.dram_tensor("ident", [128, 128], f32, kind="ExternalInput")
    W1t = nc.dram_tensor("W1t", [F_IN, F_HID], f32, kind="ExternalInput")
    b1t = nc.dram_tensor("b1t", [128, F_HID], f32, kind="ExternalInput")
    W2t = nc.dram_tensor("W2t", [F_HID, F_OUT], f32, kind="ExternalInput")
    b2t = nc.dram_tensor("b2t", [1, F_OUT], f32, kind="ExternalInput")
    onest = nc.dram_tensor("onest", [1, 128], f32, kind="ExternalInput")
    out_t = nc.dram_tensor("out", [nsh, F_OUT], f32, kind="ExternalOutput")

    with tile.TileContext(nc) as tc:
        with (
            tc.tile_pool(name="const", bufs=1) as cpool,
            tc.tile_pool(name="dram", bufs=1, space="DRAM") as dram,
            tc.tile_pool(name="gpool", bufs=3) as gp_,
            tc.tile_pool(name="mgpool", bufs=12) as mgp,
            tc.tile_pool(name="tpool", bufs=4) as tp_,
            tc.tile_pool(name="ohp", bufs=4) as ohp,
            tc.tile_pool(name="agg", bufs=1) as apool,
        ):
            iota_t = cpool.tile([128, 128], f32)
            nc.sync.dma_start(out=iota_t[:], in_=iota128[:, :])
            dloc_t = cpool.tile([128, NCW], f32)
            nc.sync.dma_start(out=dloc_t[:], in_=dloc2[:, :])
            W1s = cpool.tile([F_IN, F_HID], f32)
            nc.sync.dma_start(out=W1s[:], in_=W1t[:, :])
            b1s = cpool.tile([128, F_HID], f32)
            nc.sync.dma_start(out=b1s[:], in_=b1t[:, :])
            W2s = cpool.tile([F_HID, F_OUT], f32)
            nc.sync.dma_start(out=W2s[:], in_=W2t[:, :])
            b2s = cpool.tile([1, F_OUT], f32)
            nc.sync.dma_start(out=b2s[:], in_=b2t[:, :])
            ones_s = cpool.tile([1, 128], f32)
            nc.sync.dma_start(out=ones_s[:], in_=onest[:, :])
            id_s = cpool.tile([128, 128], f32)
            nc.sync.dma_start(out=id_s[:], in_=ident[:, :])

            shard1 = dram.tile([nsh, 128], f16)
            shard2 = dram.tile([nsh, 128], f16)
            table1 = dram.tile([ntab, 128], f16)
            table2 = dram.tile([ntab, 128], f16)

            # ---- h = x @ W1 on own shard -> shard1 ----
            with (
                tc.tile_pool(name="xt", bufs=1) as xp,
                tc.tile_pool(name="ph", bufs=2, space="PSUM") as ph,
            ):
                half = nsh // 2
                for hh in range(2):
                    xTs = xp.tile([F_IN, half], f32, tag="xts")
                    nc.sync.dma_start(out=xTs[:],
                                      in_=xT[:, hh * half:(hh + 1) * half])
                    for tt in range(half // 128):
                        hp = ph.tile([128, F_HID], f32, tag="hps")
                        nc.tensor.matmul(
                            out=hp[:], lhsT=xTs[:, tt * 128:(tt + 1) * 128],
                            rhs=W1s[:], start=True, stop=True,
                        )
                        hs = tp_.tile([128, 128], f16, tag="hsb")
                        nc.vector.memset(hs[:], 0.0)
                        nc.vector.tensor_copy(out=hs[:, :F_HID], in_=hp[:])
                        t = hh * (half // 128) + tt
                        dstp = bass.AP(
                            tensor=shard1.tensor, offset=t * 128 * 128,
                            ap=[[128, 128], [1, 128]],
                        )
                        nc.sync.dma_start(out=dstp, in_=hs[:])

            nc.gpsimd.collective_compute(
                "AllGather", mybir.AluOpType.bypass,
                ins=[shard1.opt()], outs=[table1.opt()],
                replica_groups=[list(range(CORES))],
            )

            CH = 8  # instruction blocks per input-prefetch chunk

            def run_layer(table, out1_all, close_fn, psw):
                gxc = wsc = None
                for ii in range(NI):
                    if ii % CH == 0:
                        nb_ = min(CH, NI - ii)
                        gxc = gp_.tile([128, CH * GK * 8], mybir.dt.int16,
                                       tag="gxc", name="gxc")
                        nc.sync.dma_start(
                            out=gxc[:, :nb_ * GK * 8],
                            in_=gidx[:, ii * GK * 8:(ii + nb_) * GK * 8])
                        wsc = gp_.tile([128, CH * GK], f16, tag="wsc",
                                       name="wsc")
                        nc.scalar.dma_start(
                            out=wsc[:, :nb_ * GK],
                            in_=wgrid[:, ii * GK:(ii + nb_) * GK])
                    g = instr_group[ii]
                    io = ii % CH
                    mg = mgp.tile([128, GK * 32], f16, tag="mg")
                    emit_dma_gather(
                        nc.gpsimd,
                        out_ap=mg[:].rearrange("p (k f) -> p k f", f=32),
                        in_ap=bass.AP(
                            tensor=table.tensor,
                            offset=g * geo.chunk * 128,
                            ap=[[128, geo.chunk], [1, 32]],
                        ),
                        idxs_ap=gxc[:, io * GK * 8:(io + 1) * GK * 8],
                        num_idxs=GK * 128,
                        elem_size=32,
                        elem_step=128,
                        queue_num=ii % 4,
                    )
                    nc.vector.tensor_tensor(
                        out=mg[:].rearrange("p (k f) -> p k f", f=32),
                        in0=mg[:].rearrange("p (k f) -> p k f", f=32),
                        in1=_b(wsc[:, io * GK:(io + 1) * GK], 32),
                        op=mybir.AluOpType.mult,
                    )
                    h1 = GK * 16
                    t1 = tp_.tile([128, GK * 16], f16, tag="t1")
                    nc.vector.tensor_tensor(out=t1[:], in0=mg[:, :h1],
                                            in1=mg[:, h1:2 * h1],
                                            op=mybir.AluOpType.add)
                    h2_ = GK * 8
                    nc.vector.tensor_tensor(
                        out=out1_all[:, ii * SPI // 128 * 32:
                                     (ii + 1) * SPI // 128 * 32],
                        in0=t1[:, :h2_], in1=t1[:, h2_:2 * h2_],
                        op=mybir.AluOpType.add,
                    )
                    for v in sched[ii]:
                        cols = win_cols[v]
                        nv = len(cols)
                        cw0 = int(cw_base[v])
                        oh = ohp.tile([128, OHW * 128], bf16, tag="oh")
                        nc.vector.tensor_tensor(
                            out=oh[:, :nv * 128].rearrange(
                                "p (b j) -> p b j", j=128),
                            in0=bass.AP(tensor=iota_t.tensor,
                                        offset=iota_t[:].offset,
                                        ap=[iota_t[:].ap[0], [0, nv], [1, 128]]),
                            in1=bass.AP(tensor=dloc_t.tensor,
                                        offset=dloc_t[:].offset + cw0,
                                        ap=[dloc_t[:].ap[0], [1, nv], [0, 128]]),
                            op=mybir.AluOpType.is_equal,
                        )
                        pwin = psw.tile([128, 512], f32, tag="pwin")
                        for i, col in enumerate(cols):
                            nc.tensor.matmul(
                                out=pwin[:, 0:F_HID],
                                lhsT=oh[:, i * 128:(i + 1) * 128],
                                rhs=out1_all[:, col * 32:(col + 1) * 32],
                                start=(i == 0),
                                stop=(i == nv - 1),
                            )
                        close_fn(v, pwin)

            # ---- layer 1 ----
            l1cm = tc.tile_pool(name="l1s", bufs=1)
            l1pool = l1cm.__enter__()
            out1a = l1pool.tile([128, COLS * 32], bf16, tag="o1a")
            h2 = l1pool.tile([128, nwin * 128], f16, tag="h2")
            nc.vector.memset(h2[:], 0.0)

            def close1(v, pwin):
                tmp = tp_.tile([128, F_HID], f32, tag="clo")
                nc.vector.tensor_tensor(
                    out=tmp[:], in0=pwin[:, 0:F_HID], in1=b1s[:],
                    op=mybir.AluOpType.add,
                )
                nc.scalar.activation(
                    out=h2[:, v * 128:v * 128 + F_HID], in_=tmp[:],
                    func=AF.Relu,
                )

            psw_cm = tc.tile_pool(name="psw", bufs=4, space="PSUM")
            psw = psw_cm.__enter__()
            run_layer(table1, out1a, close1, psw)
            psw_cm.__exit__(None, None, None)
            dst2 = bass.AP(tensor=shard2.tensor, offset=0,
                           ap=[[128, 128], [128 * 128, nwin], [1, 128]])
            nc.sync.dma_start(out=dst2, in_=h2[:].rearrange(
                "p (v f) -> p v f", f=128))
            l1cm.__exit__(None, None, None)
            nc.gpsimd.collective_compute(
                "AllGather", mybir.AluOpType.bypass,
                ins=[shard2.opt()], outs=[table2.opt()],
                replica_groups=[list(range(CORES))],
            )

            # ---- layer 2 (close folds in W2 + bias + exp/max pieces) ----
            l2cm = tc.tile_pool(name="l2s", bufs=1)
            l2pool = l2cm.__enter__()
            out1b = l2pool.tile([128, COLS * 32], bf16, tag="o1b")
            zall = apool.tile([128, nwin * F_OUT], f32, tag="zall")
            sall = apool.tile([128, nwin], f32, tag="sall")
            pf_cm = tc.tile_pool(name="pf", bufs=2, space="PSUM")
            pf = pf_cm.__enter__()

            def close2(v, pwin):
                ag = ohp.tile([128, F_HID], f32, tag="ag")
                nc.vector.tensor_copy(out=ag[:], in_=pwin[:, 0:F_HID])
                tp2 = pf.tile([F_HID, 128], f32, tag="tp")
                nc.tensor.transpose(out=tp2[:], in_=ag[:], identity=id_s[:])
                aT = ohp.tile([F_HID, 128], f32, tag="aT")
                nc.vector.tensor_copy(out=aT[:], in_=tp2[:])
                zp = pf.tile([128, F_OUT], f32, tag="zp")
                nc.tensor.matmul(out=zp[:], lhsT=aT[:], rhs=W2s[:],
                                 start=True, stop=False)
                nc.tensor.matmul(out=zp[:], lhsT=ones_s[:], rhs=b2s[:],
                                 start=False, stop=True)
                negm = ohp.tile([128, 1], f32, tag="negm")
                nc.vector.reduce_max(out=negm[:], in_=zp[:], axis=AX,
                                     negate=True)
                nc.vector.tensor_tensor(
                    out=zall[:, v * F_OUT:(v + 1) * F_OUT],
                    in0=zp[:], in1=_bcast_col(negm[:], F_OUT),
                    op=mybir.AluOpType.add,
                )
                etmp = ohp.tile([128, F_OUT], f32, tag="etmp")
                nc.scalar.activation(
                    out=etmp[:], in_=zall[:, v * F_OUT:(v + 1) * F_OUT],
                    func=AF.Exp, accum_out=sall[:, v:v + 1],
                )

            psw_cm2 = tc.tile_pool(name="psw2", bufs=4, space="PSUM")
            psw2 = psw_cm2.__enter__()
            run_layer(table2, out1b, close2, psw2)
            psw_cm2.__exit__(None, None, None)
            l2cm.__exit__(None, None, None)

            lns = apool.tile([128, nwin], f32, tag="lns")
            nc.scalar.activation(out=lns[:], in_=sall[:], func=AF.Ln)
            for v in range(nwin):
                nc.vector.tensor_tensor(
                    out=zall[:, v * F_OUT:(v + 1) * F_OUT],
                    in0=zall[:, v * F_OUT:(v + 1) * F_OUT],
                    in1=_bcast_col(lns[:, v:v + 1], F_OUT),
                    op=mybir.AluOpType.subtract,
                )
            outdst = bass.AP(
                tensor=out_t, offset=0,
                ap=[[F_OUT, 128], [128 * F_OUT, nwin], [1, F_OUT]],
            )
            nc.sync.dma_start(out=outdst, in_=zall[:].rearrange(
                "p (v f) -> p v f", f=F_OUT))
            pf_cm.__exit__(None, None, None)

    nc.compile()
    return nc


def make_inmaps(meta, inmaps_edges, x, W1, b1, W2, b2):
    geo: Geo = meta["geo"]
    nsh = geo.nsh
    n = geo.n_nodes
    xT_full = np.zeros((F_IN, geo.ntab), np.float32)
    xT_full[:, :n] = np.asarray(x, np.float32).T
    iota = np.tile(np.arange(128, dtype=np.float32)[None, :], (128, 1))
    ident = np.eye(128, dtype=np.float32)
    b1b = np.tile(np.asarray(b1, np.float32)[None, :], (128, 1))
    consts = dict(
        iota128=iota, ident=ident,
        W1t=np.asarray(W1, np.float32), b1t=b1b,
        W2t=np.asarray(W2, np.float32), b2t=np.asarray(b2, np.float32)[None, :],
        onest=np.ones((1, 128), np.float32),
    )
    maps = []
    for c in range(CORES):
        m = dict(inmaps_edges[c])
        m.update(consts)
        m["xT"] = np.ascontiguousarray(xT_full[:, c * nsh:(c + 1) * nsh])
        maps.append(m)
    return maps


_CACHE = {}


def run(x, edge_index, edge_weight, W1, b1, W2, b2, geo=FULL, trace=False):
    key = "geo%d" % geo.n_nodes
    meta, inmaps_edges = pack(edge_index, edge_weight, geo)
    if key in _CACHE:
        nc = _CACHE[key]
    else:
        nc = build(meta)
        _CACHE[key] = nc
    maps = make_inmaps(meta, inmaps_edges, x, W1, b1, W2, b2)
    res = run_bass_kernel_spmd(nc, maps, core_ids=list(range(CORES)), trace=trace)
    n = geo.n_nodes
    out = np.empty((n, F_OUT), np.float32)
    for c in range(CORES):
        lo = c * geo.nsh
        hi = min(lo + geo.nsh, n)
        if hi > lo:
            out[lo:hi] = res.results[c]["out"][: hi - lo]
    return out, res


def kernel(x, edge_index, edge_weight, W1, b1, W2, b2):
    out, _ = run(
        np.asarray(x), np.asarray(edge_index), np.asarray(edge_weight),
        np.asarray(W1), np.asarray(b1), np.asarray(W2), np.asarray(b2),
    )
    return out


# revision 18
# speedup vs baseline: 4.1386x; 1.0082x over previous
"""GCN (2-layer, edge-weighted, log_softmax) on 8 Trainium2 NeuronCores.

v2 strategy (dst-sharded edges, matmul segment-sum, 4-queue SWDGE gather):
  - Nodes sharded 12544/core; fp32 table rows (256B-strided, 32 used) in HBM,
    replicated via AllGather.
  - Edges packed 4-per-slot by (src-chunk, dst). Each 2048-slot instruction
    block issues FOUR 2048-token gathers (one per slot position j) on SWDGE
    queues 0..3 -> concurrent DMA transfers into four separate tiles, so the
    weighted 4->1 reduction is 4 mults + 3 adds, all contiguous two-tile ops.
  - Slot sums (bf16) for the whole layer stay resident in SBUF (out1_all).
  - Scatter runs window-major: once the last instruction touching window v
    completes, a one-shot is_equal builds all of v's one-hot columns (host
    dloc2 template, window-major order), and ~10 bf16 matmuls accumulate
    into a dedicated full PSUM bank (start/stop per window; 4 banks rotate).
    Window closes immediately (bias+relu -> table2 shard, or copy to agg2).
  - Layer 2 reuses the identical edge template on table2 = relu(agg1+b1),
    then W2 + log_softmax per 128-node window.
Host side packs indices/weights (numpy) and concatenates shards.
"""

import os
import sys

for _p in ("/opt/trn_rl_repo", "/root/.axon_site/_ro/trn_rl_repo"):
    if os.path.isdir(_p) and _p not in sys.path:
        sys.path.insert(0, _p)

import numpy as np

import concourse.ap_utils as ap_utils
import concourse.bass as bass
import concourse.mybir as mybir
from concourse import bacc, tile
from concourse.bass_utils import run_bass_kernel_spmd

CORES = 8
F_IN = 128
F_HID = 32
F_OUT = 40
KSLOT = 4      # edges per slot (same destination)
GK = 32        # k-columns per instruction block (4096 tokens, 1024 slots)
NCOH = 4       # window cohorts (striped region order for scatter overlap)
SPI = GK * 128 // KSLOT   # slots per instruction block (2048)
SUBK = GK // KSLOT        # k-columns per sub-gather (16 -> 2048 tokens)


class Geo:
    def __init__(self, n_nodes=100000, nsh=12544, chunk=25088, groups=4):
        self.n_nodes = n_nodes
        self.nsh = nsh
        self.ntab = nsh * CORES
        self.chunk = chunk
        self.groups = groups
        assert chunk * groups == self.ntab
        assert nsh % 128 == 0
        self.nwin = nsh // 128


FULL = Geo()


def _wrap16(flat, T):
    a = flat.reshape(T // 16, 16).T
    return np.tile(a, (8, 1)).copy()


def pack(edge_index, edge_weight, geo: Geo):
    src = np.asarray(edge_index[0], dtype=np.int64)
    dst = np.asarray(edge_index[1], dtype=np.int64)
    w = np.asarray(edge_weight, dtype=np.float32)
    nsh, nwin, G = geo.nsh, geo.nwin, geo.groups

    core = dst // nsh
    pc = []
    cnt = np.zeros((CORES, G, nwin), np.int64)
    for c in range(CORES):
        m = core == c
        s_c = src[m]
        dl = dst[m] - c * nsh
        wc = w[m]
        g = s_c // geo.chunk
        key = g * (2 * nsh) + dl
        order = np.argsort(key, kind="stable")
        sk = (s_c - g * geo.chunk)[order]
        dlk = dl[order]
        wk = wc[order]
        kk = key[order]
        new = np.r_[True, kk[1:] != kk[:-1]]
        run_first = np.flatnonzero(new)
        run_len = np.diff(np.r_[run_first, len(kk)])
        run_id = np.cumsum(new) - 1
        rank = np.arange(len(kk)) - run_first[run_id]
        nsl = (run_len + KSLOT - 1) // KSLOT
        g_run = (g[order])[run_first]
        dl_run = dlk[run_first]
        v_run = dl_run // 128
        np.add.at(cnt[c], (g_run, v_run), nsl)
        pc.append((sk, dlk, wk, rank, run_id, nsl, g_run, dl_run, v_run))

    # shared template: exact max-over-cores region capacity; region order is
    # (cohort, group, window) so each cohort's windows complete mid-layer;
    # every (cohort, group) segment is padded to whole instruction blocks
    cap = cnt.max(axis=0)  # [G, nwin] slots
    assert (cap.sum(axis=0) > 0).all(), "window with no edges"
    cohorts = np.array_split(np.arange(nwin), NCOH)
    off = np.zeros((G, nwin), np.int64)
    seg_bounds = []  # (start, end, g)
    b = 0
    for ch in range(NCOH):
        for g in range(G):
            s0 = b
            for v in cohorts[ch]:
                off[g, v] = b
                b += cap[g, v]
            b = -(-b // SPI) * SPI
            seg_bounds.append((s0, b, g))
    S_T = int(b)
    COLS = S_T // 128
    T = S_T * KSLOT
    KC = T // 128
    NI = S_T // SPI
    instr_group = np.zeros(NI, np.int64)
    for s0, s1, g in seg_bounds:
        instr_group[s0 // SPI:s1 // SPI] = g
    instr_group = [int(x) for x in instr_group]

    # window-major colwin template: for each window, its (col) list across
    # groups; complete_at[v] = instruction block that finishes its last region
    win_cols = [[] for _ in range(nwin)]
    for g in range(G):
        for v in range(nwin):
            if cap[g, v] == 0:
                continue
            lo = int(off[g, v])
            hi = lo + int(cap[g, v])
            for col in range(lo // 128, (hi + 127) // 128):
                if col not in win_cols[v]:
                    win_cols[v].append(col)
    complete_at = np.zeros(nwin, np.int64)
    for g in range(G):
        for v in range(nwin):
            if cap[g, v] > 0:
                last = int(off[g, v] + cap[g, v] - 1) // SPI
                complete_at[v] = max(complete_at[v], last)
    sched = [[] for _ in range(NI)]
    for v in range(nwin):
        sched[int(complete_at[v])].append(v)
    # window-major dloc2 column order
    cw_base = np.zeros(nwin + 1, np.int64)
    for v in range(nwin):
        cw_base[v + 1] = cw_base[v] + len(win_cols[v])
    NCW = int(cw_base[nwin])
    OHW = max(len(wc_) for wc_ in win_cols)

    inmaps = []
    for c in range(CORES):
        sk, dlk, wk, rank, run_id, nsl, g_run, dl_run, v_run = pc[c]
        n_runs = len(nsl)
        csum = np.cumsum(nsl)
        start_excl = np.r_[0, csum[:-1]]
        gv = g_run * nwin + v_run
        newgv = np.r_[True, gv[1:] != gv[:-1]]
        gv_first = np.flatnonzero(newgv)
        gv_id = np.cumsum(newgv) - 1
        base_in_gv = start_excl - start_excl[gv_first][gv_id]
        run_slot = off[g_run, v_run] + base_in_gv
        slot_e = run_slot[run_id] + rank // KSLOT
        j_e = rank % KSLOT
        # token position: instruction block ib, local slot ls ->
        #   kc = ib*GK + j*SUBK + ls//128, p = ls%128
        ib = slot_e // SPI
        ls = slot_e - ib * SPI
        kc_e = ib * GK + j_e * SUBK + ls // 128
        p_e = ls % 128
        tok = kc_e * 128 + p_e

        idx_flat = np.zeros(T, np.int16)
        idx_flat[tok] = sk.astype(np.int16)
        w_flat = np.zeros(T, np.float32)
        w_flat[tok] = wk

        dl_slot = np.full(S_T, -1, np.int64)
        reps = np.repeat(np.arange(n_runs), nsl)
        ar = np.arange(len(reps)) - np.repeat(start_excl, nsl)
        pos = np.repeat(run_slot, nsl) + ar
        dl_slot[pos] = np.repeat(dl_run, nsl)
        d2 = np.full((128, NCW), 512.0, np.float32)
        k = 0
        for v in range(nwin):
            for col in win_cols[v]:
                dcol = dl_slot[col * 128:(col + 1) * 128] - 128 * v
                d2[:, k] = np.where((dcol >= 0) & (dcol < 128), dcol, 512)
                k += 1
        inmaps.append(
            dict(
                gidx=_wrap16(idx_flat, T),
                wgrid=w_flat.reshape(KC, 128).T.astype(np.float16),
                dloc2=d2,
            )
        )

    meta = dict(S_T=S_T, COLS=COLS, T=T, KC=KC, NCW=NCW, NI=NI, OHW=OHW,
                win_cols=win_cols, cw_base=cw_base, sched=sched,
                instr_group=instr_group, geo=geo)
    return meta, inmaps


def emit_dma_gather(gp, out_ap, in_ap, idxs_ap, num_idxs, elem_size, elem_step,
                    queue_num=0):
    """bass.dma_gather minus the blanket 256B elem assert (verified on HW that
    the non-transpose ucode handles 128B rows)."""
    from concourse.bass import exact_div

    assert idxs_ap.dtype == mybir.dt.int16
    assert in_ap.dtype == out_ap.dtype
    assert in_ap.space == bass.MemorySpace.DRAM
    stride_bytes_256 = exact_div(elem_step * mybir.dt.size(in_ap.dtype), 256)
    _in_ap = gp.lower_ap_dma(in_ap, for_custom_bir_dma=True)
    _idxs_ap = gp.lower_ap(idxs_ap)
    _out_ap = gp.lower_ap(out_ap)
    return gp.add_instruction(
        mybir.InstDMAGatherAnt(
            name=gp.bass.get_next_instruction_name(),
            ins=[*_in_ap, _idxs_ap, gp.lower_val_access(gp.to_reg(num_idxs))],
            outs=[_out_ap],
            transpose=False,
            num_idxs=num_idxs,
            elem_size=elem_size,
            stride_bytes_256=stride_bytes_256,
            gen_mode=0,
            single_packet=False,
            queue_num=queue_num,
            sbuf_tokens_per_rank=0,
            sbuf_free_dim_per_rank=0,
            sbuf_free_dim_pad_per_rank=0,
            sbuf_byte_offset=0,
        )
    )


def _b(ap2, reps):
    return bass.AP(tensor=ap2.tensor, offset=ap2.offset, ap=[*ap2.ap, [0, reps]])


def _bcast_col(ap1, n):
    return bass.AP(tensor=ap1.tensor, offset=ap1.offset, ap=[ap1.ap[0], [0, n]])


def build(meta):
    geo: Geo = meta["geo"]
    S_T, COLS, T, KC, NCW, NI, OHW = (meta["S_T"], meta["COLS"], meta["T"],
                                      meta["KC"], meta["NCW"], meta["NI"],
                                      meta["OHW"])
    win_cols, cw_base, sched = meta["win_cols"], meta["cw_base"], meta["sched"]
    instr_group = meta["instr_group"]
    nsh, ntab, nwin, G = geo.nsh, geo.ntab, geo.nwin, geo.groups
    f32 = mybir.dt.float32
    f16 = mybir.dt.float16
    bf16 = mybir.dt.bfloat16
    AX = mybir.AxisListType.X
    AF = mybir.ActivationFunctionType

    nc = bacc.Bacc("TRN2", target_bir_lowering=False, debug=False,
                   num_devices=CORES, num_swdge_queues=4,
                   dynamic_dma_scratch_size=49152)

    xT = nc.dram_tensor("xT", [F_IN, nsh], f32, kind="ExternalInput")
    gidx = nc.dram_tensor("gidx", [128, T // 16], mybir.dt.int16,
                          kind="ExternalInput")
    wgrid = nc.dram_tensor("wgrid", [128, KC], mybir.dt.float16, kind="ExternalInput")
    dloc2 = nc.dram_tensor("dloc2", [128, NCW], f32, kind="ExternalInput")
    iota128 = nc.dram_tensor("iota128", [128, 128], f32, kind="ExternalInput")
    ident = nc.dram_tensor("ident", [128, 128], f32, kind="ExternalInput")
    W1t = nc.dram_tensor("W1t", [F_IN, F_HID], f32, kind="ExternalInput")
    b1t = nc.dram_tensor("b1t", [128, F_HID], f32, kind="ExternalInput")
    W2t = nc.dram_tensor("W2t", [F_HID, F_OUT], f32, kind="ExternalInput")
    b2t = nc.dram_tensor("b2t", [1, F_OUT], f32, kind="ExternalInput")
    onest = nc.dram_tensor("onest", [1, 128], f32, kind="ExternalInput")
    out_t = nc.dram_tensor("out", [nsh, F_OUT], f32, kind="ExternalOutput")

    with tile.TileContext(nc) as tc:
        with (
            tc.tile_pool(name="const", bufs=1) as cpool,
            tc.tile_pool(name="dram", bufs=1, space="DRAM") as dram,
            tc.tile_pool(name="gpool", bufs=3) as gp_,
            tc.tile_pool(name="mgpool", bufs=10) as mgp,
            tc.tile_pool(name="tpool", bufs=4) as tp_,
            tc.tile_pool(name="ohp", bufs=4) as ohp,
            tc.tile_pool(name="agg", bufs=1) as apool,
        ):
            iota_t = cpool.tile([128, 128], f32)
            nc.sync.dma_start(out=iota_t[:], in_=iota128[:, :])
            dloc_t = cpool.tile([128, NCW], f32)
            nc.sync.dma_start(out=dloc_t[:], in_=dloc2[:, :])
            W1s = cpool.tile([F_IN, F_HID], f32)
            nc.sync.dma_start(out=W1s[:], in_=W1t[:, :])
            b1s = cpool.tile([128, F_HID], f32)
            nc.sync.dma_start(out=b1s[:], in_=b1t[:, :])
            W2s = cpool.tile([F_HID, F_OUT], f32)
            nc.sync.dma_start(out=W2s[:], in_=W2t[:, :])
            b2s = cpool.tile([1, F_OUT], f32)
            nc.sync.dma_start(out=b2s[:], in_=b2t[:, :])
            ones_s = cpool.tile([1, 128], f32)
            nc.sync.dma_start(out=ones_s[:], in_=onest[:, :])
            id_s = cpool.tile([128, 128], f32)
            nc.sync.dma_start(out=id_s[:], in_=ident[:, :])

            shard1 = dram.tile([nsh, 128], f16)
            shard2 = dram.tile([nsh, 128], f16)
            table1 = dram.tile([ntab, 128], f16)
            table2 = dram.tile([ntab, 128], f16)

            # ---- h = x @ W1 on own shard -> shard1 ----
            with (
                tc.tile_pool(name="xt", bufs=1) as xp,
                tc.tile_pool(name="ph", bufs=2, space="PSUM") as ph,
            ):
                half = nsh // 2
                for hh in range(2):
                    xTs = xp.tile([F_IN, half], f32, tag="xts")
                    nc.sync.dma_start(out=xTs[:],
                                      in_=xT[:, hh * half:(hh + 1) * half])
                    for tt in range(half // 128):
                        hp = ph.tile([128, F_HID], f32, tag="hps")
                        nc.tensor.matmul(
                            out=hp[:], lhsT=xTs[:, tt * 128:(tt + 1) * 128],
                            rhs=W1s[:], start=True, stop=True,
                        )
                        hs = tp_.tile([128, 128], f16, tag="hsb")
                        nc.vector.memset(hs[:], 0.0)
                        nc.vector.tensor_copy(out=hs[:, :F_HID], in_=hp[:])
                        t = hh * (half // 128) + tt
                        dstp = bass.AP(
                            tensor=shard1.tensor, offset=t * 128 * 128,
                            ap=[[128, 128], [1, 128]],
                        )
                        nc.sync.dma_start(out=dstp, in_=hs[:])

            nc.gpsimd.collective_compute(
                "AllGather", mybir.AluOpType.bypass,
                ins=[shard1.opt()], outs=[table1.opt()],
                replica_groups=[list(range(CORES))],
            )

            CH = 8  # instruction blocks per input-prefetch chunk

            def run_layer(table, out1_all, close_fn, psw):
                gxc = wsc = None
                for ii in range(NI):
                    if ii % CH == 0:
                        nb_ = min(CH, NI - ii)
                        gxc = gp_.tile([128, CH * GK * 8], mybir.dt.int16,
                                       tag="gxc", name="gxc")
                        nc.sync.dma_start(
                            out=gxc[:, :nb_ * GK * 8],
                            in_=gidx[:, ii * GK * 8:(ii + nb_) * GK * 8])
                        wsc = gp_.tile([128, CH * GK], f16, tag="wsc",
                                       name="wsc")
                        nc.scalar.dma_start(
                            out=wsc[:, :nb_ * GK],
                            in_=wgrid[:, ii * GK:(ii + nb_) * GK])
                    g = instr_group[ii]
                    io = ii % CH
                    mg = mgp.tile([128, GK * 32], f16, tag="mg")
                    emit_dma_gather(
                        nc.gpsimd,
                        out_ap=mg[:].rearrange("p (k f) -> p k f", f=32),
                        in_ap=bass.AP(
                            tensor=table.tensor,
                            offset=g * geo.chunk * 128,
                            ap=[[128, geo.chunk], [1, 32]],
                        ),
                        idxs_ap=gxc[:, io * GK * 8:(io + 1) * GK * 8],
                        num_idxs=GK * 128,
                        elem_size=32,
                        elem_step=128,
                        queue_num=ii % 4,
                    )
                    nc.vector.tensor_tensor(
                        out=mg[:].rearrange("p (k f) -> p k f", f=32),
                        in0=mg[:].rearrange("p (k f) -> p k f", f=32),
                        in1=_b(wsc[:, io * GK:(io + 1) * GK], 32),
                        op=mybir.AluOpType.mult,
                    )
                    h1 = GK * 16
                    t1 = tp_.tile([128, GK * 16], f16, tag="t1")
                    nc.vector.tensor_tensor(out=t1[:], in0=mg[:, :h1],
                                            in1=mg[:, h1:2 * h1],
                                            op=mybir.AluOpType.add)
                    h2_ = GK * 8
                    nc.vector.tensor_tensor(
                        out=out1_all[:, ii * SPI // 128 * 32:
                                     (ii + 1) * SPI // 128 * 32],
                        in0=t1[:, :h2_], in1=t1[:, h2_:2 * h2_],
                        op=mybir.AluOpType.add,
                    )
                    for v in sched[ii]:
                        cols = win_cols[v]
                        nv = len(cols)
                        cw0 = int(cw_base[v])
                        oh = ohp.tile([128, OHW * 128], bf16, tag="oh")
                        nc.vector.tensor_tensor(
                            out=oh[:, :nv * 128].rearrange(
                                "p (b j) -> p b j", j=128),
                            in0=bass.AP(tensor=iota_t.tensor,
                                        offset=iota_t[:].offset,
                                        ap=[iota_t[:].ap[0], [0, nv], [1, 128]]),
                            in1=bass.AP(tensor=dloc_t.tensor,
                                        offset=dloc_t[:].offset + cw0,
                                        ap=[dloc_t[:].ap[0], [1, nv], [0, 128]]),
                            op=mybir.AluOpType.is_equal,
                        )
                        pwin = psw.tile([128, 512], f32, tag="pwin")
                        for i, col in enumerate(cols):
                            nc.tensor.matmul(
                                out=pwin[:, 0:F_HID],
                                lhsT=oh[:, i * 128:(i + 1) * 128],
                                rhs=out1_all[:, col * 32:(col + 1) * 32],
                                start=(i == 0),
                                stop=(i == nv - 1),
                            )
                        close_fn(v, pwin)

            # ---- layer 1 ----
            l1cm = tc.tile_pool(name="l1s", bufs=1)
            l1pool = l1cm.__enter__()
            out1a = l1pool.tile([128, COLS * 32], bf16, tag="o1a")
            h2 = l1pool.tile([128, nwin * 128], f16, tag="h2")
            nc.vector.memset(h2[:], 0.0)

            def close1(v, pwin):
                tmp = tp_.tile([128, F_HID], f32, tag="clo")
                nc.vector.tensor_tensor(
                    out=tmp[:], in0=pwin[:, 0:F_HID], in1=b1s[:],
                    op=mybir.AluOpType.add,
                )
                nc.scalar.activation(
                    out=h2[:, v * 128:v * 128 + F_HID], in_=tmp[:],
                    func=AF.Relu,
                )

            psw_cm = tc.tile_pool(name="psw", bufs=4, space="PSUM")
            psw = psw_cm.__enter__()
            run_layer(table1, out1a, close1, psw)
            psw_cm.__exit__(None, None, None)
            dst2 = bass.AP(tensor=shard2.tensor, offset=0,
                           ap=[[128, 128], [128 * 128, nwin], [1, 128]])
            nc.sync.dma_start(out=dst2, in_=h2[:].rearrange(
                "p (v f) -> p v f", f=128))
            l1cm.__exit__(None, None, None)
            nc.gpsimd.collective_compute(
                "AllGather", mybir.AluOpType.bypass,
                ins=[shard2.opt()], outs=[table2.opt()],
                replica_groups=[list(range(CORES))],
            )

            # ---- layer 2 (close folds in W2 + bias + exp/max pieces) ----
            l2cm = tc.tile_pool(name="l2s", bufs=1)
            l2pool = l2cm.__enter__()
            out1b = l2pool.tile([128, COLS * 32], bf16, tag="o1b")
            zall = apool.tile([128, nwin * F_OUT], f32, tag="zall")
            sall = apool.tile([128, nwin], f32, tag="sall")
            pf_cm = tc.tile_pool(name="pf", bufs=2, space="PSUM")
            pf = pf_cm.__enter__()

            def close2(v, pwin):
                ag = ohp.tile([128, F_HID], f32, tag="ag")
                nc.vector.tensor_copy(out=ag[:], in_=pwin[:, 0:F_HID])
                tp2 = pf.tile([F_HID, 128], f32, tag="tp")
                nc.tensor.transpose(out=tp2[:], in_=ag[:], identity=id_s[:])
                aT = ohp.tile([F_HID, 128], f32, tag="aT")
                nc.vector.tensor_copy(out=aT[:], in_=tp2[:])
                zp = pf.tile([128, F_OUT], f32, tag="zp")
                nc.tensor.matmul(out=zp[:], lhsT=aT[:], rhs=W2s[:],
                                 start=True, stop=False)
                nc.tensor.matmul(out=zp[:], lhsT=ones_s[:], rhs=b2s[:],
                                 start=False, stop=True)
                negm = ohp.tile([128, 1], f32, tag="negm")
                nc.vector.reduce_max(out=negm[:], in_=zp[:], axis=AX,
                                     negate=True)
                nc.vector.tensor_tensor(
                    out=zall[:, v * F_OUT:(v + 1) * F_OUT],
                    in0=zp[:], in1=_bcast_col(negm[:], F_OUT),
                    op=mybir.AluOpType.add,
                )
                etmp = ohp.tile([128, F_OUT], f32, tag="etmp")
                nc.scalar.activation(
                    out=etmp[:], in_=zall[:, v * F_OUT:(v + 1) * F_OUT],
                    func=AF.Exp, accum_out=sall[:, v:v + 1],
                )

            psw_cm2 = tc.tile_pool(name="psw2", bufs=4, space="PSUM")
            psw2 = psw_cm2.__enter__()
            run_layer(table2, out1b, close2, psw2)
            psw_cm2.__exit__(None, None, None)
            l2cm.__exit__(None, None, None)

            lns = apool.tile([128, nwin], f32, tag="lns")
            nc.scalar.activation(out=lns[:], in_=sall[:], func=AF.Ln)
            for v in range(nwin):
                nc.vector.tensor_tensor(
                    out=zall[:, v * F_OUT:(v + 1) * F_OUT],
                    in0=zall[:, v * F_OUT:(v + 1) * F_OUT],
                    in1=_bcast_col(lns[:, v:v + 1], F_OUT),
                    op=mybir.AluOpType.subtract,
                )
            outdst = bass.AP(
                tensor=out_t, offset=0,
                ap=[[F_OUT, 128], [128 * F_OUT, nwin], [1, F_OUT]],
            )
            nc.sync.dma_start(out=outdst, in_=zall[:].rearrange(
                "p (v f) -> p v f", f=F_OUT))
            pf_cm.__exit__(None, None, None)

    nc.compile()
    return nc


def make_inmaps(meta, inmaps_edges, x, W1, b1, W2, b2):
    geo: Geo = meta["geo"]
    nsh = geo.nsh
    n = geo.n_nodes
    xT_full = np.zeros((F_IN, geo.ntab), np.float32)
    xT_full[:, :n] = np.asarray(x, np.float32).T
    iota = np.tile(np.arange(128, dtype=np.float32)[None, :], (128, 1))
    ident = np.eye(128, dtype=np.float32)
    b1b = np.tile(np.asarray(b1, np.float32)[None, :], (128, 1))
    consts = dict(
        iota128=iota, ident=ident,
        W1t=np.asarray(W1, np.float32), b1t=b1b,
        W2t=np.asarray(W2, np.float32), b2t=np.asarray(b2, np.float32)[None, :],
        onest=np.ones((1, 128), np.float32),
    )
    maps = []
    for c in range(CORES):
        m = dict(inmaps_edges[c])
        m.update(consts)
        m["xT"] = np.ascontiguousarray(xT_full[:, c * nsh:(c + 1) * nsh])
        maps.append(m)
    return maps


_CACHE = {}


def run(x, edge_index, edge_weight, W1, b1, W2, b2, geo=FULL, trace=False):
    key = "geo%d" % geo.n_nodes
    meta, inmaps_edges = pack(edge_index, edge_weight, geo)
    if key in _CACHE:
        nc = _CACHE[key]
    else:
        nc = build(meta)
        _CACHE[key] = nc
    maps = make_inmaps(meta, inmaps_edges, x, W1, b1, W2, b2)
    res = run_bass_kernel_spmd(nc, maps, core_ids=list(range(CORES)), trace=trace)
    n = geo.n_nodes
    out = np.empty((n, F_OUT), np.float32)
    for c in range(CORES):
        lo = c * geo.nsh
        hi = min(lo + geo.nsh, n)
        if hi > lo:
            out[lo:hi] = res.results[c]["out"][: hi - lo]
    return out, res


def kernel(x, edge_index, edge_weight, W1, b1, W2, b2):
    out, _ = run(
        np.asarray(x), np.asarray(edge_index), np.asarray(edge_weight),
        np.asarray(W1), np.asarray(b1), np.asarray(W2), np.asarray(b2),
    )
    return out


# revision 20
# speedup vs baseline: 4.9731x; 1.2016x over previous
"""GCN (2-layer, edge-weighted, log_softmax) on 8 Trainium2 NeuronCores.

v2 strategy (dst-sharded edges, matmul segment-sum, 4-queue SWDGE gather):
  - Nodes sharded 12544/core; fp32 table rows (256B-strided, 32 used) in HBM,
    replicated via AllGather.
  - Edges packed 4-per-slot by (src-chunk, dst). Each 2048-slot instruction
    block issues FOUR 2048-token gathers (one per slot position j) on SWDGE
    queues 0..3 -> concurrent DMA transfers into four separate tiles, so the
    weighted 4->1 reduction is 4 mults + 3 adds, all contiguous two-tile ops.
  - Slot sums (bf16) for the whole layer stay resident in SBUF (out1_all).
  - Scatter runs window-major: once the last instruction touching window v
    completes, a one-shot is_equal builds all of v's one-hot columns (host
    dloc2 template, window-major order), and ~10 bf16 matmuls accumulate
    into a dedicated full PSUM bank (start/stop per window; 4 banks rotate).
    Window closes immediately (bias+relu -> table2 shard, or copy to agg2).
  - Layer 2 reuses the identical edge template on table2 = relu(agg1+b1),
    then W2 + log_softmax per 128-node window.
Host side packs indices/weights (numpy) and concatenates shards.
"""

import os
import sys

for _p in ("/opt/trn_rl_repo", "/root/.axon_site/_ro/trn_rl_repo"):
    if os.path.isdir(_p) and _p not in sys.path:
        sys.path.insert(0, _p)

import numpy as np

import concourse.ap_utils as ap_utils
import concourse.bass as bass
import concourse.mybir as mybir
from concourse import bacc, tile
from concourse.bass_utils import run_bass_kernel_spmd

CORES = 8
F_IN = 128
F_HID = 32
F_OUT = 40
KSLOT = 3      # edges per slot (same destination)
GK = 24        # k-columns per instruction block (3072 tokens, 1024 slots)
NCOH = 4       # window cohorts (striped region order for scatter overlap)
SPI = GK * 128 // KSLOT   # slots per instruction block (2048)
SUBK = GK // KSLOT        # k-columns per sub-gather (16 -> 2048 tokens)


class Geo:
    def __init__(self, n_nodes=100000, nsh=12544, chunk=25088, groups=4):
        self.n_nodes = n_nodes
        self.nsh = nsh
        self.ntab = nsh * CORES
        self.chunk = chunk
        self.groups = groups
        assert chunk * groups == self.ntab
        assert nsh % 128 == 0
        self.nwin = nsh // 128


FULL = Geo()


def _wrap16(flat, T):
    a = flat.reshape(T // 16, 16).T
    return np.tile(a, (8, 1)).copy()


def pack(edge_index, edge_weight, geo: Geo):
    src = np.asarray(edge_index[0], dtype=np.int64)
    dst = np.asarray(edge_index[1], dtype=np.int64)
    w = np.asarray(edge_weight, dtype=np.float32)
    nsh, nwin, G = geo.nsh, geo.nwin, geo.groups

    core = dst // nsh
    pc = []
    cnt = np.zeros((CORES, G, nwin), np.int64)
    for c in range(CORES):
        m = core == c
        s_c = src[m]
        dl = dst[m] - c * nsh
        wc = w[m]
        g = s_c // geo.chunk
        key = g * (2 * nsh) + dl
        order = np.argsort(key, kind="stable")
        sk = (s_c - g * geo.chunk)[order]
        dlk = dl[order]
        wk = wc[order]
        kk = key[order]
        new = np.r_[True, kk[1:] != kk[:-1]]
        run_first = np.flatnonzero(new)
        run_len = np.diff(np.r_[run_first, len(kk)])
        run_id = np.cumsum(new) - 1
        rank = np.arange(len(kk)) - run_first[run_id]
        nsl = (run_len + KSLOT - 1) // KSLOT
        g_run = (g[order])[run_first]
        dl_run = dlk[run_first]
        v_run = dl_run // 128
        np.add.at(cnt[c], (g_run, v_run), nsl)
        pc.append((sk, dlk, wk, rank, run_id, nsl, g_run, dl_run, v_run))

    # shared template: exact max-over-cores region capacity; region order is
    # (cohort, group, window) so each cohort's windows complete mid-layer;
    # every (cohort, group) segment is padded to whole instruction blocks
    cap = cnt.max(axis=0)  # [G, nwin] slots
    assert (cap.sum(axis=0) > 0).all(), "window with no edges"
    cohorts = np.array_split(np.arange(nwin), NCOH)
    off = np.zeros((G, nwin), np.int64)
    seg_bounds = []  # (start, end, g)
    b = 0
    for ch in range(NCOH):
        for g in range(G):
            s0 = b
            for v in cohorts[ch]:
                off[g, v] = b
                b += cap[g, v]
            b = -(-b // SPI) * SPI
            seg_bounds.append((s0, b, g))
    S_T = int(b)
    COLS = S_T // 128
    T = S_T * KSLOT
    KC = T // 128
    NI = S_T // SPI
    instr_group = np.zeros(NI, np.int64)
    for s0, s1, g in seg_bounds:
        instr_group[s0 // SPI:s1 // SPI] = g
    instr_group = [int(x) for x in instr_group]

    # window-major colwin template: for each window, its (col) list across
    # groups; complete_at[v] = instruction block that finishes its last region
    win_cols = [[] for _ in range(nwin)]
    for g in range(G):
        for v in range(nwin):
            if cap[g, v] == 0:
                continue
            lo = int(off[g, v])
            hi = lo + int(cap[g, v])
            for col in range(lo // 128, (hi + 127) // 128):
                if col not in win_cols[v]:
                    win_cols[v].append(col)
    complete_at = np.zeros(nwin, np.int64)
    for g in range(G):
        for v in range(nwin):
            if cap[g, v] > 0:
                last = int(off[g, v] + cap[g, v] - 1) // SPI
                complete_at[v] = max(complete_at[v], last)
    sched = [[] for _ in range(NI)]
    for v in range(nwin):
        sched[int(complete_at[v])].append(v)
    # window-major dloc2 column order
    cw_base = np.zeros(nwin + 1, np.int64)
    for v in range(nwin):
        cw_base[v + 1] = cw_base[v] + len(win_cols[v])
    NCW = int(cw_base[nwin])
    OHW = max(len(wc_) for wc_ in win_cols)

    inmaps = []
    for c in range(CORES):
        sk, dlk, wk, rank, run_id, nsl, g_run, dl_run, v_run = pc[c]
        n_runs = len(nsl)
        csum = np.cumsum(nsl)
        start_excl = np.r_[0, csum[:-1]]
        gv = g_run * nwin + v_run
        newgv = np.r_[True, gv[1:] != gv[:-1]]
        gv_first = np.flatnonzero(newgv)
        gv_id = np.cumsum(newgv) - 1
        base_in_gv = start_excl - start_excl[gv_first][gv_id]
        run_slot = off[g_run, v_run] + base_in_gv
        slot_e = run_slot[run_id] + rank // KSLOT
        j_e = rank % KSLOT
        # token position: instruction block ib, local slot ls ->
        #   kc = ib*GK + j*SUBK + ls//128, p = ls%128
        ib = slot_e // SPI
        ls = slot_e - ib * SPI
        kc_e = ib * GK + j_e * SUBK + ls // 128
        p_e = ls % 128
        tok = kc_e * 128 + p_e

        idx_flat = np.zeros(T, np.int16)
        idx_flat[tok] = sk.astype(np.int16)
        w_flat = np.zeros(T, np.float32)
        w_flat[tok] = wk

        dl_slot = np.full(S_T, -1, np.int64)
        reps = np.repeat(np.arange(n_runs), nsl)
        ar = np.arange(len(reps)) - np.repeat(start_excl, nsl)
        pos = np.repeat(run_slot, nsl) + ar
        dl_slot[pos] = np.repeat(dl_run, nsl)
        d2 = np.full((128, NCW), 512.0, np.float32)
        k = 0
        for v in range(nwin):
            for col in win_cols[v]:
                dcol = dl_slot[col * 128:(col + 1) * 128] - 128 * v
                d2[:, k] = np.where((dcol >= 0) & (dcol < 128), dcol, 512)
                k += 1
        inmaps.append(
            dict(
                gidx=_wrap16(idx_flat, T),
                wgrid=w_flat.reshape(KC, 128).T.astype(np.float16),
                dloc2=d2,
            )
        )

    meta = dict(S_T=S_T, COLS=COLS, T=T, KC=KC, NCW=NCW, NI=NI, OHW=OHW,
                win_cols=win_cols, cw_base=cw_base, sched=sched,
                instr_group=instr_group, geo=geo)
    return meta, inmaps


def emit_dma_gather(gp, out_ap, in_ap, idxs_ap, num_idxs, elem_size, elem_step,
                    queue_num=0):
    """bass.dma_gather minus the blanket 256B elem assert (verified on HW that
    the non-transpose ucode handles 128B rows)."""
    from concourse.bass import exact_div

    assert idxs_ap.dtype == mybir.dt.int16
    assert in_ap.dtype == out_ap.dtype
    assert in_ap.space == bass.MemorySpace.DRAM
    stride_bytes_256 = exact_div(elem_step * mybir.dt.size(in_ap.dtype), 256)
    _in_ap = gp.lower_ap_dma(in_ap, for_custom_bir_dma=True)
    _idxs_ap = gp.lower_ap(idxs_ap)
    _out_ap = gp.lower_ap(out_ap)
    return gp.add_instruction(
        mybir.InstDMAGatherAnt(
            name=gp.bass.get_next_instruction_name(),
            ins=[*_in_ap, _idxs_ap, gp.lower_val_access(gp.to_reg(num_idxs))],
            outs=[_out_ap],
            transpose=False,
            num_idxs=num_idxs,
            elem_size=elem_size,
            stride_bytes_256=stride_bytes_256,
            gen_mode=0,
            single_packet=False,
            queue_num=queue_num,
            sbuf_tokens_per_rank=0,
            sbuf_free_dim_per_rank=0,
            sbuf_free_dim_pad_per_rank=0,
            sbuf_byte_offset=0,
        )
    )


def _b(ap2, reps):
    return bass.AP(tensor=ap2.tensor, offset=ap2.offset, ap=[*ap2.ap, [0, reps]])


def _bcast_col(ap1, n):
    return bass.AP(tensor=ap1.tensor, offset=ap1.offset, ap=[ap1.ap[0], [0, n]])


def build(meta):
    geo: Geo = meta["geo"]
    S_T, COLS, T, KC, NCW, NI, OHW = (meta["S_T"], meta["COLS"], meta["T"],
                                      meta["KC"], meta["NCW"], meta["NI"],
                                      meta["OHW"])
    win_cols, cw_base, sched = meta["win_cols"], meta["cw_base"], meta["sched"]
    instr_group = meta["instr_group"]
    nsh, ntab, nwin, G = geo.nsh, geo.ntab, geo.nwin, geo.groups
    f32 = mybir.dt.float32
    f16 = mybir.dt.float16
    bf16 = mybir.dt.bfloat16
    AX = mybir.AxisListType.X
    AF = mybir.ActivationFunctionType

    nc = bacc.Bacc("TRN2", target_bir_lowering=False, debug=False,
                   num_devices=CORES, num_swdge_queues=4,
                   dynamic_dma_scratch_size=49152)

    xT = nc.dram_tensor("xT", [F_IN, nsh], f32, kind="ExternalInput")
    gidx = nc.dram_tensor("gidx", [128, T // 16], mybir.dt.int16,
                          kind="ExternalInput")
    wgrid = nc.dram_tensor("wgrid", [128, KC], mybir.dt.float16, kind="ExternalInput")
    dloc2 = nc.dram_tensor("dloc2", [128, NCW], f32, kind="ExternalInput")
    iota128 = nc.dram_tensor("iota128", [128, 128], f32, kind="ExternalInput")
    ident = nc.dram_tensor("ident", [128, 128], f32, kind="ExternalInput")
    W1t = nc.dram_tensor("W1t", [F_IN, F_HID], f32, kind="ExternalInput")
    b1t = nc.dram_tensor("b1t", [128, F_HID], f32, kind="ExternalInput")
    W2t = nc.dram_tensor("W2t", [F_HID, F_OUT], f32, kind="ExternalInput")
    b2t = nc.dram_tensor("b2t", [1, F_OUT], f32, kind="ExternalInput")
    onest = nc.dram_tensor("onest", [1, 128], f32, kind="ExternalInput")
    out_t = nc.dram_tensor("out", [nsh, F_OUT], f32, kind="ExternalOutput")

    with tile.TileContext(nc) as tc:
        with (
            tc.tile_pool(name="const", bufs=1) as cpool,
            tc.tile_pool(name="dram", bufs=1, space="DRAM") as dram,
            tc.tile_pool(name="gpool", bufs=3) as gp_,
            tc.tile_pool(name="mgpool", bufs=8) as mgp,
            tc.tile_pool(name="tpool", bufs=4) as tp_,
            tc.tile_pool(name="ohp", bufs=4) as ohp,
            tc.tile_pool(name="agg", bufs=1) as apool,
        ):
            iota_t = cpool.tile([128, 128], f32)
            nc.sync.dma_start(out=iota_t[:], in_=iota128[:, :])
            dloc_t = cpool.tile([128, NCW], f32)
            nc.sync.dma_start(out=dloc_t[:], in_=dloc2[:, :])
            W1s = cpool.tile([F_IN, F_HID], f32)
            nc.sync.dma_start(out=W1s[:], in_=W1t[:, :])
            b1s = cpool.tile([128, F_HID], f32)
            nc.sync.dma_start(out=b1s[:], in_=b1t[:, :])
            W2s = cpool.tile([F_HID, F_OUT], f32)
            nc.sync.dma_start(out=W2s[:], in_=W2t[:, :])
            b2s = cpool.tile([1, F_OUT], f32)
            nc.sync.dma_start(out=b2s[:], in_=b2t[:, :])
            ones_s = cpool.tile([1, 128], f32)
            nc.sync.dma_start(out=ones_s[:], in_=onest[:, :])
            id_s = cpool.tile([128, 128], f32)
            nc.sync.dma_start(out=id_s[:], in_=ident[:, :])

            shard1 = dram.tile([nsh, 128], f16)
            shard2 = dram.tile([nsh, 128], f16)
            table1 = dram.tile([ntab, 128], f16)
            table2 = dram.tile([ntab, 128], f16)

            # ---- h = x @ W1 on own shard -> shard1 ----
            with (
                tc.tile_pool(name="xt", bufs=1) as xp,
                tc.tile_pool(name="ph", bufs=2, space="PSUM") as ph,
            ):
                half = nsh // 2
                for hh in range(2):
                    xTs = xp.tile([F_IN, half], f32, tag="xts")
                    nc.sync.dma_start(out=xTs[:],
                                      in_=xT[:, hh * half:(hh + 1) * half])
                    for tt in range(half // 128):
                        hp = ph.tile([128, F_HID], f32, tag="hps")
                        nc.tensor.matmul(
                            out=hp[:], lhsT=xTs[:, tt * 128:(tt + 1) * 128],
                            rhs=W1s[:], start=True, stop=True,
                        )
                        hs = tp_.tile([128, 128], f16, tag="hsb")
                        nc.vector.memset(hs[:], 0.0)
                        nc.vector.tensor_copy(out=hs[:, :F_HID], in_=hp[:])
                        t = hh * (half // 128) + tt
                        dstp = bass.AP(
                            tensor=shard1.tensor, offset=t * 128 * 128,
                            ap=[[128, 128], [1, 128]],
                        )
                        nc.sync.dma_start(out=dstp, in_=hs[:])

            nc.gpsimd.collective_compute(
                "AllGather", mybir.AluOpType.bypass,
                ins=[shard1.opt()], outs=[table1.opt()],
                replica_groups=[list(range(CORES))],
            )

            CH = 8  # instruction blocks per input-prefetch chunk

            def run_layer(table, out1_all, close_fn, psw):
                gxc = wsc = None
                for ii in range(NI):
                    if ii % CH == 0:
                        nb_ = min(CH, NI - ii)
                        gxc = gp_.tile([128, CH * GK * 8], mybir.dt.int16,
                                       tag="gxc", name="gxc")
                        nc.sync.dma_start(
                            out=gxc[:, :nb_ * GK * 8],
                            in_=gidx[:, ii * GK * 8:(ii + nb_) * GK * 8])
                        wsc = gp_.tile([128, CH * GK], f16, tag="wsc",
                                       name="wsc")
                        nc.scalar.dma_start(
                            out=wsc[:, :nb_ * GK],
                            in_=wgrid[:, ii * GK:(ii + nb_) * GK])
                    g = instr_group[ii]
                    io = ii % CH
                    mg = mgp.tile([128, GK * 32], f16, tag="mg")
                    emit_dma_gather(
                        nc.gpsimd,
                        out_ap=mg[:].rearrange("p (k f) -> p k f", f=32),
                        in_ap=bass.AP(
                            tensor=table.tensor,
                            offset=g * geo.chunk * 128,
                            ap=[[128, geo.chunk], [1, 32]],
                        ),
                        idxs_ap=gxc[:, io * GK * 8:(io + 1) * GK * 8],
                        num_idxs=GK * 128,
                        elem_size=32,
                        elem_step=128,
                        queue_num=ii % 4,
                    )
                    nc.vector.tensor_tensor(
                        out=mg[:].rearrange("p (k f) -> p k f", f=32),
                        in0=mg[:].rearrange("p (k f) -> p k f", f=32),
                        in1=_b(wsc[:, io * GK:(io + 1) * GK], 32),
                        op=mybir.AluOpType.mult,
                    )
                    h1 = SUBK * 32
                    t1 = tp_.tile([128, SUBK * 32], f16, tag="t1")
                    nc.vector.tensor_tensor(out=t1[:], in0=mg[:, :h1],
                                            in1=mg[:, h1:2 * h1],
                                            op=mybir.AluOpType.add)
                    nc.vector.tensor_tensor(
                        out=out1_all[:, ii * SPI // 128 * 32:
                                     (ii + 1) * SPI // 128 * 32],
                        in0=t1[:], in1=mg[:, 2 * h1:3 * h1],
                        op=mybir.AluOpType.add,
                    )
                    for v in sched[ii]:
                        cols = win_cols[v]
                        nv = len(cols)
                        cw0 = int(cw_base[v])
                        oh = ohp.tile([128, OHW * 128], bf16, tag="oh")
                        nc.vector.tensor_tensor(
                            out=oh[:, :nv * 128].rearrange(
                                "p (b j) -> p b j", j=128),
                            in0=bass.AP(tensor=iota_t.tensor,
                                        offset=iota_t[:].offset,
                                        ap=[iota_t[:].ap[0], [0, nv], [1, 128]]),
                            in1=bass.AP(tensor=dloc_t.tensor,
                                        offset=dloc_t[:].offset + cw0,
                                        ap=[dloc_t[:].ap[0], [1, nv], [0, 128]]),
                            op=mybir.AluOpType.is_equal,
                        )
                        pwin = psw.tile([128, 512], f32, tag="pwin")
                        for i, col in enumerate(cols):
                            nc.tensor.matmul(
                                out=pwin[:, 0:F_HID],
                                lhsT=oh[:, i * 128:(i + 1) * 128],
                                rhs=out1_all[:, col * 32:(col + 1) * 32],
                                start=(i == 0),
                                stop=(i == nv - 1),
                            )
                        close_fn(v, pwin)

            # ---- layer 1 ----
            l1cm = tc.tile_pool(name="l1s", bufs=1)
            l1pool = l1cm.__enter__()
            out1a = l1pool.tile([128, COLS * 32], bf16, tag="o1a")
            h2 = l1pool.tile([128, nwin * 128], f16, tag="h2")
            nc.vector.memset(h2[:], 0.0)

            def close1(v, pwin):
                tmp = tp_.tile([128, F_HID], f32, tag="clo")
                nc.vector.tensor_tensor(
                    out=tmp[:], in0=pwin[:, 0:F_HID], in1=b1s[:],
                    op=mybir.AluOpType.add,
                )
                nc.scalar.activation(
                    out=h2[:, v * 128:v * 128 + F_HID], in_=tmp[:],
                    func=AF.Relu,
                )

            psw_cm = tc.tile_pool(name="psw", bufs=4, space="PSUM")
            psw = psw_cm.__enter__()
            run_layer(table1, out1a, close1, psw)
            psw_cm.__exit__(None, None, None)
            dst2 = bass.AP(tensor=shard2.tensor, offset=0,
                           ap=[[128, 128], [128 * 128, nwin], [1, 128]])
            nc.sync.dma_start(out=dst2, in_=h2[:].rearrange(
                "p (v f) -> p v f", f=128))
            l1cm.__exit__(None, None, None)
            nc.gpsimd.collective_compute(
                "AllGather", mybir.AluOpType.bypass,
                ins=[shard2.opt()], outs=[table2.opt()],
                replica_groups=[list(range(CORES))],
            )

            # ---- layer 2 (close folds in W2 + bias + exp/max pieces) ----
            l2cm = tc.tile_pool(name="l2s", bufs=1)
            l2pool = l2cm.__enter__()
            out1b = l2pool.tile([128, COLS * 32], bf16, tag="o1b")
            zall = apool.tile([128, nwin * F_OUT], f32, tag="zall")
            sall = apool.tile([128, nwin], f32, tag="sall")
            pf_cm = tc.tile_pool(name="pf", bufs=2, space="PSUM")
            pf = pf_cm.__enter__()

            def close2(v, pwin):
                ag = ohp.tile([128, F_HID], f32, tag="ag")
                nc.vector.tensor_copy(out=ag[:], in_=pwin[:, 0:F_HID])
                tp2 = pf.tile([F_HID, 128], f32, tag="tp")
                nc.tensor.transpose(out=tp2[:], in_=ag[:], identity=id_s[:])
                aT = ohp.tile([F_HID, 128], f32, tag="aT")
                nc.vector.tensor_copy(out=aT[:], in_=tp2[:])
                zp = pf.tile([128, F_OUT], f32, tag="zp")
                nc.tensor.matmul(out=zp[:], lhsT=aT[:], rhs=W2s[:],
                                 start=True, stop=False)
                nc.tensor.matmul(out=zp[:], lhsT=ones_s[:], rhs=b2s[:],
                                 start=False, stop=True)
                negm = ohp.tile([128, 1], f32, tag="negm")
                nc.vector.reduce_max(out=negm[:], in_=zp[:], axis=AX,
                                     negate=True)
                nc.vector.tensor_tensor(
                    out=zall[:, v * F_OUT:(v + 1) * F_OUT],
                    in0=zp[:], in1=_bcast_col(negm[:], F_OUT),
                    op=mybir.AluOpType.add,
                )
                etmp = ohp.tile([128, F_OUT], f32, tag="etmp")
                nc.scalar.activation(
                    out=etmp[:], in_=zall[:, v * F_OUT:(v + 1) * F_OUT],
                    func=AF.Exp, accum_out=sall[:, v:v + 1],
                )

            psw_cm2 = tc.tile_pool(name="psw2", bufs=4, space="PSUM")
            psw2 = psw_cm2.__enter__()
            run_layer(table2, out1b, close2, psw2)
            psw_cm2.__exit__(None, None, None)
            l2cm.__exit__(None, None, None)

            lns = apool.tile([128, nwin], f32, tag="lns")
            nc.scalar.activation(out=lns[:], in_=sall[:], func=AF.Ln)
            for v in range(nwin):
                nc.vector.tensor_tensor(
                    out=zall[:, v * F_OUT:(v + 1) * F_OUT],
                    in0=zall[:, v * F_OUT:(v + 1) * F_OUT],
                    in1=_bcast_col(lns[:, v:v + 1], F_OUT),
                    op=mybir.AluOpType.subtract,
                )
            outdst = bass.AP(
                tensor=out_t, offset=0,
                ap=[[F_OUT, 128], [128 * F_OUT, nwin], [1, F_OUT]],
            )
            nc.sync.dma_start(out=outdst, in_=zall[:].rearrange(
                "p (v f) -> p v f", f=F_OUT))
            pf_cm.__exit__(None, None, None)

    nc.compile()
    return nc


def make_inmaps(meta, inmaps_edges, x, W1, b1, W2, b2):
    geo: Geo = meta["geo"]
    nsh = geo.nsh
    n = geo.n_nodes
    xT_full = np.zeros((F_IN, geo.ntab), np.float32)
    xT_full[:, :n] = np.asarray(x, np.float32).T
    iota = np.tile(np.arange(128, dtype=np.float32)[None, :], (128, 1))
    ident = np.eye(128, dtype=np.float32)
    b1b = np.tile(np.asarray(b1, np.float32)[None, :], (128, 1))
    consts = dict(
        iota128=iota, ident=ident,
        W1t=np.asarray(W1, np.float32), b1t=b1b,
        W2t=np.asarray(W2, np.float32), b2t=np.asarray(b2, np.float32)[None, :],
        onest=np.ones((1, 128), np.float32),
    )
    maps = []
    for c in range(CORES):
        m = dict(inmaps_edges[c])
        m.update(consts)
        m["xT"] = np.ascontiguousarray(xT_full[:, c * nsh:(c + 1) * nsh])
        maps.append(m)
    return maps


_CACHE = {}


def run(x, edge_index, edge_weight, W1, b1, W2, b2, geo=FULL, trace=False):
    key = "geo%d" % geo.n_nodes
    meta, inmaps_edges = pack(edge_index, edge_weight, geo)
    if key in _CACHE:
        nc = _CACHE[key]
    else:
        nc = build(meta)
        _CACHE[key] = nc
    maps = make_inmaps(meta, inmaps_edges, x, W1, b1, W2, b2)
    res = run_bass_kernel_spmd(nc, maps, core_ids=list(range(CORES)), trace=trace)
    n = geo.n_nodes
    out = np.empty((n, F_OUT), np.float32)
    for c in range(CORES):
        lo = c * geo.nsh
        hi = min(lo + geo.nsh, n)
        if hi > lo:
            out[lo:hi] = res.results[c]["out"][: hi - lo]
    return out, res


def kernel(x, edge_index, edge_weight, W1, b1, W2, b2):
    out, _ = run(
        np.asarray(x), np.asarray(edge_index), np.asarray(edge_weight),
        np.asarray(W1), np.asarray(b1), np.asarray(W2), np.asarray(b2),
    )
    return out
